# revision 1
# baseline (speedup 1.0000x reference)
"""Gemma3 decoder layer on 8 Trainium2 NeuronCores (Bass/Tile), v2.

Sharding (per core c):
  - attention: SEQUENCE-parallel. Core c owns tokens [256c, 256c+256) and
    receives a 512-token halo (host-side sharding, zero communication):
    x_halo = x[256c-512 : 256c+256] (zero-padded for c<2). All attn weights
    (wq/wk/wv/wo) replicated; K/V computed for all 768 local tokens, Q only
    for the 256 own tokens. Sliding-window (512) attention is then fully
    local. Pad keys are masked via a per-core pad mask input.
  - MLP: tensor-parallel. gate/up column-sharded (1280 cols), down
    row-sharded. h2 (post-attn, pre-ff normed) is AllGathered transposed in
    token chunks (16,16,32x7 own tokens -> 128..256-token gathered chunks),
    gate/up/down run per gathered chunk, partial down outputs
    ReduceScattered in 4 x 64-own-token groups. Collectives pipeline under
    the MLP matmul stream; small head chunks start the pipeline early.
  - norms/residual: token-local. Weights are preloaded into SBUF in phases
    (wk before S1; wo+wg during attention; wu/wd after) sized to the
    ~207KB/partition SBUF budget, in <=2.6MB DMA pieces so the scheduler
    cannot starve critical loads on the DMA queue.
Matmuls in bf16 (fp32 PSUM accumulation); norms/softmax/residual fp32.
All weights host-prepacked into SBUF layout (single contiguous DMAs).
"""
import sys

if "/opt/trn_rl_repo" not in sys.path:
    sys.path.insert(0, "/opt/trn_rl_repo")

import numpy as np
import ml_dtypes

import concourse.bass as bass
import concourse.mybir as mybir
import concourse.tile as tile
from concourse import bacc
from concourse.bass_utils import run_bass_kernel_spmd
from concourse.masks import make_identity

dt = mybir.dt
AF = mybir.ActivationFunctionType
ALU = mybir.AluOpType
BF = dt.bfloat16
F32 = dt.float32

HID, NH, NKV, HD, INTER = 2560, 8, 4, 256, 10240
WIN, EPS, BASE = 512, 1e-6, 10000.0
S = 2048
NC_ = 8
TS = S // NC_              # 256 own tokens per core
HALO = 512
LT = TS + HALO             # 768 local tokens (halo + own)
TL = LT // 128             # 6 local token tiles
OT = TS // 128             # 2 own token tiles
KH = HID // 128            # 20 hidden-dim k-chunks
KA = (NH * HD) // 128      # 16 attn-dim chunks
MI = INTER // NC_ // 128   # 10 inter m-tiles per core
HALF = HD // 2
ISH = INTER // NC_         # 1280
# AllGather chunk schedule (offset, size): small head chunks so the MLP
# pipeline starts as early as possible; RS fires when each 64-token quarter
# of own tokens is fully reduced.
AG_CH = [(16 * i, 16) for i in range(6)] + \
        [(96 + 32 * i, 32) for i in range(5)]
CAG = len(AG_CH)
# ReduceScatter groups (offset, size): small tail so the last RS is short.
RS_GRP = [(0, 64), (64, 64), (128, 64), (192, 64)]
CRS = len(RS_GRP)


def rs_q(off):
    for q, (go, gs) in enumerate(RS_GRP):
        if go <= off < go + gs:
            return q, go, gs
    raise ValueError(off)


def _bcast_row(nc, sbuf_tile, dram_t, width):
    a = dram_t.ap()
    nc.sync.dma_start(sbuf_tile[:], bass.AP(
        tensor=a.tensor, offset=a.offset, ap=[[0, 128], [1, width]]))


def _swap_ap(src_ap, half):
    """View [128, 2*half] with halves swapped, as [128, 2, half]."""
    return bass.AP(tensor=src_ap.tensor, offset=src_ap.offset + half,
                   ap=[list(src_ap.ap[0]), [-half, 2], [1, half]])


def build_nc(sim=False):
    nc = bacc.Bacc("TRN2", target_bir_lowering=False, debug=False,
                   enable_asserts=True, num_devices=1 if sim else NC_)

    def _coll(kind, op, ins, outs):
        if not sim:
            nc.gpsimd.collective_compute(kind, op, replica_groups=rg,
                                         ins=ins, outs=outs)
            return
        i_ap, o_ap = ins[0], outs[0]
        if kind == "AllGather":
            n = i_ap.shape[0]
            for r in range(NC_):
                nc.sync.dma_start(o_ap[r * n:(r + 1) * n], i_ap)
        elif kind == "ReduceScatter":
            n = o_ap.shape[0]
            nc.sync.dma_start(o_ap, i_ap[0:n])

    # ---- inputs (host-prepacked layouts) ----
    x_p = nc.dram_tensor("x_p", [128, TL, HID], BF, kind="ExternalInput")
    x_own = nc.dram_tensor("x_own", [128, OT, HID], F32, kind="ExternalInput")
    pad_p = nc.dram_tensor("pad_p", [128, TL], F32, kind="ExternalInput")
    cq = nc.dram_tensor("cq", [128, OT, HD], BF, kind="ExternalInput")
    sq = nc.dram_tensor("sq", [128, OT, HD], BF, kind="ExternalInput")
    ck = nc.dram_tensor("ck", [128, TL, HD], BF, kind="ExternalInput")
    sk = nc.dram_tensor("sk", [128, TL, HD], BF, kind="ExternalInput")
    wq_p = nc.dram_tensor("wq_p", [128, KH, NH * HD], BF, kind="ExternalInput")
    wk_p = nc.dram_tensor("wk_p", [128, KH, NKV * HD], BF, kind="ExternalInput")
    wv_p = nc.dram_tensor("wv_p", [128, KH, NKV * HD], BF, kind="ExternalInput")
    wo_p = nc.dram_tensor("wo_p", [128, KA, HID], BF, kind="ExternalInput")
    wg_l = nc.dram_tensor("wg_l", [128, MI, KH, 128], BF, kind="ExternalInput")
    wu_l = nc.dram_tensor("wu_l", [128, MI, KH, 128], BF, kind="ExternalInput")
    wd_p = nc.dram_tensor("wd_p", [128, MI, HID], BF, kind="ExternalInput")
    w1_in = nc.dram_tensor("w1_in", [HID], BF, kind="ExternalInput")
    w1_pa = nc.dram_tensor("w1_pa", [HID], BF, kind="ExternalInput")
    w1_pf = nc.dram_tensor("w1_pf", [HID], BF, kind="ExternalInput")
    w1_po = nc.dram_tensor("w1_po", [HID], F32, kind="ExternalInput")
    out_shard = nc.dram_tensor("out_shard", [TS, HID], F32, kind="ExternalOutput")

    rg = [list(range(NC_))]
    stages = {}
    nc._stage_ids = stages

    def mark(name):
        stages[name] = nc.next_id()

    with tile.TileContext(nc) as tc:
        with (
            tc.tile_pool(name="dram", bufs=1, space="DRAM") as dram,
            tc.tile_pool(name="glob", bufs=1) as glob,
            tc.tile_pool(name="nrm", bufs=3) as nrm,
            tc.tile_pool(name="psP", bufs=1, space="PSUM") as psP,
        ):
            # DRAM scratch
            ag_in = [dram.tile([HID, sz], BF, name=f"agin{j}")
                     for j, (off, sz) in enumerate(AG_CH)]
            ag_out = [dram.tile([NC_ * HID, sz], BF, name=f"agout{j}",
                                addr_space="Local" if sim else "Shared")
                      for j, (off, sz) in enumerate(AG_CH)]
            rs_in = [dram.tile([NC_ * gs, HID], BF, name=f"rsin{q}")
                     for q, (go, gs) in enumerate(RS_GRP)]
            rs_out = [dram.tile([gs, HID], BF, name=f"rsout{q}")
                      for q, (go, gs) in enumerate(RS_GRP)]
            x2_spill = dram.tile([TS, HID], F32)

            ident = glob.tile([128, 128], BF)
            make_identity(nc, ident[:])
            eps_t = glob.tile([128, 1], F32)
            nc.vector.memset(eps_t[:], EPS)
            ones_t = glob.tile([128, 1], BF)
            nc.vector.memset(ones_t[:], 1.0)

            # attention-phase residents: a2 lives through S3, a1 dies at S2 end
            pool_a2 = tc.alloc_tile_pool(name="pa2", bufs=1)
            KT = pool_a2.tile([128, NKV * 2, LT], BF)  # K^T [d, tok]
            QT = pool_a2.tile([128, KA, TS], BF)       # Q^T [d, tok]
            V = [pool_a2.tile([128, NKV * HD], BF, name=f"V{t}")
                 for t in range(TL)]
            sc_pool = tc.alloc_tile_pool(name="sc", bufs=1)
            pool_a = tc.alloc_tile_pool(name="pa1", bufs=1)
            hT = pool_a.tile([128, KH, LT], BF)       # h^T
            wk_sb = pool_a.tile([128, KH, NKV * HD], BF)
            nc.sync.dma_start(wk_sb[:, :, 0:512], wk_p.ap()[:, :, 0:512])
            nc.sync.dma_start(wk_sb[:, :, 512:1024], wk_p.ap()[:, :, 512:1024])
            cq_sb = pool_a.tile([128, OT, HD], BF)
            sq_sb = pool_a.tile([128, OT, HD], BF)
            ck_sb = pool_a.tile([128, TL, HD], BF)
            sk_sb = pool_a.tile([128, TL, HD], BF)
            nc.sync.dma_start(cq_sb[:], cq.ap())
            nc.sync.dma_start(sq_sb[:], sq.ap())
            nc.sync.dma_start(ck_sb[:], ck.ap())
            nc.sync.dma_start(sk_sb[:], sk.ap())

            # window/causal masks [128 ko, 256 qo] for k-tiles 0,1,4,5
            # valid iff 0 <= (512+qo) - (128*kt+ko) < 512
            masks = pool_a2.tile([128, 4, TS], BF)
            for i, kt in enumerate((0, 1, 4, 5)):
                mk = masks[:, i, :]
                nc.gpsimd.memset(mk, 1.0)
                if kt in (0, 1):
                    # keep where ko + (128*kt - 1) - qo >= 0
                    nc.gpsimd.affine_select(
                        out=mk, in_=mk, compare_op=ALU.is_ge, fill=0.0,
                        base=128 * kt - 1, pattern=[[-1, TS]],
                        channel_multiplier=1)
                else:
                    # keep where qo - ko + (512 - 128*kt) >= 0
                    nc.gpsimd.affine_select(
                        out=mk, in_=mk, compare_op=ALU.is_ge, fill=0.0,
                        base=512 - 128 * kt, pattern=[[1, TS]],
                        channel_multiplier=-1)
            pad_sb = pool_a2.tile([128, TL], F32)
            nc.sync.dma_start(pad_sb[:], pad_p.ap())

            def rinv_from_stats(stats, name):
                mv = nrm.tile([128, 2], F32, tag="nmv", name=f"{name}_mv")
                nc.vector.bn_aggr(out=mv[:], in_=stats[:])
                ms = nrm.tile([128, 1], F32, tag="nms", name=f"{name}_ms")
                nc.vector.scalar_tensor_tensor(ms[:], mv[:, 0:1], mv[:, 0:1],
                                               mv[:, 1:2], op0=ALU.mult,
                                               op1=ALU.add)
                nc.vector.tensor_scalar_add(ms[:], ms[:], EPS)
                rec = nrm.tile([128, 1], F32, tag="nrc", name=f"{name}_rc")
                nc.vector.reciprocal(rec[:], ms[:])
                rinv = nrm.tile([128, 1], F32, tag="nrv", name=f"{name}_rv")
                nc.scalar.activation(rinv[:], rec[:], AF.Sqrt)
                return rinv

            def rmsnorm_rinv(src_ap, d, name, rows=128):
                """rinv[p,1] = 1/sqrt(mean(src^2)+EPS) via bn_stats."""
                nsub = max(1, d // 512)
                stats = nrm.tile([128, nsub, 6], F32, tag="nst", name=f"{name}_st")
                if nsub > 1:
                    view = src_ap.rearrange("p (s f) -> p s f", s=nsub)
                    for i in range(nsub):
                        nc.vector.bn_stats(out=stats[:rows, i, :],
                                           in_=view[:, i, :])
                else:
                    nc.vector.bn_stats(out=stats[:rows, 0, :], in_=src_ap)
                mv = nrm.tile([128, 2], F32, tag="nmv", name=f"{name}_mv")
                nc.vector.bn_aggr(out=mv[:rows], in_=stats[:rows])
                ms = nrm.tile([128, 1], F32, tag="nms", name=f"{name}_ms")
                nc.vector.scalar_tensor_tensor(ms[:rows], mv[:rows, 0:1],
                                               mv[:rows, 0:1], mv[:rows, 1:2],
                                               op0=ALU.mult, op1=ALU.add)
                nc.vector.tensor_scalar_add(ms[:rows], ms[:rows], EPS)
                rec = nrm.tile([128, 1], F32, tag="nrc", name=f"{name}_rc")
                nc.vector.reciprocal(rec[:rows], ms[:rows])
                rinv = nrm.tile([128, 1], F32, tag="nrv", name=f"{name}_rv")
                nc.scalar.activation(rinv[:rows], rec[:rows], AF.Sqrt)
                return rinv

            def k_group(pool, nch, t):
                ps = psP.tile([128, 512], F32, tag="mm", bufs=6,
                              name=f"psk{nch}_{t}")
                for k in range(KH):
                    nc.tensor.matmul(ps[:], hT[:, k, t * 128:(t + 1) * 128],
                                     wk_sb[:, k, nch * 512:(nch + 1) * 512],
                                     start=(k == 0), stop=(k == KH - 1))
                pcp = pool.tile([128, 512], BF, tag="pcp",
                                name=f"kcp{nch}_{t}", bufs=4)
                nc.scalar.copy(out=pcp[:], in_=ps[:])
                rinv2 = rmsnorm_rinv2(pcp[:], f"kn{nch}_{t}")
                kh = pool.tile([128, 512], BF, tag="kh",
                               name=f"kh{nch}_{t}", bufs=3)
                t2 = pool.tile([128, 512], BF, tag="t2",
                               name=f"t2k{nch}_{t}", bufs=3)
                for hh in range(2):
                    srcp = pcp[:, hh * HD:(hh + 1) * HD]
                    nc.vector.scalar_tensor_tensor(
                        kh[:, hh * HD:(hh + 1) * HD], srcp,
                        rinv2[:, hh:hh + 1], ck_sb[:, t, :],
                        op0=ALU.mult, op1=ALU.mult)
                    nc.vector.scalar_tensor_tensor(
                        t2[:, hh * HD:(hh + 1) * HD]
                        .rearrange("p (a b) -> p a b", a=2),
                        _swap_ap(srcp, HALF), rinv2[:, hh:hh + 1],
                        sk_sb[:, t, :].rearrange("p (a b) -> p a b", a=2),
                        op0=ALU.mult, op1=ALU.mult)
                nc.gpsimd.tensor_add(kh[:], kh[:], t2[:])
                ptr = psP.tile([128, 512], BF, tag="tr", bufs=2,
                               name=f"ktr{nch}_{t}")
                for mm in range(4):
                    nc.tensor.transpose(ptr[:, mm * 128:(mm + 1) * 128],
                                        kh[:, mm * 128:(mm + 1) * 128],
                                        ident[:])
                nc.scalar.copy(
                    out=KT[:, nch * 4:(nch + 1) * 4, t * 128:(t + 1) * 128],
                    in_=ptr[:].rearrange("p (a b) -> p a b", a=4))

            def rmsnorm_rinv2(src_ap, name):
                """Two per-head rinvs for a [128, 512] psum (2 heads of 256).
                Returns [128, 2] f32 tile."""
                stats = nrm.tile([128, 2, 6], F32, tag="hst", name=f"{name}_st")
                view = src_ap.rearrange("p (s f) -> p s f", s=2)
                for i in range(2):
                    nc.vector.bn_stats(out=stats[:, i, :], in_=view[:, i, :])
                mv = nrm.tile([128, 2, 2], F32, tag="hmv", name=f"{name}_mv")
                for i in range(2):
                    nc.vector.bn_aggr(out=mv[:, i, :], in_=stats[:, i, :])
                ms = nrm.tile([128, 2], F32, tag="hms", name=f"{name}_ms")
                nc.vector.tensor_mul(ms[:], mv[:, :, 0], mv[:, :, 0])
                nc.vector.tensor_add(ms[:], ms[:], mv[:, :, 1])
                nc.vector.tensor_scalar_add(ms[:], ms[:], EPS)
                rec = nrm.tile([128, 2], F32, tag="hrc", name=f"{name}_rc")
                nc.vector.reciprocal(rec[:], ms[:])
                rinv = nrm.tile([128, 2], F32, tag="hrv", name=f"{name}_rv")
                nc.scalar.activation(rinv[:], rec[:], AF.Sqrt)
                return rinv

            mark('S1')
            # ============ S1: in_ln over 768 local tokens + transpose ======
            with tc.tile_pool(name="s1", bufs=2) as s1:
                w1_in_b = s1.tile([128, HID], BF, bufs=1)
                _bcast_row(nc, w1_in_b, w1_in, HID)
                for t in range(TL):
                    xt = s1.tile([128, HID], BF, tag="xt", name=f"xt{t}", bufs=2)
                    nc.scalar.dma_start(xt[:], x_p.ap()[:, t, :])
                    rinv = rmsnorm_rinv(xt[:], HID, f"inln{t}")
                    ht = s1.tile([128, HID], BF, tag="ht", name=f"ht{t}", bufs=2)
                    for cch in range(5):
                        sl = slice(cch * 512, (cch + 1) * 512)
                        nc.vector.scalar_tensor_tensor(
                            ht[:, sl], xt[:, sl], rinv[:], w1_in_b[:, sl],
                            op0=ALU.mult, op1=ALU.mult)
                    for g in range(5):   # 4 k-chunks per transpose group
                        ptr = psP.tile([128, 512], BF, tag="tr", bufs=2,
                                       name=f"s1tr{t}_{g}")
                        for kk in range(4):
                            nc.tensor.transpose(
                                ptr[:, kk * 128:(kk + 1) * 128],
                                ht[:, (g * 4 + kk) * 128:(g * 4 + kk + 1) * 128],
                                ident[:])
                        dst = hT[:, g * 4:(g + 1) * 4, t * 128:(t + 1) * 128]
                        src_ = ptr[:].rearrange("p (a b) -> p a b", a=4)
                        nc.scalar.copy(out=dst, in_=src_)

            mark('S2')
            # ====== S2: Q projection, then scores+exp, then V (overlap) =====
            kt_mask = {0: 0, 1: 1, 4: 2, 5: 3}
            PRS = [[None] * TL for _ in range(NH)]
            with tc.tile_pool(name="s2", bufs=2) as s2:
                # --- K: per (nch, t): 2 heads norm+rope, transpose ---
                for nch in range(2):
                    for t in range(TL):
                        k_group(s2, nch, t)

                # --- Q: own tokens only (local tiles 4,5) ---
                for nch in range(4):
                    wqt = s2.tile([128, KH, 512], BF, tag="wst",
                                  name=f"wq{nch}", bufs=2)
                    nc.sync.dma_start(wqt[:], wq_p.ap()[:, :, nch * 512:(nch + 1) * 512])
                    for t in range(OT):
                        lt = 4 + t
                        ps = psP.tile([128, 512], F32, tag="mm", bufs=6,
                                      name=f"psq{nch}_{t}")
                        for k in range(KH):
                            nc.tensor.matmul(ps[:], hT[:, k, lt * 128:(lt + 1) * 128],
                                             wqt[:, k, :], start=(k == 0),
                                             stop=(k == KH - 1))
                        pcp = s2.tile([128, 512], BF, tag="pcp",
                                      name=f"qcp{nch}_{t}", bufs=4)
                        nc.scalar.copy(out=pcp[:], in_=ps[:])
                        rinv2 = rmsnorm_rinv2(pcp[:], f"qn{nch}_{t}")
                        qh = s2.tile([128, 512], BF, tag="qh",
                                     name=f"qh{nch}_{t}", bufs=3)
                        t2 = s2.tile([128, 512], BF, tag="t2",
                                     name=f"t2q{nch}_{t}", bufs=3)
                        for hh in range(2):
                            srcp = pcp[:, hh * HD:(hh + 1) * HD]
                            nc.vector.scalar_tensor_tensor(
                                qh[:, hh * HD:(hh + 1) * HD], srcp,
                                rinv2[:, hh:hh + 1], cq_sb[:, t, :],
                                op0=ALU.mult, op1=ALU.mult)
                            nc.vector.scalar_tensor_tensor(
                                t2[:, hh * HD:(hh + 1) * HD]
                                .rearrange("p (a b) -> p a b", a=2),
                                _swap_ap(srcp, HALF), rinv2[:, hh:hh + 1],
                                sq_sb[:, t, :].rearrange("p (a b) -> p a b", a=2),
                                op0=ALU.mult, op1=ALU.mult)
                        nc.gpsimd.tensor_add(qh[:], qh[:], t2[:])
                        ptr = psP.tile([128, 512], BF, tag="tr", bufs=2,
                                       name=f"qtr{nch}_{t}")
                        for mm in range(4):
                            nc.tensor.transpose(ptr[:, mm * 128:(mm + 1) * 128],
                                                qh[:, mm * 128:(mm + 1) * 128],
                                                ident[:])
                        nc.scalar.copy(
                            out=QT[:, nch * 4:(nch + 1) * 4, t * 128:(t + 1) * 128],
                            in_=ptr[:].rearrange("p (a b) -> p a b", a=4))

                # --- scores + exp + mask for all heads (PE overlaps V next) ---
                for h in range(NH):
                    md = h - (h % 2)      # KT m-tile base for kv head h//2
                    for kt in range(TL):
                        psc = psP.tile([128, TS], F32, tag="mm", bufs=6,
                                       name=f"psc{h}_{kt}")
                        for dh in range(2):
                            nc.tensor.matmul(
                                psc[:], KT[:, md + dh, kt * 128:(kt + 1) * 128],
                                QT[:, 2 * h + dh, :],
                                start=(dh == 0), stop=(dh == 1))
                        pr = sc_pool.tile([128, TS], BF, name=f"pr{h}_{kt}")
                        nc.scalar.activation(pr[:], psc[:], AF.Exp,
                                             scale=1.0 / 16.0)
                        if kt in (0, 1):
                            nc.vector.scalar_tensor_tensor(
                                pr[:], pr[:], pad_sb[:, kt:kt + 1],
                                masks[:, kt_mask[kt], :],
                                op0=ALU.mult, op1=ALU.mult)
                        elif kt in (2, 3):
                            nc.vector.tensor_scalar_mul(pr[:], pr[:],
                                                        pad_sb[:, kt:kt + 1])
                        else:
                            nc.vector.tensor_mul(pr[:], pr[:],
                                                 masks[:, kt_mask[kt], :])
                        PRS[h][kt] = pr

                # --- V projection (PE; exp/mask of scores overlaps) ---
                for nch in range(2):
                    wvt = s2.tile([128, KH, 512], BF, tag="wst",
                                  name=f"wv{nch}", bufs=2)
                    nc.sync.dma_start(wvt[:], wv_p.ap()[:, :, nch * 512:(nch + 1) * 512])
                    for t in range(TL):
                        ps = psP.tile([128, 512], F32, tag="mm", bufs=6,
                                      name=f"psv{nch}_{t}")
                        for k in range(KH):
                            nc.tensor.matmul(ps[:], hT[:, k, t * 128:(t + 1) * 128],
                                             wvt[:, k, :], start=(k == 0),
                                             stop=(k == KH - 1))
                        nc.scalar.copy(out=V[t][:, nch * 512:(nch + 1) * 512],
                                       in_=ps[:])

            mark('S3')
            # ===== S3: attention PV + output (scores already computed) =====
            pool_a.release()   # hT + wk + rope tables
            # MLP gate weights + full wo: preload now (DMA idle, PE busy)
            pool_d = tc.alloc_tile_pool(name="pd", bufs=1, side="right")
            wg_sb = pool_d.tile([128, MI, KH, 128], BF)
            pool_b = tc.alloc_tile_pool(name="pb", bufs=1, side="right")
            attnT = pool_b.tile([128, KA, TS], BF)
            wo_sb = pool_b.tile([128, KA, HID], BF)
            for _i in range(8):
                nc.sync.dma_start(wo_sb[:, 2 * _i:2 * _i + 2, :],
                                  wo_p.ap()[:, 2 * _i:2 * _i + 2, :])
            for _i in range(4):
                nc.sync.dma_start(
                    wg_sb[:, :, 5 * _i:5 * _i + 5, :],
                    wg_l.ap()[:, :, 5 * _i:5 * _i + 5, :])
            with tc.tile_pool(name="s3", bufs=2) as s3:
                for qt in range(OT):
                  for h in range(NH):
                    prs = PRS[h]
                    if True:
                        po = psP.tile([128, HD + 1], F32, tag="mm", bufs=6,
                                      name=f"po{h}_{qt}")
                        for kt in range(TL):
                            sl = slice(qt * 128, (qt + 1) * 128)
                            nc.tensor.matmul(po[:, 0:HD], prs[kt][:, sl],
                                             V[kt][:, (h // 2) * HD:(h // 2 + 1) * HD],
                                             start=(kt == 0), stop=(kt == TL - 1))
                        for kt in range(TL):
                            sl = slice(qt * 128, (qt + 1) * 128)
                            nc.tensor.matmul(po[:, HD:HD + 1], prs[kt][:, sl],
                                             ones_t[:], start=(kt == 0),
                                             stop=(kt == TL - 1))
                        rec = s3.tile([128, 1], F32, tag="rec",
                                      name=f"rec{h}_{qt}")
                        nc.vector.reciprocal(rec[:], po[:, HD:HD + 1])
                        an = s3.tile([128, HD], BF, tag="an",
                                     name=f"an{h}_{qt}")
                        nc.vector.tensor_scalar_mul(an[:], po[:, 0:HD], rec[:])
                        ptr = psP.tile([128, HD], BF, tag="tr", bufs=2,
                                       name=f"atr{h}_{qt}")
                        for mm in range(2):
                            nc.tensor.transpose(ptr[:, mm * 128:(mm + 1) * 128],
                                                an[:, mm * 128:(mm + 1) * 128],
                                                ident[:])
                        nc.scalar.copy(
                            out=attnT[:, 2 * h:2 * h + 2, qt * 128:(qt + 1) * 128],
                            in_=ptr[:].rearrange("p (a b) -> p a b", a=2))

            mark('S4')
            # ======== S4: wo + post_attn + residual + pre_ff + AG ========
            sc_pool.release()   # probs
            pool_a2.release()   # frees KT/QT/V/masks
            pool_c = tc.alloc_tile_pool(name="pc", bufs=1)
            h2T = pool_c.tile([128, KH, TS], BF)
            with tc.tile_pool(name="s4", bufs=2) as s4:
                w1_pa_b = s4.tile([128, HID], BF, bufs=1)
                w1_pf_b = s4.tile([128, HID], BF, bufs=1)
                _bcast_row(nc, w1_pa_b, w1_pa, HID)
                _bcast_row(nc, w1_pf_b, w1_pf, HID)
                xos = [s4.tile([128, HID], F32, tag="xo", name=f"xo{t}",
                               bufs=2) for t in range(OT)]
                for t in range(OT):
                    nc.sync.dma_start(xos[t][:], x_own.ap()[:, t, :])
                for t in range(OT):
                    ao32 = s4.tile([128, HID], F32, tag="ao32",
                                   name=f"ao32_{t}", bufs=1)
                    stats_a = nrm.tile([128, 5, 6], F32, tag="nst",
                                       name=f"pan{t}_st")
                    for n in range(5):
                        pw = psP.tile([128, 512], F32, tag="mm", bufs=6,
                                      name=f"pw{t}_{n}")
                        for m in range(KA):
                            nc.tensor.matmul(
                                pw[:], attnT[:, m, t * 128:(t + 1) * 128],
                                wo_sb[:, m, n * 512:(n + 1) * 512],
                                start=(m == 0), stop=(m == KA - 1))
                        nc.scalar.copy(out=ao32[:, n * 512:(n + 1) * 512],
                                       in_=pw[:])
                        nc.vector.bn_stats(out=stats_a[:, n, :],
                                           in_=ao32[:, n * 512:(n + 1) * 512])
                    rinv_a = rinv_from_stats(stats_a, f"pan{t}")
                    xo = xos[t]
                    x2 = s4.tile([128, HID], F32, tag="x2", name=f"x2_{t}",
                                 bufs=1)
                    stats_f = nrm.tile([128, 5, 6], F32, tag="nst",
                                       name=f"pff{t}_st")
                    for n in range(5):
                        sl = slice(n * 512, (n + 1) * 512)
                        nc.vector.scalar_tensor_tensor(
                            x2[:, sl], ao32[:, sl], rinv_a[:], w1_pa_b[:, sl],
                            op0=ALU.mult, op1=ALU.mult)
                        if n % 2 == 0:
                            nc.gpsimd.tensor_add(x2[:, sl], x2[:, sl],
                                                 xo[:, sl])
                        else:
                            nc.vector.tensor_add(x2[:, sl], x2[:, sl],
                                                 xo[:, sl])
                        nc.vector.bn_stats(out=stats_f[:, n, :], in_=x2[:, sl])
                    nc.sync.dma_start(x2_spill[t * 128:(t + 1) * 128, :], x2[:])
                    rinv_f = rinv_from_stats(stats_f, f"pff{t}")
                    h2 = s4.tile([128, HID], BF, tag="h2", name=f"h2_{t}",
                                 bufs=1)
                    for g in range(5):
                        sl = slice(g * 512, (g + 1) * 512)
                        nc.vector.scalar_tensor_tensor(
                            h2[:, sl], x2[:, sl], rinv_f[:],
                            w1_pf_b[:, sl], op0=ALU.mult, op1=ALU.mult)
                        ptr = psP.tile([128, 512], BF, tag="tr", bufs=2,
                                       name=f"s4tr{t}_{g}")
                        for kk in range(4):
                            nc.tensor.transpose(
                                ptr[:, kk * 128:(kk + 1) * 128],
                                h2[:, (g * 4 + kk) * 128:(g * 4 + kk + 1) * 128],
                                ident[:])
                        nc.vector.tensor_copy(
                            h2T[:, g * 4:(g + 1) * 4, t * 128:(t + 1) * 128],
                            ptr[:].rearrange("p (a b) -> p a b", a=4))
                    # spill h2T slices to DRAM and AllGather this tile's chunks
                    for j, (off, sz) in enumerate(AG_CH):
                        if not (t * 128 <= off < (t + 1) * 128):
                            continue
                        agv = ag_in[j][:].rearrange("(k p) t -> p k t", p=128)
                        nc.sync.dma_start(agv, h2T[:, :, off:off + sz])
                        _coll("AllGather", ALU.bypass, [ag_in[j][:]],
                              [ag_out[j][:]])
            pool_b.release()   # attnT + wo
            pool_c.release()   # h2T
            pool_d2 = tc.alloc_tile_pool(name="pd2", bufs=1, side="right")
            wu_sb = pool_d2.tile([128, MI, KH, 128], BF)
            for _i in range(4):
                nc.sync.dma_start(
                    wu_sb[:, :, 5 * _i:5 * _i + 5, :],
                    wu_l.ap()[:, :, 5 * _i:5 * _i + 5, :])

            mark('S5')
            # ================= S5: MLP, chunk-pipelined =================
            with tc.tile_pool(name="s5w", bufs=1) as s5w:
                wd_sb = s5w.tile([128, MI, HID], BF)
                for _i in range(4):
                    nc.sync.dma_start(
                        wd_sb[:, :, 640 * _i:640 * _i + 640],
                        wd_p.ap()[:, :, 640 * _i:640 * _i + 640])
                def ag_v(j):
                    return ag_out[j][:].rearrange("(r k p) t -> r p k t",
                                                  p=128, r=NC_)
                def load_hf(dst, j):
                    v = ag_v(j)
                    for r in range(NC_):
                        nc.gpsimd.dma_start(dst[:, :, r, :], v[r])
                with tc.tile_pool(name="s5", bufs=2) as s5:
                    hfs = []
                    for j, (off, sz) in enumerate(AG_CH):
                        hft = s5.tile([128, KH, NC_, 32], BF, tag="hf",
                                      name=f"hf{j}", bufs=3)
                        hfs.append(hft[:, :, :, 0:sz])
                    load_hf(hfs[0], 0)
                    load_hf(hfs[1], 1)
                    for j, (off, sz) in enumerate(AG_CH):
                        if j + 2 < CAG:
                            load_hf(hfs[j + 2], j + 2)
                        hf = hfs[j]
                        N = NC_ * sz
                        actT = s5.tile([128, MI, NC_ * 32], BF, tag="actT",
                                       name=f"actT{j}", bufs=2)[:, :, 0:N]
                        for m in range(MI):
                            pg = psP.tile([128, N], F32, tag="mm",
                                          bufs=6, name=f"pg{j}_{m}")
                            pu = psP.tile([128, N], F32, tag="mm",
                                          bufs=6, name=f"pu{j}_{m}")
                            pg3 = pg[:].rearrange("p (r t) -> p r t", r=NC_)
                            pu3 = pu[:].rearrange("p (r t) -> p r t", r=NC_)
                            for k in range(KH):
                                st, sp = (k == 0), (k == KH - 1)
                                nc.tensor.matmul(pg3, wg_sb[:, m, k, :],
                                                 hf[:, k], start=st, stop=sp)
                                nc.tensor.matmul(pu3, wu_sb[:, m, k, :],
                                                 hf[:, k], start=st, stop=sp)
                            gsc = s5.tile([128, NC_ * 32], F32, tag="gsc",
                                          name=f"gsc{j}_{m}", bufs=3)[:, 0:N]
                            nc.scalar.activation(gsc, pg[:],
                                                 AF.Gelu_apprx_tanh)
                            nc.vector.tensor_mul(actT[:, m, :], gsc, pu[:])
                        # down: token-tiles x 5 n-chunks
                        q, g_off, g_sz = rs_q(off)
                        nblocks = 128 // sz
                        for tt in range(N // 128):
                            for ng in ((0, 1), (2, 3), (4,)):
                                pds = {n: psP.tile([128, 512], F32, tag="mm",
                                                   bufs=6,
                                                   name=f"pd{j}_{tt}_{n}")
                                       for n in ng}
                                for m in range(MI):
                                    for n in ng:
                                        nc.tensor.matmul(
                                            pds[n][:],
                                            actT[:, m, tt * 128:(tt + 1) * 128],
                                            wd_sb[:, m, n * 512:(n + 1) * 512],
                                            start=(m == 0), stop=(m == MI - 1))
                                for n in ng:
                                    pd = pds[n]
                                    dcp = s5.tile([128, 512], BF, tag="dcp",
                                                  name=f"dcp{j}_{tt}_{n}",
                                                  bufs=8)
                                    if n % 2 == 0:
                                        nc.vector.tensor_copy(dcp[:], pd[:])
                                    else:
                                        nc.scalar.copy(out=dcp[:], in_=pd[:])
                                    # pd partition p = rr*sz+i -> rs_in[q] row
                                    # (nblocks*tt+rr)*64 + (off-64q) + i
                                    a = rs_in[q][:]
                                    dst = bass.AP(
                                        tensor=a.tensor,
                                        offset=a.offset
                                        + (nblocks * tt * g_sz + off - g_off)
                                        * HID + n * 512,
                                        ap=[[g_sz * HID, nblocks], [HID, sz],
                                            [1, 512]])
                                    nc.sync.dma_start(dst, dcp[:])
                        if off + sz - 64 * q == 64:
                                                        _coll("ReduceScatter", ALU.add, [rs_in[q][:]],
                                  [rs_out[q][:]])
            pool_d2.release()
            pool_d.release()

            mark('S6')
            # ============ S6: post_ff norm + residual ============
            with tc.tile_pool(name="s6", bufs=2) as s6:
                w1_po_b = s6.tile([128, HID], F32, bufs=1)
                _bcast_row(nc, w1_po_b, w1_po, HID)
                for t in range(OT):
                    mlp16 = s6.tile([128, HID], BF, tag="mlp", name=f"mlp{t}",
                                    bufs=2)
                    pos = 0
                    for q, (go, gs) in enumerate(RS_GRP):
                        if not (t * 128 <= go < (t + 1) * 128):
                            continue
                        nc.sync.dma_start(mlp16[pos:pos + gs, :], rs_out[q][:])
                        pos += gs
                    x2l = s6.tile([128, HID], F32, tag="x2l", name=f"x2l{t}",
                                  bufs=2)
                    nc.sync.dma_start(x2l[:], x2_spill[t * 128:(t + 1) * 128, :])
                    rinv_o = rmsnorm_rinv(mlp16[:], HID, f"pon{t}")
                    o32 = s6.tile([128, HID], F32, tag="o32", name=f"o32_{t}",
                                  bufs=2)
                    for n in range(4):
                        sl = slice(n * 640, (n + 1) * 640)
                        nc.vector.scalar_tensor_tensor(
                            o32[:, sl], mlp16[:, sl], rinv_o[:],
                            w1_po_b[:, sl], op0=ALU.mult, op1=ALU.mult)
                        if n % 2 == 0:
                            nc.gpsimd.tensor_add(o32[:, sl], o32[:, sl],
                                                 x2l[:, sl])
                        else:
                            nc.vector.tensor_add(o32[:, sl], o32[:, sl],
                                                 x2l[:, sl])
                        nc.sync.dma_start(
                            out_shard.ap()[t * 128:(t + 1) * 128, sl],
                            o32[:, sl])

    nc.compile()
    return nc


_NC_CACHE = None


def _get_nc():
    global _NC_CACHE
    if _NC_CACHE is None:
        _NC_CACHE = build_nc()
    return _NC_CACHE


def make_in_maps(hidden_states, position_ids, wq, wk, wv, wo, q_ln_w, k_ln_w,
                 in_ln_w, post_attn_ln_w, pre_ff_ln_w, post_ff_ln_w,
                 w_gate, w_up, w_down):
    bf16 = ml_dtypes.bfloat16
    f32 = np.float32
    x = np.asarray(hidden_states, f32).reshape(S, HID)
    pos = np.asarray(position_ids).reshape(S).astype(np.float64)

    inv_freq = 1.0 / (BASE ** (np.arange(0, HD, 2, dtype=np.float64) / HD))
    w1q = 1.0 + np.asarray(q_ln_w, f32)
    w1k = 1.0 + np.asarray(k_ln_w, f32)

    def rope_tabs(p, w1):
        emb = np.concatenate([p[:, None] * inv_freq[None, :]] * 2, axis=1)
        cos = np.cos(emb).astype(f32)
        sin = np.sin(emb).astype(f32)
        w1sw = np.concatenate([w1[HALF:], w1[:HALF]])
        sgn = np.concatenate([-np.ones(HALF, f32), np.ones(HALF, f32)])
        n = len(p)
        c = (cos * w1[None, :]).astype(bf16).reshape(n // 128, 128, HD)
        s_ = (sin * (w1sw * sgn)[None, :]).astype(bf16).reshape(n // 128, 128, HD)
        return (np.ascontiguousarray(c.transpose(1, 0, 2)),
                np.ascontiguousarray(s_.transpose(1, 0, 2)))

    def pack(w, kt, n):
        return np.ascontiguousarray(
            np.asarray(w, f32).reshape(kt, 128, n).transpose(1, 0, 2)).astype(bf16)

    wg_r = np.asarray(w_gate, f32).reshape(HID, NC_, ISH)
    wu_r = np.asarray(w_up, f32).reshape(HID, NC_, ISH)
    wd_r = np.asarray(w_down, f32).reshape(NC_, ISH, HID)

    common = {
        "wq_p": pack(wq, KH, NH * HD),
        "wk_p": pack(wk, KH, NKV * HD),
        "wv_p": pack(wv, KH, NKV * HD),
        "wo_p": pack(wo, KA, HID),
        "w1_in": (1.0 + np.asarray(in_ln_w, f32)).astype(bf16),
        "w1_pa": (1.0 + np.asarray(post_attn_ln_w, f32)).astype(bf16),
        "w1_pf": (1.0 + np.asarray(pre_ff_ln_w, f32)).astype(bf16),
        "w1_po": 1.0 + np.asarray(post_ff_ln_w, f32),
    }
    in_maps = []
    for c in range(NC_):
        lo = c * TS - HALO
        xh = np.zeros((LT, HID), f32)
        src_lo = max(0, lo)
        xh[src_lo - lo:] = x[src_lo:lo + LT]
        x_pk = np.ascontiguousarray(
            xh.reshape(TL, 128, HID).transpose(1, 0, 2)).astype(bf16)
        x_ow = np.ascontiguousarray(
            x[c * TS:(c + 1) * TS].reshape(OT, 128, HID).transpose(1, 0, 2))
        padv = (np.arange(lo, lo + LT) >= 0).astype(f32)
        pad_pk = np.ascontiguousarray(padv.reshape(TL, 128).T)
        kpos = np.where(np.arange(lo, lo + LT) >= 0,
                        pos[np.clip(np.arange(lo, lo + LT), 0, S - 1)], 0.0)
        qpos = pos[c * TS:(c + 1) * TS]
        ckw, skw = rope_tabs(kpos, w1k)
        cqw, sqw = rope_tabs(qpos, w1q)
        wg_c = wg_r[:, c, :]   # [HID, 1280]
        wu_c = wu_r[:, c, :]
        in_maps.append({
            "x_p": x_pk,
            "x_own": x_ow,
            "pad_p": pad_pk,
            "cq": cqw, "sq": sqw, "ck": ckw, "sk": skw,
            "wg_l": np.ascontiguousarray(
                wg_c.reshape(KH, 128, MI, 128).transpose(1, 2, 0, 3)
            ).astype(bf16),
            "wu_l": np.ascontiguousarray(
                wu_c.reshape(KH, 128, MI, 128).transpose(1, 2, 0, 3)
            ).astype(bf16),
            "wd_p": np.ascontiguousarray(
                wd_r[c].reshape(MI, 128, HID).transpose(1, 0, 2)).astype(bf16),
            **common,
        })
    return in_maps


def kernel(**inputs):
    in_maps = make_in_maps(**inputs)
    nc = _get_nc()
    res = run_bass_kernel_spmd(nc, in_maps, core_ids=list(range(NC_)))
    out = np.concatenate([res.results[c]["out_shard"] for c in range(NC_)], axis=0)
    return out.reshape(1, S, HID).astype(np.float32)



# revision 17
# speedup vs baseline: 1.0598x; 1.0598x over previous
"""Gemma3 decoder layer on 8 Trainium2 NeuronCores (Bass/Tile), v3.

Sharding (per core c): fully sequence-parallel, ZERO collectives.
  - attention: core c owns tokens [256c, 256c+256) and receives a 512-token
    halo (host-side sharding): x_halo = x[256c-512 : 256c+256] (zero-padded
    for c<2). All attn weights (wq/wk/wv/wo) replicated; K/V computed for all
    768 local tokens, Q only for the 256 own tokens. Sliding-window (512)
    attention is then fully local. Pad keys masked via per-core pad mask.
  - MLP: sequence-parallel too. Each core runs the FULL 10240-dim MLP for its
    own 256 tokens, streaming gate/up/down weights (157MB bf16) from DRAM in
    128-inter-dim chunks, double-buffered under the matmul stream. Down
    partials accumulate in PSUM per 8-chunk group, then into an f32 SBUF
    accumulator (DVE/Pool adds). No AllGather, no ReduceScatter, no DRAM
    spill of h2/x2.
  - norms/residual: token-local.
Matmuls in bf16 (fp32 PSUM accumulation); norms/softmax/residual fp32.
All weights host-prepacked into SBUF layout (contiguous 5-10KB/partition DMAs).
"""
import sys

if "/opt/trn_rl_repo" not in sys.path:
    sys.path.insert(0, "/opt/trn_rl_repo")

import numpy as np
import ml_dtypes

import concourse.bass as bass
import concourse.mybir as mybir
import concourse.tile as tile
from concourse import bacc
from concourse.bass_utils import run_bass_kernel_spmd
from concourse.masks import make_identity

dt = mybir.dt
AF = mybir.ActivationFunctionType
ALU = mybir.AluOpType
BF = dt.bfloat16
F32 = dt.float32

HID, NH, NKV, HD, INTER = 2560, 8, 4, 256, 10240
WIN, EPS, BASE = 512, 1e-6, 10000.0
S = 2048
NC_ = 8
TS = S // NC_              # 256 own tokens per core
HALO = 512
LT = TS + HALO             # 768 local tokens (halo + own)
TL = LT // 128             # 6 local token tiles
OT = TS // 128             # 2 own token tiles
KH = HID // 128            # 20 hidden-dim k-chunks
KA = (NH * HD) // 128      # 16 attn-dim chunks
MIF = INTER // 128         # 80 inter m-chunks (full, per core)
GM = 4                     # m-chunks per down-accumulation group
NGRP = MIF // GM           # 20 groups
PREF = 2                   # stream prefetch depth (m-chunks)
HALF = HD // 2


def _bcast_row(nc, sbuf_tile, dram_t, width):
    a = dram_t.ap()
    nc.sync.dma_start(sbuf_tile[:], bass.AP(
        tensor=a.tensor, offset=a.offset, ap=[[0, 128], [1, width]]))


def _swap_ap(src_ap, half):
    """View [128, 2*half] with halves swapped, as [128, 2, half]."""
    return bass.AP(tensor=src_ap.tensor, offset=src_ap.offset + half,
                   ap=[list(src_ap.ap[0]), [-half, 2], [1, half]])


def build_nc(sim=False):
    nc = bacc.Bacc("TRN2", target_bir_lowering=False, debug=False,
                   enable_asserts=True, num_devices=1 if sim else NC_)

    # ---- inputs (host-prepacked layouts) ----
    x_p = nc.dram_tensor("x_p", [128, TL, HID], BF, kind="ExternalInput")
    x_own = nc.dram_tensor("x_own", [128, OT, HID], F32, kind="ExternalInput")
    pad_p = nc.dram_tensor("pad_p", [128, TL], F32, kind="ExternalInput")
    cq = nc.dram_tensor("cq", [128, OT, HD], BF, kind="ExternalInput")
    sq = nc.dram_tensor("sq", [128, OT, HD], BF, kind="ExternalInput")
    ck = nc.dram_tensor("ck", [128, TL, HD], BF, kind="ExternalInput")
    sk = nc.dram_tensor("sk", [128, TL, HD], BF, kind="ExternalInput")
    wq_p = nc.dram_tensor("wq_p", [128, KH, NH * HD], BF, kind="ExternalInput")
    wk_p = nc.dram_tensor("wk_p", [128, KH, NKV * HD], BF, kind="ExternalInput")
    wv_p = nc.dram_tensor("wv_p", [128, KH, NKV * HD], BF, kind="ExternalInput")
    wo_p = nc.dram_tensor("wo_p", [128, KA, HID], BF, kind="ExternalInput")
    # gate+up interleaved: [kpart, m-chunk, {g,u}, k-chunk, m-col]
    wgu_f = nc.dram_tensor("wgu_f", [128, MIF, 2, KH, 128], BF,
                           kind="ExternalInput")
    # down: [inter-part-within-chunk, m-chunk, hid]
    wd_f = nc.dram_tensor("wd_f", [128, MIF, HID], BF, kind="ExternalInput")
    w1_in = nc.dram_tensor("w1_in", [HID], BF, kind="ExternalInput")
    w1_pa = nc.dram_tensor("w1_pa", [HID], BF, kind="ExternalInput")
    w1_pf = nc.dram_tensor("w1_pf", [HID], BF, kind="ExternalInput")
    w1_po = nc.dram_tensor("w1_po", [HID], F32, kind="ExternalInput")
    out_shard = nc.dram_tensor("out_shard", [TS, HID], F32, kind="ExternalOutput")

    stages = {}
    nc._stage_ids = stages

    def mark(name):
        stages[name] = nc.next_id()

    with tile.TileContext(nc) as tc:
        with (
            tc.tile_pool(name="dram", bufs=1, space="DRAM") as dram,
            tc.tile_pool(name="glob", bufs=1) as glob,
            tc.tile_pool(name="nrm", bufs=3) as nrm,
            tc.tile_pool(name="psP", bufs=1, space="PSUM") as psP,
        ):
            x2_spill = dram.tile([TS, HID], F32)
            ident = glob.tile([128, 128], BF)
            make_identity(nc, ident[:])
            eps_t = glob.tile([128, 1], F32)
            nc.vector.memset(eps_t[:], EPS)
            ones_t = glob.tile([128, 1], BF)
            nc.vector.memset(ones_t[:], 1.0)
            # first token tile + in_ln weight: issue DMAs before anything else
            # so S1's first norm chain starts ASAP
            xt0 = glob.tile([128, HID], BF)
            nc.scalar.dma_start(xt0[:], x_p.ap()[:, 0, :])
            w1_in_b = glob.tile([128, HID], BF)
            _bcast_row(nc, w1_in_b, w1_in, HID)
            # warm the activation-function tables during the DMAs above
            warm = glob.tile([128, 1], F32)
            for af in (AF.Sqrt, AF.Exp, AF.Gelu_apprx_tanh):
                nc.scalar.activation(warm[:], eps_t[:], af)

            # attention-phase residents: a2 lives through S3, a1 dies at S2 end
            pool_a2 = tc.alloc_tile_pool(name="pa2", bufs=1)
            KT = pool_a2.tile([128, NKV * 2, LT], BF)  # K^T [d, tok]
            QT = pool_a2.tile([128, KA, TS], BF)       # Q^T [d, tok]
            V = [pool_a2.tile([128, NKV * HD], BF, name=f"V{t}")
                 for t in range(TL)]
            sc_pool = tc.alloc_tile_pool(name="sc", bufs=1)
            pool_a = tc.alloc_tile_pool(name="pa1", bufs=1)
            hT = pool_a.tile([128, KH, LT], BF)       # h^T
            wk_sb = pool_a.tile([128, KH, NKV * HD], BF)
            nc.sync.dma_start(wk_sb[:, :, 0:512], wk_p.ap()[:, :, 0:512])
            nc.sync.dma_start(wk_sb[:, :, 512:1024], wk_p.ap()[:, :, 512:1024])
            cq_sb = pool_a.tile([128, OT, HD], BF)
            sq_sb = pool_a.tile([128, OT, HD], BF)
            ck_sb = pool_a.tile([128, TL, HD], BF)
            sk_sb = pool_a.tile([128, TL, HD], BF)
            nc.sync.dma_start(cq_sb[:], cq.ap())
            nc.sync.dma_start(sq_sb[:], sq.ap())
            nc.sync.dma_start(ck_sb[:], ck.ap())
            nc.sync.dma_start(sk_sb[:], sk.ap())

            # window/causal masks [128 ko, 256 qo] for k-tiles 0,1,4,5
            # valid iff 0 <= (512+qo) - (128*kt+ko) < 512
            masks = pool_a2.tile([128, 4, TS], BF)
            for i, kt in enumerate((0, 1, 4, 5)):
                mk = masks[:, i, :]
                nc.gpsimd.memset(mk, 1.0)
                if kt in (0, 1):
                    # keep where ko + (128*kt - 1) - qo >= 0
                    nc.gpsimd.affine_select(
                        out=mk, in_=mk, compare_op=ALU.is_ge, fill=0.0,
                        base=128 * kt - 1, pattern=[[-1, TS]],
                        channel_multiplier=1)
                else:
                    # keep where qo - ko + (512 - 128*kt) >= 0
                    nc.gpsimd.affine_select(
                        out=mk, in_=mk, compare_op=ALU.is_ge, fill=0.0,
                        base=512 - 128 * kt, pattern=[[1, TS]],
                        channel_multiplier=-1)
            pad_sb = pool_a2.tile([128, TL], F32)
            nc.sync.dma_start(pad_sb[:], pad_p.ap())

            def rinv_from_stats(stats, name):
                mv = nrm.tile([128, 2], F32, tag="nmv", name=f"{name}_mv")
                nc.vector.bn_aggr(out=mv[:], in_=stats[:])
                ms = nrm.tile([128, 1], F32, tag="nms", name=f"{name}_ms")
                nc.vector.scalar_tensor_tensor(ms[:], mv[:, 0:1], mv[:, 0:1],
                                               mv[:, 1:2], op0=ALU.mult,
                                               op1=ALU.add)
                nc.vector.tensor_scalar_add(ms[:], ms[:], EPS)
                rec = nrm.tile([128, 1], F32, tag="nrc", name=f"{name}_rc")
                nc.vector.reciprocal(rec[:], ms[:])
                rinv = nrm.tile([128, 1], F32, tag="nrv", name=f"{name}_rv")
                nc.scalar.activation(rinv[:], rec[:], AF.Sqrt)
                return rinv

            def rmsnorm_rinv(src_ap, d, name, rows=128):
                """rinv[p,1] = 1/sqrt(mean(src^2)+EPS) via bn_stats."""
                nsub = max(1, d // 512)
                stats = nrm.tile([128, nsub, 6], F32, tag="nst", name=f"{name}_st")
                if nsub > 1:
                    view = src_ap.rearrange("p (s f) -> p s f", s=nsub)
                    for i in range(nsub):
                        nc.vector.bn_stats(out=stats[:rows, i, :],
                                           in_=view[:, i, :])
                else:
                    nc.vector.bn_stats(out=stats[:rows, 0, :], in_=src_ap)
                mv = nrm.tile([128, 2], F32, tag="nmv", name=f"{name}_mv")
                nc.vector.bn_aggr(out=mv[:rows], in_=stats[:rows])
                ms = nrm.tile([128, 1], F32, tag="nms", name=f"{name}_ms")
                nc.vector.scalar_tensor_tensor(ms[:rows], mv[:rows, 0:1],
                                               mv[:rows, 0:1], mv[:rows, 1:2],
                                               op0=ALU.mult, op1=ALU.add)
                nc.vector.tensor_scalar_add(ms[:rows], ms[:rows], EPS)
                rec = nrm.tile([128, 1], F32, tag="nrc", name=f"{name}_rc")
                nc.vector.reciprocal(rec[:rows], ms[:rows])
                rinv = nrm.tile([128, 1], F32, tag="nrv", name=f"{name}_rv")
                nc.scalar.activation(rinv[:rows], rec[:rows], AF.Sqrt)
                return rinv

            def rmsnorm_rinv2(src_ap, name):
                """Two per-head rinvs for a [128, 512] psum (2 heads of 256).
                Returns [128, 2] f32 tile."""
                stats = nrm.tile([128, 2, 6], F32, tag="hst", name=f"{name}_st")
                view = src_ap.rearrange("p (s f) -> p s f", s=2)
                for i in range(2):
                    nc.vector.bn_stats(out=stats[:, i, :], in_=view[:, i, :])
                mv = nrm.tile([128, 2, 2], F32, tag="hmv", name=f"{name}_mv")
                for i in range(2):
                    nc.vector.bn_aggr(out=mv[:, i, :], in_=stats[:, i, :])
                ms = nrm.tile([128, 2], F32, tag="hms", name=f"{name}_ms")
                nc.vector.tensor_mul(ms[:], mv[:, :, 0], mv[:, :, 0])
                nc.vector.tensor_add(ms[:], ms[:], mv[:, :, 1])
                nc.vector.tensor_scalar_add(ms[:], ms[:], EPS)
                rec = nrm.tile([128, 2], F32, tag="hrc", name=f"{name}_rc")
                nc.vector.reciprocal(rec[:], ms[:])
                rinv = nrm.tile([128, 2], F32, tag="hrv", name=f"{name}_rv")
                nc.scalar.activation(rinv[:], rec[:], AF.Sqrt)
                return rinv

            def k_group(pool, nch, t):
                ps = psP.tile([128, 512], F32, tag="mm", bufs=6,
                              name=f"psk{nch}_{t}")
                for k in range(KH):
                    nc.tensor.matmul(ps[:], hT[:, k, t * 128:(t + 1) * 128],
                                     wk_sb[:, k, nch * 512:(nch + 1) * 512],
                                     start=(k == 0), stop=(k == KH - 1))
                pcp = pool.tile([128, 512], BF, tag="pcp",
                                name=f"kcp{nch}_{t}", bufs=4)
                nc.scalar.copy(out=pcp[:], in_=ps[:])
                rinv2 = rmsnorm_rinv2(pcp[:], f"kn{nch}_{t}")
                kh = pool.tile([128, 512], BF, tag="kh",
                               name=f"kh{nch}_{t}", bufs=3)
                t2 = pool.tile([128, 512], BF, tag="t2",
                               name=f"t2k{nch}_{t}", bufs=3)
                for hh in range(2):
                    srcp = pcp[:, hh * HD:(hh + 1) * HD]
                    nc.vector.scalar_tensor_tensor(
                        kh[:, hh * HD:(hh + 1) * HD], srcp,
                        rinv2[:, hh:hh + 1], ck_sb[:, t, :],
                        op0=ALU.mult, op1=ALU.mult)
                    nc.vector.scalar_tensor_tensor(
                        t2[:, hh * HD:(hh + 1) * HD]
                        .rearrange("p (a b) -> p a b", a=2),
                        _swap_ap(srcp, HALF), rinv2[:, hh:hh + 1],
                        sk_sb[:, t, :].rearrange("p (a b) -> p a b", a=2),
                        op0=ALU.mult, op1=ALU.mult)
                nc.gpsimd.tensor_add(kh[:], kh[:], t2[:])
                ptr = psP.tile([128, 512], BF, tag="tr", bufs=2,
                               name=f"ktr{nch}_{t}")
                for mm in range(4):
                    nc.tensor.transpose(ptr[:, mm * 128:(mm + 1) * 128],
                                        kh[:, mm * 128:(mm + 1) * 128],
                                        ident[:])
                nc.scalar.copy(
                    out=KT[:, nch * 4:(nch + 1) * 4, t * 128:(t + 1) * 128],
                    in_=ptr[:].rearrange("p (a b) -> p a b", a=4))

            mark('S1')
            # ============ S1: in_ln over 768 local tokens + transpose ======
            with tc.tile_pool(name="s1", bufs=2) as s1:
                for t in range(TL):
                    if t == 0:
                        xt = xt0
                    else:
                        xt = s1.tile([128, HID], BF, tag="xt", name=f"xt{t}",
                                     bufs=2)
                        nc.scalar.dma_start(xt[:], x_p.ap()[:, t, :])
                    rinv = rmsnorm_rinv(xt[:], HID, f"inln{t}")
                    ht = s1.tile([128, HID], BF, tag="ht", name=f"ht{t}", bufs=2)
                    for cch in range(5):
                        sl = slice(cch * 512, (cch + 1) * 512)
                        nc.vector.scalar_tensor_tensor(
                            ht[:, sl], xt[:, sl], rinv[:], w1_in_b[:, sl],
                            op0=ALU.mult, op1=ALU.mult)
                    for g in range(5):   # 4 k-chunks per transpose group
                        ptr = psP.tile([128, 512], BF, tag="tr", bufs=2,
                                       name=f"s1tr{t}_{g}")
                        for kk in range(4):
                            nc.tensor.transpose(
                                ptr[:, kk * 128:(kk + 1) * 128],
                                ht[:, (g * 4 + kk) * 128:(g * 4 + kk + 1) * 128],
                                ident[:])
                        dst = hT[:, g * 4:(g + 1) * 4, t * 128:(t + 1) * 128]
                        src_ = ptr[:].rearrange("p (a b) -> p a b", a=4)
                        nc.scalar.copy(out=dst, in_=src_)

            mark('S2')
            # ====== S2: Q projection, then scores+exp, then V (overlap) =====
            kt_mask = {0: 0, 1: 1, 4: 2, 5: 3}
            PRS = [[None] * TL for _ in range(NH)]
            with tc.tile_pool(name="s2", bufs=2) as s2:
                # --- K: per (nch, t): 2 heads norm+rope, transpose ---
                for nch in range(2):
                    for t in range(TL):
                        k_group(s2, nch, t)

                # --- Q: own tokens only (local tiles 4,5) ---
                for nch in range(4):
                    wqt = s2.tile([128, KH, 512], BF, tag="wst",
                                  name=f"wq{nch}", bufs=2)
                    nc.sync.dma_start(wqt[:], wq_p.ap()[:, :, nch * 512:(nch + 1) * 512])
                    for t in range(OT):
                        lt = 4 + t
                        ps = psP.tile([128, 512], F32, tag="mm", bufs=6,
                                      name=f"psq{nch}_{t}")
                        for k in range(KH):
                            nc.tensor.matmul(ps[:], hT[:, k, lt * 128:(lt + 1) * 128],
                                             wqt[:, k, :], start=(k == 0),
                                             stop=(k == KH - 1))
                        pcp = s2.tile([128, 512], BF, tag="pcp",
                                      name=f"qcp{nch}_{t}", bufs=4)
                        nc.scalar.copy(out=pcp[:], in_=ps[:])
                        rinv2 = rmsnorm_rinv2(pcp[:], f"qn{nch}_{t}")
                        qh = s2.tile([128, 512], BF, tag="qh",
                                     name=f"qh{nch}_{t}", bufs=3)
                        t2 = s2.tile([128, 512], BF, tag="t2",
                                     name=f"t2q{nch}_{t}", bufs=3)
                        for hh in range(2):
                            srcp = pcp[:, hh * HD:(hh + 1) * HD]
                            nc.vector.scalar_tensor_tensor(
                                qh[:, hh * HD:(hh + 1) * HD], srcp,
                                rinv2[:, hh:hh + 1], cq_sb[:, t, :],
                                op0=ALU.mult, op1=ALU.mult)
                            nc.vector.scalar_tensor_tensor(
                                t2[:, hh * HD:(hh + 1) * HD]
                                .rearrange("p (a b) -> p a b", a=2),
                                _swap_ap(srcp, HALF), rinv2[:, hh:hh + 1],
                                sq_sb[:, t, :].rearrange("p (a b) -> p a b", a=2),
                                op0=ALU.mult, op1=ALU.mult)
                        nc.gpsimd.tensor_add(qh[:], qh[:], t2[:])
                        ptr = psP.tile([128, 512], BF, tag="tr", bufs=2,
                                       name=f"qtr{nch}_{t}")
                        for mm in range(4):
                            nc.tensor.transpose(ptr[:, mm * 128:(mm + 1) * 128],
                                                qh[:, mm * 128:(mm + 1) * 128],
                                                ident[:])
                        nc.scalar.copy(
                            out=QT[:, nch * 4:(nch + 1) * 4, t * 128:(t + 1) * 128],
                            in_=ptr[:].rearrange("p (a b) -> p a b", a=4))

                # --- scores + exp + mask for all heads (PE overlaps V next) ---
                for h in range(NH):
                    md = h - (h % 2)      # KT m-tile base for kv head h//2
                    for kt in range(TL):
                        psc = psP.tile([128, TS], F32, tag="mm", bufs=6,
                                       name=f"psc{h}_{kt}")
                        for dh in range(2):
                            nc.tensor.matmul(
                                psc[:], KT[:, md + dh, kt * 128:(kt + 1) * 128],
                                QT[:, 2 * h + dh, :],
                                start=(dh == 0), stop=(dh == 1))
                        pr = sc_pool.tile([128, TS], BF, name=f"pr{h}_{kt}")
                        nc.scalar.activation(pr[:], psc[:], AF.Exp,
                                             scale=1.0 / 16.0)
                        if kt in (0, 1):
                            nc.vector.scalar_tensor_tensor(
                                pr[:], pr[:], pad_sb[:, kt:kt + 1],
                                masks[:, kt_mask[kt], :],
                                op0=ALU.mult, op1=ALU.mult)
                        elif kt in (2, 3):
                            nc.vector.tensor_scalar_mul(pr[:], pr[:],
                                                        pad_sb[:, kt:kt + 1])
                        else:
                            nc.vector.tensor_mul(pr[:], pr[:],
                                                 masks[:, kt_mask[kt], :])
                        PRS[h][kt] = pr

                # --- V projection (PE; exp/mask of scores overlaps) ---
                for nch in range(2):
                    wvt = s2.tile([128, KH, 512], BF, tag="wst",
                                  name=f"wv{nch}", bufs=2)
                    nc.sync.dma_start(wvt[:], wv_p.ap()[:, :, nch * 512:(nch + 1) * 512])
                    for t in range(TL):
                        ps = psP.tile([128, 512], F32, tag="mm", bufs=6,
                                      name=f"psv{nch}_{t}")
                        for k in range(KH):
                            nc.tensor.matmul(ps[:], hT[:, k, t * 128:(t + 1) * 128],
                                             wvt[:, k, :], start=(k == 0),
                                             stop=(k == KH - 1))
                        nc.scalar.copy(out=V[t][:, nch * 512:(nch + 1) * 512],
                                       in_=ps[:])

            mark('S3')
            # ===== S3: attention PV + output (scores already computed) =====
            pool_a.release()   # hT + wk + rope tables
            # MLP weight stream pool (persists through S5; allocated before
            # pool_b so the right-side pool stack pops in LIFO order)
            pool_s = tc.alloc_tile_pool(name="ps5", bufs=1, side="right")
            # wo is streamed in [KA, 512] n-chunks (first preloaded now);
            # MLP weight stream starts prefetching here too (DMA idle).
            pool_b = tc.alloc_tile_pool(name="pb", bufs=1, side="right")
            attnT = pool_b.tile([128, KA, TS], BF)
            wo_t = {}

            def load_wo(n):
                wo_t[n] = pool_b.tile([128, KA, 512], BF, tag="wo",
                                      name=f"wo{n}", bufs=3)
                nc.sync.dma_start(wo_t[n][:], wo_p.ap()[:, :, n * 512:(n + 1) * 512])

            for _n in range(3):
                load_wo(_n)
            # post-attn / pre-ff norm weights: broadcast now so S4's SP queue
            # only carries the remaining wo chunks
            w1_pa_b = pool_b.tile([128, HID], BF)
            w1_pf_b = pool_b.tile([128, HID], BF)
            _bcast_row(nc, w1_pa_b, w1_pa, HID)
            _bcast_row(nc, w1_pf_b, w1_pf, HID)
            wgu_t = {}
            wd_t = {}

            def load_m(m):
                wgu_t[m] = pool_s.tile([128, 2, KH, 128], BF, tag="wgu",
                                       name=f"wgu{m}", bufs=PREF + 1)
                nc.sync.dma_start(wgu_t[m][:], wgu_f.ap()[:, m])
                wd_t[m] = pool_s.tile([128, HID], BF, tag="wd",
                                      name=f"wd{m}", bufs=GM + PREF + 1)
                nc.gpsimd.dma_start(wd_t[m][:], wd_f.ap()[:, m])

            for m in range(PREF):
                load_m(m)
            with tc.tile_pool(name="s3", bufs=2) as s3:
                for qt in range(OT):
                  for h in range(NH):
                    prs = PRS[h]
                    if True:
                        po = psP.tile([128, HD + 1], F32, tag="mm", bufs=6,
                                      name=f"po{h}_{qt}")
                        for kt in range(TL):
                            sl = slice(qt * 128, (qt + 1) * 128)
                            nc.tensor.matmul(po[:, 0:HD], prs[kt][:, sl],
                                             V[kt][:, (h // 2) * HD:(h // 2 + 1) * HD],
                                             start=(kt == 0), stop=(kt == TL - 1))
                        for kt in range(TL):
                            sl = slice(qt * 128, (qt + 1) * 128)
                            nc.tensor.matmul(po[:, HD:HD + 1], prs[kt][:, sl],
                                             ones_t[:], start=(kt == 0),
                                             stop=(kt == TL - 1))
                        rec = s3.tile([128, 1], F32, tag="rec",
                                      name=f"rec{h}_{qt}")
                        nc.vector.reciprocal(rec[:], po[:, HD:HD + 1])
                        an = s3.tile([128, HD], BF, tag="an",
                                     name=f"an{h}_{qt}")
                        nc.vector.tensor_scalar_mul(an[:], po[:, 0:HD], rec[:])
                        ptr = psP.tile([128, HD], BF, tag="tr", bufs=2,
                                       name=f"atr{h}_{qt}")
                        for mm in range(2):
                            nc.tensor.transpose(ptr[:, mm * 128:(mm + 1) * 128],
                                                an[:, mm * 128:(mm + 1) * 128],
                                                ident[:])
                        nc.scalar.copy(
                            out=attnT[:, 2 * h:2 * h + 2, qt * 128:(qt + 1) * 128],
                            in_=ptr[:].rearrange("p (a b) -> p a b", a=2))

            mark('S4')
            # ======== S4: wo + post_attn + residual + pre_ff norm ========
            sc_pool.release()   # probs
            pool_a2.release()   # frees KT/QT/V/masks
            # S4-S5 persistent: h2T (pre-ff normed own tokens, transposed)
            pool_c = tc.alloc_tile_pool(name="pc", bufs=1)
            h2T = pool_c.tile([128, KH, TS], BF)
            with tc.tile_pool(name="s4", bufs=2) as s4:
                xos = [s4.tile([128, HID], F32, tag="xo", name=f"xo{t}",
                               bufs=2) for t in range(OT)]
                for t in range(OT):
                    nc.scalar.dma_start(xos[t][:], x_own.ap()[:, t, :])
                ao32s = [s4.tile([128, HID], F32, tag="ao32",
                                 name=f"ao32_{t}", bufs=2) for t in range(OT)]
                stats_a = [nrm.tile([128, 5, 6], F32, tag="nst",
                                    name=f"pan{t}_st") for t in range(OT)]
                # n-outer so each wo chunk is loaded once and used for both t
                for n in range(5):
                    if n + 3 < 5:
                        load_wo(n + 3)
                    wo_n = wo_t.pop(n)
                    for t in range(OT):
                        pw = psP.tile([128, 512], F32, tag="mm", bufs=6,
                                      name=f"pw{t}_{n}")
                        for m in range(KA):
                            nc.tensor.matmul(
                                pw[:], attnT[:, m, t * 128:(t + 1) * 128],
                                wo_n[:, m, :],
                                start=(m == 0), stop=(m == KA - 1))
                        nc.scalar.copy(out=ao32s[t][:, n * 512:(n + 1) * 512],
                                       in_=pw[:])
                        nc.vector.bn_stats(
                            out=stats_a[t][:, n, :],
                            in_=ao32s[t][:, n * 512:(n + 1) * 512])
                for t in range(OT):
                    # t=0 chain runs on DVE, t=1 on Pool so they interleave
                    e0, e1 = (nc.vector, nc.gpsimd) if t == 0 else \
                             (nc.gpsimd, nc.vector)
                    ao32 = ao32s[t]
                    rinv_a = rinv_from_stats(stats_a[t], f"pan{t}")
                    xo = xos[t]
                    x2 = s4.tile([128, HID], F32, tag="x2", name=f"x2_{t}",
                                 bufs=1)
                    stats_f = nrm.tile([128, 5, 6], F32, tag="nst",
                                       name=f"pff{t}_st")
                    for n in range(5):
                        sl = slice(n * 512, (n + 1) * 512)
                        e0.scalar_tensor_tensor(
                            x2[:, sl], ao32[:, sl], rinv_a[:], w1_pa_b[:, sl],
                            op0=ALU.mult, op1=ALU.mult)
                        e1.tensor_add(x2[:, sl], x2[:, sl], xo[:, sl])
                        nc.vector.bn_stats(out=stats_f[:, n, :], in_=x2[:, sl])
                    nc.sync.dma_start(x2_spill[t * 128:(t + 1) * 128, :], x2[:])
                    rinv_f = rinv_from_stats(stats_f, f"pff{t}")
                    h2 = s4.tile([128, HID], BF, tag="h2", name=f"h2_{t}",
                                 bufs=1)
                    for g in range(5):
                        sl = slice(g * 512, (g + 1) * 512)
                        e0.scalar_tensor_tensor(
                            h2[:, sl], x2[:, sl], rinv_f[:],
                            w1_pf_b[:, sl], op0=ALU.mult, op1=ALU.mult)
                        ptr = psP.tile([128, 512], BF, tag="tr", bufs=2,
                                       name=f"s4tr{t}_{g}")
                        for kk in range(4):
                            nc.tensor.transpose(
                                ptr[:, kk * 128:(kk + 1) * 128],
                                h2[:, (g * 4 + kk) * 128:(g * 4 + kk + 1) * 128],
                                ident[:])
                        nc.vector.tensor_copy(
                            h2T[:, g * 4:(g + 1) * 4, t * 128:(t + 1) * 128],
                            ptr[:].rearrange("p (a b) -> p a b", a=4))
            pool_b.release()   # attnT + wo

            mark('S5')
            # ===== S5: sequence-parallel MLP, weights streamed in m-chunks ==
            pool_acc = tc.alloc_tile_pool(name="pacc", bufs=1)
            accs = [pool_acc.tile([128, HID], F32, name=f"acc{t}")
                    for t in range(OT)]
            # S6 inputs: fetch during the MLP so the tail chain is short
            w1_po_b = pool_acc.tile([128, HID], F32)
            _bcast_row(nc, w1_po_b, w1_po, HID)
            x2ls = [pool_acc.tile([128, HID], F32, name=f"x2l{t}")
                    for t in range(OT)]
            o32s = [pool_acc.tile([128, HID], F32, name=f"o32_{t}")
                    for t in range(OT)]
            for t in range(OT):
                nc.scalar.dma_start(x2ls[t][:],
                                    x2_spill[t * 128:(t + 1) * 128, :])

            def s6_tile(t):
                # post_ff norm + residual + output for one token tile
                e0, e1 = (nc.vector, nc.gpsimd) if t == 0 else \
                         (nc.gpsimd, nc.vector)
                rinv_o = rmsnorm_rinv(accs[t][:], HID, f"pon{t}")
                o32 = o32s[t]
                for n in range(4):
                    sl = slice(n * 640, (n + 1) * 640)
                    e0.scalar_tensor_tensor(
                        o32[:, sl], accs[t][:, sl], rinv_o[:],
                        w1_po_b[:, sl], op0=ALU.mult, op1=ALU.mult)
                    e1.tensor_add(o32[:, sl], o32[:, sl], x2ls[t][:, sl])
                    nc.sync.dma_start(
                        out_shard.ap()[t * 128:(t + 1) * 128, sl],
                        o32[:, sl])

            with tc.tile_pool(name="s5", bufs=2) as s5:
                actT_t = {}

                def down_group(g0, m, tt):
                    first = (g0 == 0)
                    for ng in ((0, 1), (2, 3), (4,)):
                        pds = {n: psP.tile([128, 512], F32, tag="mm",
                                           bufs=6, name=f"pd{m}_{tt}_{n}")
                               for n in ng}
                        for j in range(GM):
                            mj = g0 + j
                            for n in ng:
                                nc.tensor.matmul(
                                    pds[n][:],
                                    actT_t[mj][:, tt * 128:(tt + 1) * 128],
                                    wd_t[mj][:, n * 512:(n + 1) * 512],
                                    start=(j == 0), stop=(j == GM - 1))
                        for n in ng:
                            dst = accs[tt][:, n * 512:(n + 1) * 512]
                            if first:
                                if n % 2 == 0:
                                    nc.vector.tensor_copy(dst, pds[n][:])
                                else:
                                    nc.scalar.copy(out=dst, in_=pds[n][:])
                            else:
                                if n % 2 == 0:
                                    nc.vector.tensor_add(dst, dst, pds[n][:])
                                else:
                                    nc.gpsimd.tensor_add(dst, dst, pds[n][:])

                for m in range(MIF):
                    if m + PREF < MIF:
                        load_m(m + PREF)
                    wgu = wgu_t.pop(m)
                    pg = psP.tile([128, TS], F32, tag="mm", bufs=6,
                                  name=f"pg{m}")
                    pu = psP.tile([128, TS], F32, tag="mm", bufs=6,
                                  name=f"pu{m}")
                    gsc = s5.tile([128, TS], F32, tag="gsc",
                                  name=f"gsc{m}", bufs=3)
                    at = s5.tile([128, TS], BF, tag="act",
                                 name=f"act{m}", bufs=GM + 2)
                    if m < 8:
                        # per-token-tile so tile-0 work overlaps S4's t=1 tail
                        for tt in range(OT):
                            sl = slice(tt * 128, (tt + 1) * 128)
                            for k in range(KH):
                                nc.tensor.matmul(pg[:, sl], wgu[:, 0, k, :],
                                                 h2T[:, k, sl],
                                                 start=(k == 0),
                                                 stop=(k == KH - 1))
                            for k in range(KH):
                                nc.tensor.matmul(pu[:, sl], wgu[:, 1, k, :],
                                                 h2T[:, k, sl],
                                                 start=(k == 0),
                                                 stop=(k == KH - 1))
                            nc.scalar.activation(gsc[:, sl], pg[:, sl],
                                                 AF.Gelu_apprx_tanh)
                            nc.vector.tensor_mul(at[:, sl], gsc[:, sl],
                                                 pu[:, sl])
                    else:
                        for k in range(KH):
                            nc.tensor.matmul(pg[:], wgu[:, 0, k, :],
                                             h2T[:, k, :],
                                             start=(k == 0), stop=(k == KH - 1))
                        for k in range(KH):
                            nc.tensor.matmul(pu[:], wgu[:, 1, k, :],
                                             h2T[:, k, :],
                                             start=(k == 0), stop=(k == KH - 1))
                        nc.scalar.activation(gsc[:], pg[:],
                                             AF.Gelu_apprx_tanh)
                        nc.vector.tensor_mul(at[:], gsc[:], pu[:])
                    actT_t[m] = at
                    if m % GM != GM - 1:
                        continue
                    # ---- down for this group of GM m-chunks ----
                    g0 = m - GM + 1
                    if m == MIF - 1:
                        # last group: finish each token tile's output inline
                        down_group(g0, m, 0)
                        s6_tile(0)
                        down_group(g0, m, 1)
                        s6_tile(1)
                    else:
                        for tt in range(OT):
                            down_group(g0, m, tt)
                    for j in range(GM):
                        del actT_t[g0 + j], wd_t[g0 + j]
            pool_s.release()

            mark('S6')
            pool_acc.release()
            pool_c.release()

    nc.compile()
    return nc


_NC_CACHE = None


def _get_nc():
    global _NC_CACHE
    if _NC_CACHE is None:
        _NC_CACHE = build_nc()
    return _NC_CACHE


def make_in_maps(hidden_states, position_ids, wq, wk, wv, wo, q_ln_w, k_ln_w,
                 in_ln_w, post_attn_ln_w, pre_ff_ln_w, post_ff_ln_w,
                 w_gate, w_up, w_down):
    bf16 = ml_dtypes.bfloat16
    f32 = np.float32
    x = np.asarray(hidden_states, f32).reshape(S, HID)
    pos = np.asarray(position_ids).reshape(S).astype(np.float64)

    inv_freq = 1.0 / (BASE ** (np.arange(0, HD, 2, dtype=np.float64) / HD))
    w1q = 1.0 + np.asarray(q_ln_w, f32)
    w1k = 1.0 + np.asarray(k_ln_w, f32)

    def rope_tabs(p, w1):
        emb = np.concatenate([p[:, None] * inv_freq[None, :]] * 2, axis=1)
        cos = np.cos(emb).astype(f32)
        sin = np.sin(emb).astype(f32)
        w1sw = np.concatenate([w1[HALF:], w1[:HALF]])
        sgn = np.concatenate([-np.ones(HALF, f32), np.ones(HALF, f32)])
        n = len(p)
        c = (cos * w1[None, :]).astype(bf16).reshape(n // 128, 128, HD)
        s_ = (sin * (w1sw * sgn)[None, :]).astype(bf16).reshape(n // 128, 128, HD)
        return (np.ascontiguousarray(c.transpose(1, 0, 2)),
                np.ascontiguousarray(s_.transpose(1, 0, 2)))

    def pack(w, kt, n):
        return np.ascontiguousarray(
            np.asarray(w, f32).reshape(kt, 128, n).transpose(1, 0, 2)).astype(bf16)

    # gate+up interleaved [128, MIF, 2, KH, 128]
    def pack_gu(w):
        # [HID, INTER] -> [KH, 128, MIF, 128] -> [128, MIF, KH, 128]
        return np.asarray(w, f32).reshape(KH, 128, MIF, 128).transpose(1, 2, 0, 3)

    wgu = np.stack([pack_gu(w_gate), pack_gu(w_up)], axis=2)  # [128,MIF,2,KH,128]
    wd_pk = np.ascontiguousarray(
        np.asarray(w_down, f32).reshape(MIF, 128, HID).transpose(1, 0, 2))

    common = {
        "wq_p": pack(wq, KH, NH * HD),
        "wk_p": pack(wk, KH, NKV * HD),
        "wv_p": pack(wv, KH, NKV * HD),
        "wo_p": pack(wo, KA, HID),
        "wgu_f": np.ascontiguousarray(wgu).astype(bf16),
        "wd_f": wd_pk.astype(bf16),
        "w1_in": (1.0 + np.asarray(in_ln_w, f32)).astype(bf16),
        "w1_pa": (1.0 + np.asarray(post_attn_ln_w, f32)).astype(bf16),
        "w1_pf": (1.0 + np.asarray(pre_ff_ln_w, f32)).astype(bf16),
        "w1_po": 1.0 + np.asarray(post_ff_ln_w, f32),
    }
    in_maps = []
    for c in range(NC_):
        lo = c * TS - HALO
        xh = np.zeros((LT, HID), f32)
        src_lo = max(0, lo)
        xh[src_lo - lo:] = x[src_lo:lo + LT]
        x_pk = np.ascontiguousarray(
            xh.reshape(TL, 128, HID).transpose(1, 0, 2)).astype(bf16)
        x_ow = np.ascontiguousarray(
            x[c * TS:(c + 1) * TS].reshape(OT, 128, HID).transpose(1, 0, 2))
        padv = (np.arange(lo, lo + LT) >= 0).astype(f32)
        pad_pk = np.ascontiguousarray(padv.reshape(TL, 128).T)
        kpos = np.where(np.arange(lo, lo + LT) >= 0,
                        pos[np.clip(np.arange(lo, lo + LT), 0, S - 1)], 0.0)
        qpos = pos[c * TS:(c + 1) * TS]
        ckw, skw = rope_tabs(kpos, w1k)
        cqw, sqw = rope_tabs(qpos, w1q)
        in_maps.append({
            "x_p": x_pk,
            "x_own": x_ow,
            "pad_p": pad_pk,
            "cq": cqw, "sq": sqw, "ck": ckw, "sk": skw,
            **common,
        })
    return in_maps


def kernel(**inputs):
    in_maps = make_in_maps(**inputs)
    nc = _get_nc()
    res = run_bass_kernel_spmd(nc, in_maps, core_ids=list(range(NC_)))
    out = np.concatenate([res.results[c]["out_shard"] for c in range(NC_)], axis=0)
    return out.reshape(1, S, HID).astype(np.float32)


# revision 22
# speedup vs baseline: 1.0919x; 1.0303x over previous
"""Gemma3 decoder layer on 8 Trainium2 NeuronCores (Bass/Tile), v3.

Sharding (per core c): fully sequence-parallel, ZERO collectives.
  - attention: core c owns tokens [256c, 256c+256) and receives a 512-token
    halo (host-side sharding): x_halo = x[256c-512 : 256c+256] (zero-padded
    for c<2). All attn weights (wq/wk/wv/wo) replicated; K/V computed for all
    768 local tokens, Q only for the 256 own tokens. Sliding-window (512)
    attention is then fully local. Pad keys masked via per-core pad mask.
  - MLP: sequence-parallel too. Each core runs the FULL 10240-dim MLP for its
    own 256 tokens, streaming gate/up/down weights (157MB bf16) from DRAM in
    128-inter-dim chunks, double-buffered under the matmul stream. Down
    partials accumulate in PSUM per 8-chunk group, then into an f32 SBUF
    accumulator (DVE/Pool adds). No AllGather, no ReduceScatter, no DRAM
    spill of h2/x2.
  - norms/residual: token-local.
Matmuls in bf16 (fp32 PSUM accumulation); norms/softmax/residual fp32.
All weights host-prepacked into SBUF layout (contiguous 5-10KB/partition DMAs).
"""
import sys

if "/opt/trn_rl_repo" not in sys.path:
    sys.path.insert(0, "/opt/trn_rl_repo")

import numpy as np
import ml_dtypes

import concourse.bass as bass
import concourse.mybir as mybir
import concourse.tile as tile
from concourse import bacc
from concourse.bass_utils import run_bass_kernel_spmd
from concourse.masks import make_identity

dt = mybir.dt
AF = mybir.ActivationFunctionType
ALU = mybir.AluOpType
BF = dt.bfloat16
F32 = dt.float32

HID, NH, NKV, HD, INTER = 2560, 8, 4, 256, 10240
WIN, EPS, BASE = 512, 1e-6, 10000.0
S = 2048
NC_ = 8
TS = S // NC_              # 256 own tokens per core
HALO = 512
LT = TS + HALO             # 768 local tokens (halo + own)
TL = LT // 128             # 6 local token tiles
OT = TS // 128             # 2 own token tiles
KH = HID // 128            # 20 hidden-dim k-chunks
KA = (NH * HD) // 128      # 16 attn-dim chunks
MIF = INTER // 128         # 80 inter m-chunks (full, per core)
GM = 4                     # m-chunks per down-accumulation group
NGRP = MIF // GM           # 20 groups
PREF = 3                   # stream prefetch depth (m-chunks)
HALF = HD // 2


def _bcast_row(nc, sbuf_tile, dram_t, width):
    a = dram_t.ap()
    nc.sync.dma_start(sbuf_tile[:], bass.AP(
        tensor=a.tensor, offset=a.offset, ap=[[0, 128], [1, width]]))


def _swap_ap(src_ap, half):
    """View [128, 2*half] with halves swapped, as [128, 2, half]."""
    return bass.AP(tensor=src_ap.tensor, offset=src_ap.offset + half,
                   ap=[list(src_ap.ap[0]), [-half, 2], [1, half]])


def build_nc(sim=False):
    nc = bacc.Bacc("TRN2", target_bir_lowering=False, debug=False,
                   enable_asserts=True, num_devices=1 if sim else NC_)

    # ---- inputs (host-prepacked layouts) ----
    x_p = nc.dram_tensor("x_p", [128, TL, HID], BF, kind="ExternalInput")
    x_own = nc.dram_tensor("x_own", [128, OT, HID], F32, kind="ExternalInput")
    pad_p = nc.dram_tensor("pad_p", [128, TL], F32, kind="ExternalInput")
    cq = nc.dram_tensor("cq", [128, OT, HD], BF, kind="ExternalInput")
    sq = nc.dram_tensor("sq", [128, OT, HD], BF, kind="ExternalInput")
    ck = nc.dram_tensor("ck", [128, TL, HD], BF, kind="ExternalInput")
    sk = nc.dram_tensor("sk", [128, TL, HD], BF, kind="ExternalInput")
    wq_p = nc.dram_tensor("wq_p", [128, KH, NH * HD], BF, kind="ExternalInput")
    wk_p = nc.dram_tensor("wk_p", [128, KH, NKV * HD], BF, kind="ExternalInput")
    wv_p = nc.dram_tensor("wv_p", [128, KH, NKV * HD], BF, kind="ExternalInput")
    wo_p = nc.dram_tensor("wo_p", [128, KA, HID], BF, kind="ExternalInput")
    # gate+up interleaved: [kpart, m-chunk, {g,u}, k-chunk, m-col]
    wgu_f = nc.dram_tensor("wgu_f", [128, MIF, 2, KH, 128], BF,
                           kind="ExternalInput")
    # down: [inter-part-within-chunk, m-chunk, hid]
    wd_f = nc.dram_tensor("wd_f", [128, MIF, HID], BF, kind="ExternalInput")
    w1_in = nc.dram_tensor("w1_in", [HID], BF, kind="ExternalInput")
    w1_pa = nc.dram_tensor("w1_pa", [HID], BF, kind="ExternalInput")
    w1_pf = nc.dram_tensor("w1_pf", [HID], BF, kind="ExternalInput")
    w1_po = nc.dram_tensor("w1_po", [HID], F32, kind="ExternalInput")
    out_shard = nc.dram_tensor("out_shard", [TS, HID], F32, kind="ExternalOutput")

    stages = {}
    nc._stage_ids = stages

    def mark(name):
        stages[name] = nc.next_id()

    with tile.TileContext(nc) as tc:
        with (
            tc.tile_pool(name="dram", bufs=1, space="DRAM") as dram,
            tc.tile_pool(name="glob", bufs=1) as glob,
            tc.tile_pool(name="nrm", bufs=3) as nrm,
            tc.tile_pool(name="psP", bufs=1, space="PSUM") as psP,
        ):
            x2_spill = dram.tile([TS, HID], F32)
            ident = glob.tile([128, 128], BF)
            make_identity(nc, ident[:])
            eps_t = glob.tile([128, 1], F32)
            nc.vector.memset(eps_t[:], EPS)
            ones_t = glob.tile([128, 1], BF)
            nc.vector.memset(ones_t[:], 1.0)
            # warm the activation-function tables while the first DMAs run
            warm = glob.tile([128, 1], F32)
            for af in (AF.Sqrt, AF.Exp, AF.Gelu_apprx_tanh):
                nc.scalar.activation(warm[:], eps_t[:], af)

            # attention-phase residents: a2 lives through S3, a1 dies at S2 end
            pool_a2 = tc.alloc_tile_pool(name="pa2", bufs=1)
            KT = pool_a2.tile([128, NKV * 2, LT], BF)  # K^T [d, tok]
            QT = pool_a2.tile([128, KA, TS], BF)       # Q^T [d, tok]
            V = [pool_a2.tile([128, NKV * HD], BF, name=f"V{t}")
                 for t in range(TL)]
            sc_pool = tc.alloc_tile_pool(name="sc", bufs=1)
            pool_a = tc.alloc_tile_pool(name="pa1", bufs=1)
            # first token tile + in_ln weight: issue DMAs before the wk/rope
            # loads so S1's first norm chain starts ASAP
            xt0 = pool_a.tile([128, HID], BF)
            nc.scalar.dma_start(xt0[:], x_p.ap()[:, 0, :])
            w1_in_b = pool_a.tile([128, HID], BF)
            _bcast_row(nc, w1_in_b, w1_in, HID)
            hT = pool_a.tile([128, KH, LT], BF)       # h^T
            wk_sb = pool_a.tile([128, KH, NKV * HD], BF)
            nc.sync.dma_start(wk_sb[:, :, 0:512], wk_p.ap()[:, :, 0:512])
            nc.sync.dma_start(wk_sb[:, :, 512:1024], wk_p.ap()[:, :, 512:1024])
            cq_sb = pool_a.tile([128, OT, HD], BF)
            sq_sb = pool_a.tile([128, OT, HD], BF)
            ck_sb = pool_a.tile([128, TL, HD], BF)
            sk_sb = pool_a.tile([128, TL, HD], BF)
            nc.sync.dma_start(cq_sb[:], cq.ap())
            nc.sync.dma_start(sq_sb[:], sq.ap())
            nc.sync.dma_start(ck_sb[:], ck.ap())
            nc.sync.dma_start(sk_sb[:], sk.ap())

            # window/causal masks [128 ko, 256 qo] for k-tiles 0,1,4,5
            # valid iff 0 <= (512+qo) - (128*kt+ko) < 512
            masks = pool_a2.tile([128, 4, TS], BF)
            for i, kt in enumerate((0, 1, 4, 5)):
                mk = masks[:, i, :]
                nc.gpsimd.memset(mk, 1.0)
                if kt in (0, 1):
                    # keep where ko + (128*kt - 1) - qo >= 0
                    nc.gpsimd.affine_select(
                        out=mk, in_=mk, compare_op=ALU.is_ge, fill=0.0,
                        base=128 * kt - 1, pattern=[[-1, TS]],
                        channel_multiplier=1)
                else:
                    # keep where qo - ko + (512 - 128*kt) >= 0
                    nc.gpsimd.affine_select(
                        out=mk, in_=mk, compare_op=ALU.is_ge, fill=0.0,
                        base=512 - 128 * kt, pattern=[[1, TS]],
                        channel_multiplier=-1)
            pad_sb = pool_a2.tile([128, TL], F32)
            nc.sync.dma_start(pad_sb[:], pad_p.ap())

            def rinv_from_stats(stats, name):
                mv = nrm.tile([128, 2], F32, tag="nmv", name=f"{name}_mv")
                nc.vector.bn_aggr(out=mv[:], in_=stats[:])
                ms = nrm.tile([128, 1], F32, tag="nms", name=f"{name}_ms")
                nc.vector.scalar_tensor_tensor(ms[:], mv[:, 0:1], mv[:, 0:1],
                                               mv[:, 1:2], op0=ALU.mult,
                                               op1=ALU.add)
                nc.vector.tensor_scalar_add(ms[:], ms[:], EPS)
                rec = nrm.tile([128, 1], F32, tag="nrc", name=f"{name}_rc")
                nc.vector.reciprocal(rec[:], ms[:])
                rinv = nrm.tile([128, 1], F32, tag="nrv", name=f"{name}_rv")
                nc.scalar.activation(rinv[:], rec[:], AF.Sqrt)
                return rinv

            def rmsnorm_rinv(src_ap, d, name, rows=128):
                """rinv[p,1] = 1/sqrt(mean(src^2)+EPS) via bn_stats."""
                nsub = max(1, d // 512)
                stats = nrm.tile([128, nsub, 6], F32, tag="nst", name=f"{name}_st")
                if nsub > 1:
                    view = src_ap.rearrange("p (s f) -> p s f", s=nsub)
                    for i in range(nsub):
                        nc.vector.bn_stats(out=stats[:rows, i, :],
                                           in_=view[:, i, :])
                else:
                    nc.vector.bn_stats(out=stats[:rows, 0, :], in_=src_ap)
                mv = nrm.tile([128, 2], F32, tag="nmv", name=f"{name}_mv")
                nc.vector.bn_aggr(out=mv[:rows], in_=stats[:rows])
                ms = nrm.tile([128, 1], F32, tag="nms", name=f"{name}_ms")
                nc.vector.scalar_tensor_tensor(ms[:rows], mv[:rows, 0:1],
                                               mv[:rows, 0:1], mv[:rows, 1:2],
                                               op0=ALU.mult, op1=ALU.add)
                nc.vector.tensor_scalar_add(ms[:rows], ms[:rows], EPS)
                rec = nrm.tile([128, 1], F32, tag="nrc", name=f"{name}_rc")
                nc.vector.reciprocal(rec[:rows], ms[:rows])
                rinv = nrm.tile([128, 1], F32, tag="nrv", name=f"{name}_rv")
                nc.scalar.activation(rinv[:rows], rec[:rows], AF.Sqrt)
                return rinv

            def rmsnorm_rinv2(src_ap, name):
                """Two per-head rinvs for a [128, 512] psum (2 heads of 256).
                Returns [128, 2] f32 tile."""
                stats = nrm.tile([128, 2, 6], F32, tag="hst", name=f"{name}_st")
                view = src_ap.rearrange("p (s f) -> p s f", s=2)
                for i in range(2):
                    nc.vector.bn_stats(out=stats[:, i, :], in_=view[:, i, :])
                mv = nrm.tile([128, 2, 2], F32, tag="hmv", name=f"{name}_mv")
                for i in range(2):
                    nc.vector.bn_aggr(out=mv[:, i, :], in_=stats[:, i, :])
                ms = nrm.tile([128, 2], F32, tag="hms", name=f"{name}_ms")
                nc.vector.tensor_mul(ms[:], mv[:, :, 0], mv[:, :, 0])
                nc.vector.tensor_add(ms[:], ms[:], mv[:, :, 1])
                nc.vector.tensor_scalar_add(ms[:], ms[:], EPS)
                rec = nrm.tile([128, 2], F32, tag="hrc", name=f"{name}_rc")
                nc.vector.reciprocal(rec[:], ms[:])
                rinv = nrm.tile([128, 2], F32, tag="hrv", name=f"{name}_rv")
                nc.scalar.activation(rinv[:], rec[:], AF.Sqrt)
                return rinv

            def k_group(pool, nch, t):
                ps = psP.tile([128, 512], F32, tag="mm", bufs=6,
                              name=f"psk{nch}_{t}")
                for k in range(KH):
                    nc.tensor.matmul(ps[:], hT[:, k, t * 128:(t + 1) * 128],
                                     wk_sb[:, k, nch * 512:(nch + 1) * 512],
                                     start=(k == 0), stop=(k == KH - 1))
                pcp = pool.tile([128, 512], BF, tag="pcp",
                                name=f"kcp{nch}_{t}", bufs=4)
                nc.scalar.copy(out=pcp[:], in_=ps[:])
                rinv2 = rmsnorm_rinv2(pcp[:], f"kn{nch}_{t}")
                kh = pool.tile([128, 512], BF, tag="kh",
                               name=f"kh{nch}_{t}", bufs=3)
                t2 = pool.tile([128, 512], BF, tag="t2",
                               name=f"t2k{nch}_{t}", bufs=3)
                for hh in range(2):
                    srcp = pcp[:, hh * HD:(hh + 1) * HD]
                    nc.vector.scalar_tensor_tensor(
                        kh[:, hh * HD:(hh + 1) * HD], srcp,
                        rinv2[:, hh:hh + 1], ck_sb[:, t, :],
                        op0=ALU.mult, op1=ALU.mult)
                    nc.vector.scalar_tensor_tensor(
                        t2[:, hh * HD:(hh + 1) * HD]
                        .rearrange("p (a b) -> p a b", a=2),
                        _swap_ap(srcp, HALF), rinv2[:, hh:hh + 1],
                        sk_sb[:, t, :].rearrange("p (a b) -> p a b", a=2),
                        op0=ALU.mult, op1=ALU.mult)
                nc.gpsimd.tensor_add(kh[:], kh[:], t2[:])
                ptr = psP.tile([128, 512], BF, tag="tr", bufs=2,
                               name=f"ktr{nch}_{t}")
                for mm in range(4):
                    nc.tensor.transpose(ptr[:, mm * 128:(mm + 1) * 128],
                                        kh[:, mm * 128:(mm + 1) * 128],
                                        ident[:])
                nc.scalar.copy(
                    out=KT[:, nch * 4:(nch + 1) * 4, t * 128:(t + 1) * 128],
                    in_=ptr[:].rearrange("p (a b) -> p a b", a=4))

            mark('S1')
            # ============ S1: in_ln over 768 local tokens + transpose ======
            with tc.tile_pool(name="s1", bufs=2) as s1:
                for t in range(TL):
                    if t == 0:
                        xt = xt0
                    else:
                        xt = s1.tile([128, HID], BF, tag="xt", name=f"xt{t}",
                                     bufs=2)
                        nc.scalar.dma_start(xt[:], x_p.ap()[:, t, :])
                    rinv = rmsnorm_rinv(xt[:], HID, f"inln{t}")
                    ht = s1.tile([128, HID], BF, tag="ht", name=f"ht{t}", bufs=2)
                    for cch in range(5):
                        sl = slice(cch * 512, (cch + 1) * 512)
                        nc.vector.scalar_tensor_tensor(
                            ht[:, sl], xt[:, sl], rinv[:], w1_in_b[:, sl],
                            op0=ALU.mult, op1=ALU.mult)
                    for g in range(5):   # 4 k-chunks per transpose group
                        ptr = psP.tile([128, 512], BF, tag="tr", bufs=2,
                                       name=f"s1tr{t}_{g}")
                        for kk in range(4):
                            nc.tensor.transpose(
                                ptr[:, kk * 128:(kk + 1) * 128],
                                ht[:, (g * 4 + kk) * 128:(g * 4 + kk + 1) * 128],
                                ident[:])
                        dst = hT[:, g * 4:(g + 1) * 4, t * 128:(t + 1) * 128]
                        src_ = ptr[:].rearrange("p (a b) -> p a b", a=4)
                        nc.scalar.copy(out=dst, in_=src_)

            mark('S2')
            # ====== S2: Q projection, then scores+exp, then V (overlap) =====
            kt_mask = {0: 0, 1: 1, 4: 2, 5: 3}
            PRS = [[None] * TL for _ in range(NH)]
            with tc.tile_pool(name="s2", bufs=2) as s2:
                # --- K: per (nch, t): 2 heads norm+rope, transpose ---
                for nch in range(2):
                    for t in range(TL):
                        k_group(s2, nch, t)

                # --- Q: own tokens only (local tiles 4,5) ---
                for nch in range(4):
                    wqt = s2.tile([128, KH, 512], BF, tag="wst",
                                  name=f"wq{nch}", bufs=2)
                    nc.sync.dma_start(wqt[:], wq_p.ap()[:, :, nch * 512:(nch + 1) * 512])
                    for t in range(OT):
                        lt = 4 + t
                        ps = psP.tile([128, 512], F32, tag="mm", bufs=6,
                                      name=f"psq{nch}_{t}")
                        for k in range(KH):
                            nc.tensor.matmul(ps[:], hT[:, k, lt * 128:(lt + 1) * 128],
                                             wqt[:, k, :], start=(k == 0),
                                             stop=(k == KH - 1))
                        pcp = s2.tile([128, 512], BF, tag="pcp",
                                      name=f"qcp{nch}_{t}", bufs=4)
                        nc.scalar.copy(out=pcp[:], in_=ps[:])
                        rinv2 = rmsnorm_rinv2(pcp[:], f"qn{nch}_{t}")
                        qh = s2.tile([128, 512], BF, tag="qh",
                                     name=f"qh{nch}_{t}", bufs=3)
                        t2 = s2.tile([128, 512], BF, tag="t2",
                                     name=f"t2q{nch}_{t}", bufs=3)
                        for hh in range(2):
                            srcp = pcp[:, hh * HD:(hh + 1) * HD]
                            nc.vector.scalar_tensor_tensor(
                                qh[:, hh * HD:(hh + 1) * HD], srcp,
                                rinv2[:, hh:hh + 1], cq_sb[:, t, :],
                                op0=ALU.mult, op1=ALU.mult)
                            nc.vector.scalar_tensor_tensor(
                                t2[:, hh * HD:(hh + 1) * HD]
                                .rearrange("p (a b) -> p a b", a=2),
                                _swap_ap(srcp, HALF), rinv2[:, hh:hh + 1],
                                sq_sb[:, t, :].rearrange("p (a b) -> p a b", a=2),
                                op0=ALU.mult, op1=ALU.mult)
                        nc.gpsimd.tensor_add(qh[:], qh[:], t2[:])
                        ptr = psP.tile([128, 512], BF, tag="tr", bufs=2,
                                       name=f"qtr{nch}_{t}")
                        for mm in range(4):
                            nc.tensor.transpose(ptr[:, mm * 128:(mm + 1) * 128],
                                                qh[:, mm * 128:(mm + 1) * 128],
                                                ident[:])
                        nc.scalar.copy(
                            out=QT[:, nch * 4:(nch + 1) * 4, t * 128:(t + 1) * 128],
                            in_=ptr[:].rearrange("p (a b) -> p a b", a=4))

                # --- scores + exp + mask for all heads (PE overlaps V next) ---
                for h in range(NH):
                    md = h - (h % 2)      # KT m-tile base for kv head h//2
                    for kt in range(TL):
                        psc = psP.tile([128, TS], F32, tag="mm", bufs=6,
                                       name=f"psc{h}_{kt}")
                        for dh in range(2):
                            nc.tensor.matmul(
                                psc[:], KT[:, md + dh, kt * 128:(kt + 1) * 128],
                                QT[:, 2 * h + dh, :],
                                start=(dh == 0), stop=(dh == 1))
                        pr = sc_pool.tile([128, TS], BF, name=f"pr{h}_{kt}")
                        nc.scalar.activation(pr[:], psc[:], AF.Exp,
                                             scale=1.0 / 16.0)
                        if kt in (0, 1):
                            nc.vector.scalar_tensor_tensor(
                                pr[:], pr[:], pad_sb[:, kt:kt + 1],
                                masks[:, kt_mask[kt], :],
                                op0=ALU.mult, op1=ALU.mult)
                        elif kt in (2, 3):
                            nc.vector.tensor_scalar_mul(pr[:], pr[:],
                                                        pad_sb[:, kt:kt + 1])
                        else:
                            nc.vector.tensor_mul(pr[:], pr[:],
                                                 masks[:, kt_mask[kt], :])
                        PRS[h][kt] = pr

                # --- V projection (PE; exp/mask of scores overlaps) ---
                for nch in range(2):
                    wvt = s2.tile([128, KH, 512], BF, tag="wst",
                                  name=f"wv{nch}", bufs=2)
                    nc.sync.dma_start(wvt[:], wv_p.ap()[:, :, nch * 512:(nch + 1) * 512])
                    for t in range(TL):
                        ps = psP.tile([128, 512], F32, tag="mm", bufs=6,
                                      name=f"psv{nch}_{t}")
                        for k in range(KH):
                            nc.tensor.matmul(ps[:], hT[:, k, t * 128:(t + 1) * 128],
                                             wvt[:, k, :], start=(k == 0),
                                             stop=(k == KH - 1))
                        nc.scalar.copy(out=V[t][:, nch * 512:(nch + 1) * 512],
                                       in_=ps[:])

            mark('S3')
            # ===== S3: attention PV + output (scores already computed) =====
            pool_a.release()   # hT + wk + rope tables
            # MLP weight stream pool (persists through S5; allocated before
            # pool_b so the right-side pool stack pops in LIFO order)
            pool_s = tc.alloc_tile_pool(name="ps5", bufs=1, side="right")
            # wo is streamed in [KA, 512] n-chunks (first preloaded now);
            # MLP weight stream starts prefetching here too (DMA idle).
            pool_b = tc.alloc_tile_pool(name="pb", bufs=1, side="right")
            attnT = pool_b.tile([128, KA, TS], BF)
            wo_t = {}

            def load_wo(n):
                wo_t[n] = pool_b.tile([128, KA, 512], BF, tag="wo",
                                      name=f"wo{n}", bufs=2)
                nc.sync.dma_start(wo_t[n][:], wo_p.ap()[:, :, n * 512:(n + 1) * 512])

            for _n in range(2):
                load_wo(_n)
            # post-attn / pre-ff norm weights: broadcast now so S4's SP queue
            # only carries the remaining wo chunks
            w1_pa_b = pool_b.tile([128, HID], BF)
            w1_pf_b = pool_b.tile([128, HID], BF)
            _bcast_row(nc, w1_pa_b, w1_pa, HID)
            _bcast_row(nc, w1_pf_b, w1_pf, HID)
            wgu_t = {}
            wd_t = {}

            def load_m(m):
                wgu_t[m] = pool_s.tile([128, 2, KH, 128], BF, tag="wgu",
                                       name=f"wgu{m}", bufs=PREF + 1)
                nc.sync.dma_start(wgu_t[m][:], wgu_f.ap()[:, m])
                wd_t[m] = pool_s.tile([128, HID], BF, tag="wd",
                                      name=f"wd{m}", bufs=GM + PREF + 1)
                nc.gpsimd.dma_start(wd_t[m][:], wd_f.ap()[:, m])

            for m in range(PREF):
                load_m(m)
            with tc.tile_pool(name="s3", bufs=2) as s3:
                for qt in range(OT):
                  for h in range(NH):
                    prs = PRS[h]
                    if True:
                        po = psP.tile([128, HD + 1], F32, tag="mm", bufs=6,
                                      name=f"po{h}_{qt}")
                        for kt in range(TL):
                            sl = slice(qt * 128, (qt + 1) * 128)
                            nc.tensor.matmul(po[:, 0:HD], prs[kt][:, sl],
                                             V[kt][:, (h // 2) * HD:(h // 2 + 1) * HD],
                                             start=(kt == 0), stop=(kt == TL - 1))
                        for kt in range(TL):
                            sl = slice(qt * 128, (qt + 1) * 128)
                            nc.tensor.matmul(po[:, HD:HD + 1], prs[kt][:, sl],
                                             ones_t[:], start=(kt == 0),
                                             stop=(kt == TL - 1))
                        rec = s3.tile([128, 1], F32, tag="rec",
                                      name=f"rec{h}_{qt}")
                        nc.vector.reciprocal(rec[:], po[:, HD:HD + 1])
                        an = s3.tile([128, HD], BF, tag="an",
                                     name=f"an{h}_{qt}")
                        nc.vector.tensor_scalar_mul(an[:], po[:, 0:HD], rec[:])
                        ptr = psP.tile([128, HD], BF, tag="tr", bufs=2,
                                       name=f"atr{h}_{qt}")
                        for mm in range(2):
                            nc.tensor.transpose(ptr[:, mm * 128:(mm + 1) * 128],
                                                an[:, mm * 128:(mm + 1) * 128],
                                                ident[:])
                        nc.scalar.copy(
                            out=attnT[:, 2 * h:2 * h + 2, qt * 128:(qt + 1) * 128],
                            in_=ptr[:].rearrange("p (a b) -> p a b", a=2))

            mark('S4')
            # ======== S4: wo + post_attn + residual + pre_ff norm ========
            sc_pool.release()   # probs
            pool_a2.release()   # frees KT/QT/V/masks
            # S4-S5 persistent: h2T (pre-ff normed own tokens, transposed)
            pool_c = tc.alloc_tile_pool(name="pc", bufs=1)
            h2T = pool_c.tile([128, KH, TS], BF)
            with tc.tile_pool(name="s4", bufs=2) as s4:
                xos = [s4.tile([128, HID], F32, tag="xo", name=f"xo{t}",
                               bufs=2) for t in range(OT)]
                for t in range(OT):
                    nc.scalar.dma_start(xos[t][:], x_own.ap()[:, t, :])
                ao32s = [s4.tile([128, HID], F32, tag="ao32",
                                 name=f"ao32_{t}", bufs=2) for t in range(OT)]
                stats_a = [nrm.tile([128, 5, 6], F32, tag="nst",
                                    name=f"pan{t}_st") for t in range(OT)]
                # n-outer so each wo chunk is loaded once and used for both t
                for n in range(5):
                    if n + 2 < 5:
                        load_wo(n + 2)
                    wo_n = wo_t.pop(n)
                    for t in range(OT):
                        pw = psP.tile([128, 512], F32, tag="mm", bufs=6,
                                      name=f"pw{t}_{n}")
                        for m in range(KA):
                            nc.tensor.matmul(
                                pw[:], attnT[:, m, t * 128:(t + 1) * 128],
                                wo_n[:, m, :],
                                start=(m == 0), stop=(m == KA - 1))
                        nc.scalar.copy(out=ao32s[t][:, n * 512:(n + 1) * 512],
                                       in_=pw[:])
                        nc.vector.bn_stats(
                            out=stats_a[t][:, n, :],
                            in_=ao32s[t][:, n * 512:(n + 1) * 512])
                for t in range(OT):
                    # t=0 chain runs on DVE, t=1 on Pool so they interleave
                    e0, e1 = (nc.vector, nc.gpsimd) if t == 0 else \
                             (nc.gpsimd, nc.vector)
                    ao32 = ao32s[t]
                    rinv_a = rinv_from_stats(stats_a[t], f"pan{t}")
                    xo = xos[t]
                    x2 = s4.tile([128, HID], F32, tag="x2", name=f"x2_{t}",
                                 bufs=1)
                    stats_f = nrm.tile([128, 5, 6], F32, tag="nst",
                                       name=f"pff{t}_st")
                    for n in range(5):
                        sl = slice(n * 512, (n + 1) * 512)
                        e0.scalar_tensor_tensor(
                            x2[:, sl], ao32[:, sl], rinv_a[:], w1_pa_b[:, sl],
                            op0=ALU.mult, op1=ALU.mult)
                        e1.tensor_add(x2[:, sl], x2[:, sl], xo[:, sl])
                        nc.vector.bn_stats(out=stats_f[:, n, :], in_=x2[:, sl])
                    nc.sync.dma_start(x2_spill[t * 128:(t + 1) * 128, :], x2[:])
                    rinv_f = rinv_from_stats(stats_f, f"pff{t}")
                    h2 = s4.tile([128, HID], BF, tag="h2", name=f"h2_{t}",
                                 bufs=1)
                    for g in range(5):
                        sl = slice(g * 512, (g + 1) * 512)
                        e0.scalar_tensor_tensor(
                            h2[:, sl], x2[:, sl], rinv_f[:],
                            w1_pf_b[:, sl], op0=ALU.mult, op1=ALU.mult)
                        ptr = psP.tile([128, 512], BF, tag="tr", bufs=2,
                                       name=f"s4tr{t}_{g}")
                        for kk in range(4):
                            nc.tensor.transpose(
                                ptr[:, kk * 128:(kk + 1) * 128],
                                h2[:, (g * 4 + kk) * 128:(g * 4 + kk + 1) * 128],
                                ident[:])
                        nc.vector.tensor_copy(
                            h2T[:, g * 4:(g + 1) * 4, t * 128:(t + 1) * 128],
                            ptr[:].rearrange("p (a b) -> p a b", a=4))
            pool_b.release()   # attnT + wo

            mark('S5')
            # ===== S5: sequence-parallel MLP, weights streamed in m-chunks ==
            pool_acc = tc.alloc_tile_pool(name="pacc", bufs=1)
            accs = [pool_acc.tile([128, HID], F32, name=f"acc{t}")
                    for t in range(OT)]
            # S6 inputs: fetch during the MLP so the tail chain is short
            w1_po_b = pool_acc.tile([128, HID], F32)
            _bcast_row(nc, w1_po_b, w1_po, HID)
            x2ls = [pool_acc.tile([128, HID], F32, name=f"x2l{t}")
                    for t in range(OT)]
            o32s = [pool_acc.tile([128, HID], F32, name=f"o32_{t}")
                    for t in range(OT)]
            for t in range(OT):
                nc.scalar.dma_start(x2ls[t][:],
                                    x2_spill[t * 128:(t + 1) * 128, :])

            def s6_tile(t):
                # post_ff norm + residual + output for one token tile
                e0, e1 = (nc.vector, nc.gpsimd) if t == 0 else \
                         (nc.gpsimd, nc.vector)
                rinv_o = rmsnorm_rinv(accs[t][:], HID, f"pon{t}")
                o32 = o32s[t]
                for n in range(4):
                    sl = slice(n * 640, (n + 1) * 640)
                    e0.scalar_tensor_tensor(
                        o32[:, sl], accs[t][:, sl], rinv_o[:],
                        w1_po_b[:, sl], op0=ALU.mult, op1=ALU.mult)
                    e1.tensor_add(o32[:, sl], o32[:, sl], x2ls[t][:, sl])
                    nc.sync.dma_start(
                        out_shard.ap()[t * 128:(t + 1) * 128, sl],
                        o32[:, sl])

            with tc.tile_pool(name="s5", bufs=2) as s5:
                actT_t = {}

                def down_group(g0, m, tt):
                    first = (g0 == 0)
                    for ng in ((0, 1), (2, 3), (4,)):
                        pds = {n: psP.tile([128, 512], F32, tag="mm",
                                           bufs=6, name=f"pd{m}_{tt}_{n}")
                               for n in ng}
                        for j in range(GM):
                            mj = g0 + j
                            for n in ng:
                                nc.tensor.matmul(
                                    pds[n][:],
                                    actT_t[mj][:, tt * 128:(tt + 1) * 128],
                                    wd_t[mj][:, n * 512:(n + 1) * 512],
                                    start=(j == 0), stop=(j == GM - 1))
                        for n in ng:
                            dst = accs[tt][:, n * 512:(n + 1) * 512]
                            if first:
                                if n % 2 == 0:
                                    nc.vector.tensor_copy(dst, pds[n][:])
                                else:
                                    nc.scalar.copy(out=dst, in_=pds[n][:])
                            else:
                                if n % 2 == 0:
                                    nc.vector.tensor_add(dst, dst, pds[n][:])
                                else:
                                    nc.gpsimd.tensor_add(dst, dst, pds[n][:])

                for m in range(MIF):
                    if m + PREF < MIF:
                        load_m(m + PREF)
                    wgu = wgu_t.pop(m)
                    pg = psP.tile([128, TS], F32, tag="mm", bufs=6,
                                  name=f"pg{m}")
                    pu = psP.tile([128, TS], F32, tag="mm", bufs=6,
                                  name=f"pu{m}")
                    gsc = s5.tile([128, TS], F32, tag="gsc",
                                  name=f"gsc{m}", bufs=3)
                    at = s5.tile([128, TS], BF, tag="act",
                                 name=f"act{m}", bufs=GM + 2)
                    if m < 8:
                        # per-token-tile so tile-0 work overlaps S4's t=1 tail
                        for tt in range(OT):
                            sl = slice(tt * 128, (tt + 1) * 128)
                            for k in range(KH):
                                nc.tensor.matmul(pg[:, sl], wgu[:, 0, k, :],
                                                 h2T[:, k, sl],
                                                 start=(k == 0),
                                                 stop=(k == KH - 1))
                            for k in range(KH):
                                nc.tensor.matmul(pu[:, sl], wgu[:, 1, k, :],
                                                 h2T[:, k, sl],
                                                 start=(k == 0),
                                                 stop=(k == KH - 1))
                            nc.scalar.activation(gsc[:, sl], pg[:, sl],
                                                 AF.Gelu_apprx_tanh)
                            nc.vector.tensor_mul(at[:, sl], gsc[:, sl],
                                                 pu[:, sl])
                    else:
                        for k in range(KH):
                            nc.tensor.matmul(pg[:], wgu[:, 0, k, :],
                                             h2T[:, k, :],
                                             start=(k == 0), stop=(k == KH - 1))
                        for k in range(KH):
                            nc.tensor.matmul(pu[:], wgu[:, 1, k, :],
                                             h2T[:, k, :],
                                             start=(k == 0), stop=(k == KH - 1))
                        nc.scalar.activation(gsc[:], pg[:],
                                             AF.Gelu_apprx_tanh)
                        nc.vector.tensor_mul(at[:], gsc[:], pu[:])
                    actT_t[m] = at
                    if m % GM != GM - 1:
                        continue
                    # ---- down for this group of GM m-chunks ----
                    g0 = m - GM + 1
                    if m == MIF - 1:
                        # last group: finish each token tile's output inline
                        down_group(g0, m, 0)
                        s6_tile(0)
                        down_group(g0, m, 1)
                        s6_tile(1)
                    else:
                        for tt in range(OT):
                            down_group(g0, m, tt)
                    for j in range(GM):
                        del actT_t[g0 + j], wd_t[g0 + j]
            pool_s.release()

            mark('S6')
            pool_acc.release()
            pool_c.release()

    nc.compile()
    return nc


_NC_CACHE = None


def _get_nc():
    global _NC_CACHE
    if _NC_CACHE is None:
        _NC_CACHE = build_nc()
    return _NC_CACHE


def make_in_maps(hidden_states, position_ids, wq, wk, wv, wo, q_ln_w, k_ln_w,
                 in_ln_w, post_attn_ln_w, pre_ff_ln_w, post_ff_ln_w,
                 w_gate, w_up, w_down):
    bf16 = ml_dtypes.bfloat16
    f32 = np.float32
    x = np.asarray(hidden_states, f32).reshape(S, HID)
    pos = np.asarray(position_ids).reshape(S).astype(np.float64)

    inv_freq = 1.0 / (BASE ** (np.arange(0, HD, 2, dtype=np.float64) / HD))
    w1q = 1.0 + np.asarray(q_ln_w, f32)
    w1k = 1.0 + np.asarray(k_ln_w, f32)

    def rope_tabs(p, w1):
        emb = np.concatenate([p[:, None] * inv_freq[None, :]] * 2, axis=1)
        cos = np.cos(emb).astype(f32)
        sin = np.sin(emb).astype(f32)
        w1sw = np.concatenate([w1[HALF:], w1[:HALF]])
        sgn = np.concatenate([-np.ones(HALF, f32), np.ones(HALF, f32)])
        n = len(p)
        c = (cos * w1[None, :]).astype(bf16).reshape(n // 128, 128, HD)
        s_ = (sin * (w1sw * sgn)[None, :]).astype(bf16).reshape(n // 128, 128, HD)
        return (np.ascontiguousarray(c.transpose(1, 0, 2)),
                np.ascontiguousarray(s_.transpose(1, 0, 2)))

    def pack(w, kt, n):
        return np.ascontiguousarray(
            np.asarray(w, f32).reshape(kt, 128, n).transpose(1, 0, 2)).astype(bf16)

    # gate+up interleaved [128, MIF, 2, KH, 128]
    def pack_gu(w):
        # [HID, INTER] -> [KH, 128, MIF, 128] -> [128, MIF, KH, 128]
        return np.asarray(w, f32).reshape(KH, 128, MIF, 128).transpose(1, 2, 0, 3)

    wgu = np.stack([pack_gu(w_gate), pack_gu(w_up)], axis=2)  # [128,MIF,2,KH,128]
    wd_pk = np.ascontiguousarray(
        np.asarray(w_down, f32).reshape(MIF, 128, HID).transpose(1, 0, 2))

    common = {
        "wq_p": pack(wq, KH, NH * HD),
        "wk_p": pack(wk, KH, NKV * HD),
        "wv_p": pack(wv, KH, NKV * HD),
        "wo_p": pack(wo, KA, HID),
        "wgu_f": np.ascontiguousarray(wgu).astype(bf16),
        "wd_f": wd_pk.astype(bf16),
        "w1_in": (1.0 + np.asarray(in_ln_w, f32)).astype(bf16),
        "w1_pa": (1.0 + np.asarray(post_attn_ln_w, f32)).astype(bf16),
        "w1_pf": (1.0 + np.asarray(pre_ff_ln_w, f32)).astype(bf16),
        "w1_po": 1.0 + np.asarray(post_ff_ln_w, f32),
    }
    in_maps = []
    for c in range(NC_):
        lo = c * TS - HALO
        xh = np.zeros((LT, HID), f32)
        src_lo = max(0, lo)
        xh[src_lo - lo:] = x[src_lo:lo + LT]
        x_pk = np.ascontiguousarray(
            xh.reshape(TL, 128, HID).transpose(1, 0, 2)).astype(bf16)
        x_ow = np.ascontiguousarray(
            x[c * TS:(c + 1) * TS].reshape(OT, 128, HID).transpose(1, 0, 2))
        padv = (np.arange(lo, lo + LT) >= 0).astype(f32)
        pad_pk = np.ascontiguousarray(padv.reshape(TL, 128).T)
        kpos = np.where(np.arange(lo, lo + LT) >= 0,
                        pos[np.clip(np.arange(lo, lo + LT), 0, S - 1)], 0.0)
        qpos = pos[c * TS:(c + 1) * TS]
        ckw, skw = rope_tabs(kpos, w1k)
        cqw, sqw = rope_tabs(qpos, w1q)
        in_maps.append({
            "x_p": x_pk,
            "x_own": x_ow,
            "pad_p": pad_pk,
            "cq": cqw, "sq": sqw, "ck": ckw, "sk": skw,
            **common,
        })
    return in_maps


def kernel(**inputs):
    in_maps = make_in_maps(**inputs)
    nc = _get_nc()
    res = run_bass_kernel_spmd(nc, in_maps, core_ids=list(range(NC_)))
    out = np.concatenate([res.results[c]["out_shard"] for c in range(NC_)], axis=0)
    return out.reshape(1, S, HID).astype(np.float32)


# revision 29
# speedup vs baseline: 1.1075x; 1.0143x over previous
"""Gemma3 decoder layer on 8 Trainium2 NeuronCores (Bass/Tile), v3.

Sharding (per core c): fully sequence-parallel, ZERO collectives.
  - attention: core c owns tokens [256c, 256c+256) and receives a 512-token
    halo (host-side sharding): x_halo = x[256c-512 : 256c+256] (zero-padded
    for c<2). All attn weights (wq/wk/wv/wo) replicated; K/V computed for all
    768 local tokens, Q only for the 256 own tokens. Sliding-window (512)
    attention is then fully local. Pad keys masked via per-core pad mask.
  - MLP: sequence-parallel too. Each core runs the FULL 10240-dim MLP for its
    own 256 tokens, streaming gate/up/down weights (157MB bf16) from DRAM in
    128-inter-dim chunks, double-buffered under the matmul stream. Down
    partials accumulate in PSUM per 8-chunk group, then into an f32 SBUF
    accumulator (DVE/Pool adds). No AllGather, no ReduceScatter, no DRAM
    spill of h2/x2.
  - norms/residual: token-local.
Matmuls in bf16 (fp32 PSUM accumulation); norms/softmax/residual fp32.
All weights host-prepacked into SBUF layout (contiguous 5-10KB/partition DMAs).
"""
import sys

if "/opt/trn_rl_repo" not in sys.path:
    sys.path.insert(0, "/opt/trn_rl_repo")

import numpy as np
import ml_dtypes

import concourse.bass as bass
import concourse.mybir as mybir
import concourse.tile as tile
from concourse import bacc
from concourse.bass_utils import run_bass_kernel_spmd
from concourse.masks import make_identity

dt = mybir.dt
AF = mybir.ActivationFunctionType
ALU = mybir.AluOpType
BF = dt.bfloat16
F32 = dt.float32

HID, NH, NKV, HD, INTER = 2560, 8, 4, 256, 10240
WIN, EPS, BASE = 512, 1e-6, 10000.0
S = 2048
NC_ = 8
TS = S // NC_              # 256 own tokens per core
HALO = 512
LT = TS + HALO             # 768 local tokens (halo + own)
TL = LT // 128             # 6 local token tiles
OT = TS // 128             # 2 own token tiles
KH = HID // 128            # 20 hidden-dim k-chunks
KA = (NH * HD) // 128      # 16 attn-dim chunks
MIF = INTER // 128         # 80 inter m-chunks (full, per core)
GM = 4                     # m-chunks per down-accumulation group
NGRP = MIF // GM           # 20 groups
PREF = 3                   # stream prefetch depth (m-chunks)
HALF = HD // 2


def _bcast_row(nc, sbuf_tile, dram_t, width):
    a = dram_t.ap()
    nc.sync.dma_start(sbuf_tile[:], bass.AP(
        tensor=a.tensor, offset=a.offset, ap=[[0, 128], [1, width]]))


def _swap_ap(src_ap, half):
    """View [128, 2*half] with halves swapped, as [128, 2, half]."""
    return bass.AP(tensor=src_ap.tensor, offset=src_ap.offset + half,
                   ap=[list(src_ap.ap[0]), [-half, 2], [1, half]])


def build_nc(sim=False):
    nc = bacc.Bacc("TRN2", target_bir_lowering=False, debug=False,
                   enable_asserts=True, num_devices=1 if sim else NC_)

    # ---- inputs (host-prepacked layouts) ----
    x_p = nc.dram_tensor("x_p", [128, TL, HID], BF, kind="ExternalInput")
    x_own = nc.dram_tensor("x_own", [128, OT, HID], F32, kind="ExternalInput")
    pad_p = nc.dram_tensor("pad_p", [128, TL], F32, kind="ExternalInput")
    cq = nc.dram_tensor("cq", [128, OT, HD], BF, kind="ExternalInput")
    sq = nc.dram_tensor("sq", [128, OT, HD], BF, kind="ExternalInput")
    ck = nc.dram_tensor("ck", [128, TL, HD], BF, kind="ExternalInput")
    sk = nc.dram_tensor("sk", [128, TL, HD], BF, kind="ExternalInput")
    wq_p = nc.dram_tensor("wq_p", [128, KH, NH * HD], BF, kind="ExternalInput")
    wk_p = nc.dram_tensor("wk_p", [128, KH, NKV * HD], BF, kind="ExternalInput")
    wv_p = nc.dram_tensor("wv_p", [128, KH, NKV * HD], BF, kind="ExternalInput")
    wo_p = nc.dram_tensor("wo_p", [128, KA, HID], BF, kind="ExternalInput")
    # gate+up interleaved: [kpart, m-chunk, {g,u}, k-chunk, m-col]
    wgu_f = nc.dram_tensor("wgu_f", [128, MIF, 2, KH, 128], BF,
                           kind="ExternalInput")
    # down: [inter-part-within-chunk, m-chunk, hid]
    wd_f = nc.dram_tensor("wd_f", [128, MIF, HID], BF, kind="ExternalInput")
    w1_in = nc.dram_tensor("w1_in", [HID], BF, kind="ExternalInput")
    w1_pa = nc.dram_tensor("w1_pa", [HID], BF, kind="ExternalInput")
    w1_pf = nc.dram_tensor("w1_pf", [HID], BF, kind="ExternalInput")
    w1_po = nc.dram_tensor("w1_po", [HID], F32, kind="ExternalInput")
    out_shard = nc.dram_tensor("out_shard", [TS, HID], F32, kind="ExternalOutput")

    stages = {}
    nc._stage_ids = stages

    def mark(name):
        stages[name] = nc.next_id()

    with tile.TileContext(nc) as tc:
        with (
            tc.tile_pool(name="dram", bufs=1, space="DRAM") as dram,
            tc.tile_pool(name="glob", bufs=1) as glob,
            tc.tile_pool(name="nrm", bufs=3) as nrm,
            tc.tile_pool(name="psP", bufs=1, space="PSUM") as psP,
        ):
            x2_spill = dram.tile([TS, HID], F32)
            ident = glob.tile([128, 128], BF)
            make_identity(nc, ident[:])
            eps_t = glob.tile([128, 1], F32)
            nc.vector.memset(eps_t[:], EPS)
            ones_t = glob.tile([128, 1], BF)
            nc.vector.memset(ones_t[:], 1.0)


            # attention-phase residents: a2 lives through S3, a1 dies at S2 end
            pool_a2 = tc.alloc_tile_pool(name="pa2", bufs=1)
            KT = pool_a2.tile([128, NKV * 2, LT], BF)  # K^T [d, tok]
            QT = pool_a2.tile([128, KA, TS], BF)       # Q^T [d, tok]
            V = [pool_a2.tile([128, NKV * HD], BF, name=f"V{t}")
                 for t in range(TL)]
            sc_pool = tc.alloc_tile_pool(name="sc", bufs=1)
            pool_a = tc.alloc_tile_pool(name="pa1", bufs=1)
            # first two token tiles + in_ln weight: issue these DMAs before
            # the wk/rope loads so S1's first norm chains start ASAP
            xt01 = [pool_a.tile([128, HID], BF, name=f"xt0{t}")
                    for t in range(2)]
            for t in range(2):
                nc.scalar.dma_start(xt01[t][:], x_p.ap()[:, t, :])
            w1_in_b = pool_a.tile([128, HID], BF)
            _bcast_row(nc, w1_in_b, w1_in, HID)
            # warm the activation-function tables while those DMAs run
            warm = glob.tile([128, 1], F32)
            for af in (AF.Sqrt, AF.Exp, AF.Gelu_apprx_tanh, AF.Copy):
                nc.scalar.activation(warm[:], eps_t[:], af)
            hT = pool_a.tile([128, KH, LT], BF)       # h^T
            wk_sb = pool_a.tile([128, KH, NKV * HD], BF)
            for _i in range(4):
                nc.sync.dma_start(wk_sb[:, :, 256 * _i:256 * (_i + 1)],
                                  wk_p.ap()[:, :, 256 * _i:256 * (_i + 1)])
            cq_sb = pool_a.tile([128, OT, HD], BF)
            sq_sb = pool_a.tile([128, OT, HD], BF)
            ck_sb = pool_a.tile([128, TL, HD], BF)
            sk_sb = pool_a.tile([128, TL, HD], BF)
            nc.sync.dma_start(cq_sb[:], cq.ap())
            nc.sync.dma_start(sq_sb[:], sq.ap())
            nc.sync.dma_start(ck_sb[:], ck.ap())
            nc.sync.dma_start(sk_sb[:], sk.ap())

            # window/causal masks [128 ko, 256 qo] for k-tiles 0,1,4,5
            # valid iff 0 <= (512+qo) - (128*kt+ko) < 512
            masks = pool_a2.tile([128, 4, TS], BF)
            for i, kt in enumerate((0, 1, 4, 5)):
                mk = masks[:, i, :]
                nc.gpsimd.memset(mk, 1.0)
                if kt in (0, 1):
                    # keep where ko + (128*kt - 1) - qo >= 0
                    nc.gpsimd.affine_select(
                        out=mk, in_=mk, compare_op=ALU.is_ge, fill=0.0,
                        base=128 * kt - 1, pattern=[[-1, TS]],
                        channel_multiplier=1)
                else:
                    # keep where qo - ko + (512 - 128*kt) >= 0
                    nc.gpsimd.affine_select(
                        out=mk, in_=mk, compare_op=ALU.is_ge, fill=0.0,
                        base=512 - 128 * kt, pattern=[[1, TS]],
                        channel_multiplier=-1)
            pad_sb = pool_a2.tile([128, TL], F32)
            nc.sync.dma_start(pad_sb[:], pad_p.ap())

            def rinv_from_stats(stats, name):
                mv = nrm.tile([128, 2], F32, tag="nmv", name=f"{name}_mv")
                nc.vector.bn_aggr(out=mv[:], in_=stats[:])
                ms = nrm.tile([128, 1], F32, tag="nms", name=f"{name}_ms")
                nc.vector.scalar_tensor_tensor(ms[:], mv[:, 0:1], mv[:, 0:1],
                                               mv[:, 1:2], op0=ALU.mult,
                                               op1=ALU.add)
                nc.vector.tensor_scalar_add(ms[:], ms[:], EPS)
                rec = nrm.tile([128, 1], F32, tag="nrc", name=f"{name}_rc")
                nc.vector.reciprocal(rec[:], ms[:])
                rinv = nrm.tile([128, 1], F32, tag="nrv", name=f"{name}_rv")
                nc.scalar.activation(rinv[:], rec[:], AF.Sqrt)
                return rinv

            def rmsnorm_rinv(src_ap, d, name, rows=128):
                """rinv[p,1] = 1/sqrt(mean(src^2)+EPS) via bn_stats."""
                nsub = max(1, d // 512)
                stats = nrm.tile([128, nsub, 6], F32, tag="nst", name=f"{name}_st")
                if nsub > 1:
                    view = src_ap.rearrange("p (s f) -> p s f", s=nsub)
                    for i in range(nsub):
                        nc.vector.bn_stats(out=stats[:rows, i, :],
                                           in_=view[:, i, :])
                else:
                    nc.vector.bn_stats(out=stats[:rows, 0, :], in_=src_ap)
                mv = nrm.tile([128, 2], F32, tag="nmv", name=f"{name}_mv")
                nc.vector.bn_aggr(out=mv[:rows], in_=stats[:rows])
                ms = nrm.tile([128, 1], F32, tag="nms", name=f"{name}_ms")
                nc.vector.scalar_tensor_tensor(ms[:rows], mv[:rows, 0:1],
                                               mv[:rows, 0:1], mv[:rows, 1:2],
                                               op0=ALU.mult, op1=ALU.add)
                nc.vector.tensor_scalar_add(ms[:rows], ms[:rows], EPS)
                rec = nrm.tile([128, 1], F32, tag="nrc", name=f"{name}_rc")
                nc.vector.reciprocal(rec[:rows], ms[:rows])
                rinv = nrm.tile([128, 1], F32, tag="nrv", name=f"{name}_rv")
                nc.scalar.activation(rinv[:rows], rec[:rows], AF.Sqrt)
                return rinv

            def rmsnorm_rinv2(src_ap, name):
                """Two per-head rinvs for a [128, 512] psum (2 heads of 256).
                Returns [128, 2] f32 tile."""
                stats = nrm.tile([128, 2, 6], F32, tag="hst", name=f"{name}_st")
                view = src_ap.rearrange("p (s f) -> p s f", s=2)
                for i in range(2):
                    nc.vector.bn_stats(out=stats[:, i, :], in_=view[:, i, :])
                mv = nrm.tile([128, 2, 2], F32, tag="hmv", name=f"{name}_mv")
                for i in range(2):
                    nc.vector.bn_aggr(out=mv[:, i, :], in_=stats[:, i, :])
                ms = nrm.tile([128, 2], F32, tag="hms", name=f"{name}_ms")
                nc.vector.tensor_mul(ms[:], mv[:, :, 0], mv[:, :, 0])
                nc.vector.tensor_add(ms[:], ms[:], mv[:, :, 1])
                nc.vector.tensor_scalar_add(ms[:], ms[:], EPS)
                rec = nrm.tile([128, 2], F32, tag="hrc", name=f"{name}_rc")
                nc.vector.reciprocal(rec[:], ms[:])
                rinv = nrm.tile([128, 2], F32, tag="hrv", name=f"{name}_rv")
                nc.scalar.activation(rinv[:], rec[:], AF.Sqrt)
                return rinv

            def k_group(pool, nch, t):
                ps = psP.tile([128, 512], F32, tag="mm", bufs=6,
                              name=f"psk{nch}_{t}")
                for k in range(KH):
                    nc.tensor.matmul(ps[:], hT[:, k, t * 128:(t + 1) * 128],
                                     wk_sb[:, k, nch * 512:(nch + 1) * 512],
                                     start=(k == 0), stop=(k == KH - 1))
                pcp = pool.tile([128, 512], BF, tag="pcp",
                                name=f"kcp{nch}_{t}", bufs=4)
                nc.scalar.copy(out=pcp[:], in_=ps[:])
                rinv2 = rmsnorm_rinv2(pcp[:], f"kn{nch}_{t}")
                kh = pool.tile([128, 512], BF, tag="kh",
                               name=f"kh{nch}_{t}", bufs=3)
                t2 = pool.tile([128, 512], BF, tag="t2",
                               name=f"t2k{nch}_{t}", bufs=3)
                for hh in range(2):
                    srcp = pcp[:, hh * HD:(hh + 1) * HD]
                    nc.vector.scalar_tensor_tensor(
                        kh[:, hh * HD:(hh + 1) * HD], srcp,
                        rinv2[:, hh:hh + 1], ck_sb[:, t, :],
                        op0=ALU.mult, op1=ALU.mult)
                    nc.vector.scalar_tensor_tensor(
                        t2[:, hh * HD:(hh + 1) * HD]
                        .rearrange("p (a b) -> p a b", a=2),
                        _swap_ap(srcp, HALF), rinv2[:, hh:hh + 1],
                        sk_sb[:, t, :].rearrange("p (a b) -> p a b", a=2),
                        op0=ALU.mult, op1=ALU.mult)
                nc.gpsimd.tensor_add(kh[:], kh[:], t2[:])
                ptr = psP.tile([128, 512], BF, tag="tr", bufs=2,
                               name=f"ktr{nch}_{t}")
                for mm in range(4):
                    nc.tensor.transpose(ptr[:, mm * 128:(mm + 1) * 128],
                                        kh[:, mm * 128:(mm + 1) * 128],
                                        ident[:])
                nc.scalar.copy(
                    out=KT[:, nch * 4:(nch + 1) * 4, t * 128:(t + 1) * 128],
                    in_=ptr[:].rearrange("p (a b) -> p a b", a=4))

            mark('S1')
            # ============ S1: in_ln over 768 local tokens + transpose ======
            with tc.tile_pool(name="s1", bufs=2) as s1:
                for t in range(TL):
                    if t < 2:
                        xt = xt01[t]
                    else:
                        xt = s1.tile([128, HID], BF, tag="xt", name=f"xt{t}",
                                     bufs=2)
                        nc.scalar.dma_start(xt[:], x_p.ap()[:, t, :])
                    rinv = rmsnorm_rinv(xt[:], HID, f"inln{t}")
                    ht = s1.tile([128, HID], BF, tag="ht", name=f"ht{t}", bufs=2)
                    for cch in range(5):
                        sl = slice(cch * 512, (cch + 1) * 512)
                        nc.vector.scalar_tensor_tensor(
                            ht[:, sl], xt[:, sl], rinv[:], w1_in_b[:, sl],
                            op0=ALU.mult, op1=ALU.mult)
                    for g in range(5):   # 4 k-chunks per transpose group
                        ptr = psP.tile([128, 512], BF, tag="tr", bufs=2,
                                       name=f"s1tr{t}_{g}")
                        for kk in range(4):
                            nc.tensor.transpose(
                                ptr[:, kk * 128:(kk + 1) * 128],
                                ht[:, (g * 4 + kk) * 128:(g * 4 + kk + 1) * 128],
                                ident[:])
                        dst = hT[:, g * 4:(g + 1) * 4, t * 128:(t + 1) * 128]
                        src_ = ptr[:].rearrange("p (a b) -> p a b", a=4)
                        nc.scalar.copy(out=dst, in_=src_)

            mark('S2')
            # ====== S2: Q projection, then scores+exp, then V (overlap) =====
            kt_mask = {0: 0, 1: 1, 4: 2, 5: 3}
            PRS = [[None] * TL for _ in range(NH)]
            with tc.tile_pool(name="s2", bufs=2) as s2:
                # --- K: per (nch, t): 2 heads norm+rope, transpose ---
                for nch in range(2):
                    for t in range(TL):
                        k_group(s2, nch, t)

                # --- Q: own tokens only (local tiles 4,5) ---
                for nch in range(4):
                    wqt = s2.tile([128, KH, 512], BF, tag="wst",
                                  name=f"wq{nch}", bufs=2)
                    nc.sync.dma_start(wqt[:], wq_p.ap()[:, :, nch * 512:(nch + 1) * 512])
                    for t in range(OT):
                        lt = 4 + t
                        ps = psP.tile([128, 512], F32, tag="mm", bufs=6,
                                      name=f"psq{nch}_{t}")
                        for k in range(KH):
                            nc.tensor.matmul(ps[:], hT[:, k, lt * 128:(lt + 1) * 128],
                                             wqt[:, k, :], start=(k == 0),
                                             stop=(k == KH - 1))
                        pcp = s2.tile([128, 512], BF, tag="pcp",
                                      name=f"qcp{nch}_{t}", bufs=4)
                        nc.scalar.copy(out=pcp[:], in_=ps[:])
                        rinv2 = rmsnorm_rinv2(pcp[:], f"qn{nch}_{t}")
                        qh = s2.tile([128, 512], BF, tag="qh",
                                     name=f"qh{nch}_{t}", bufs=3)
                        t2 = s2.tile([128, 512], BF, tag="t2",
                                     name=f"t2q{nch}_{t}", bufs=3)
                        for hh in range(2):
                            srcp = pcp[:, hh * HD:(hh + 1) * HD]
                            nc.vector.scalar_tensor_tensor(
                                qh[:, hh * HD:(hh + 1) * HD], srcp,
                                rinv2[:, hh:hh + 1], cq_sb[:, t, :],
                                op0=ALU.mult, op1=ALU.mult)
                            nc.vector.scalar_tensor_tensor(
                                t2[:, hh * HD:(hh + 1) * HD]
                                .rearrange("p (a b) -> p a b", a=2),
                                _swap_ap(srcp, HALF), rinv2[:, hh:hh + 1],
                                sq_sb[:, t, :].rearrange("p (a b) -> p a b", a=2),
                                op0=ALU.mult, op1=ALU.mult)
                        nc.gpsimd.tensor_add(qh[:], qh[:], t2[:])
                        ptr = psP.tile([128, 512], BF, tag="tr", bufs=2,
                                       name=f"qtr{nch}_{t}")
                        for mm in range(4):
                            nc.tensor.transpose(ptr[:, mm * 128:(mm + 1) * 128],
                                                qh[:, mm * 128:(mm + 1) * 128],
                                                ident[:])
                        nc.scalar.copy(
                            out=QT[:, nch * 4:(nch + 1) * 4, t * 128:(t + 1) * 128],
                            in_=ptr[:].rearrange("p (a b) -> p a b", a=4))

                # --- scores + exp + mask for all heads (PE overlaps V next) ---
                for h in range(NH):
                    md = h - (h % 2)      # KT m-tile base for kv head h//2
                    for kt in range(TL):
                        psc = psP.tile([128, TS], F32, tag="mm", bufs=6,
                                       name=f"psc{h}_{kt}")
                        for dh in range(2):
                            nc.tensor.matmul(
                                psc[:], KT[:, md + dh, kt * 128:(kt + 1) * 128],
                                QT[:, 2 * h + dh, :],
                                start=(dh == 0), stop=(dh == 1))
                        pr = sc_pool.tile([128, TS], BF, name=f"pr{h}_{kt}")
                        nc.scalar.activation(pr[:], psc[:], AF.Exp,
                                             scale=1.0 / 16.0)
                        if kt in (0, 1):
                            nc.vector.scalar_tensor_tensor(
                                pr[:], pr[:], pad_sb[:, kt:kt + 1],
                                masks[:, kt_mask[kt], :],
                                op0=ALU.mult, op1=ALU.mult)
                        elif kt in (2, 3):
                            nc.vector.tensor_scalar_mul(pr[:], pr[:],
                                                        pad_sb[:, kt:kt + 1])
                        else:
                            nc.vector.tensor_mul(pr[:], pr[:],
                                                 masks[:, kt_mask[kt], :])
                        PRS[h][kt] = pr

                # --- V projection (PE; exp/mask of scores overlaps) ---
                for nch in range(2):
                    wvt = s2.tile([128, KH, 512], BF, tag="wst",
                                  name=f"wv{nch}", bufs=2)
                    nc.sync.dma_start(wvt[:], wv_p.ap()[:, :, nch * 512:(nch + 1) * 512])
                    for t in range(TL):
                        ps = psP.tile([128, 512], F32, tag="mm", bufs=6,
                                      name=f"psv{nch}_{t}")
                        for k in range(KH):
                            nc.tensor.matmul(ps[:], hT[:, k, t * 128:(t + 1) * 128],
                                             wvt[:, k, :], start=(k == 0),
                                             stop=(k == KH - 1))
                        nc.scalar.copy(out=V[t][:, nch * 512:(nch + 1) * 512],
                                       in_=ps[:])

            mark('S3')
            # ===== S3: attention PV + output (scores already computed) =====
            pool_a.release()   # hT + wk + rope tables
            # MLP weight stream pool (persists through S5; allocated before
            # pool_b so the right-side pool stack pops in LIFO order)
            pool_s = tc.alloc_tile_pool(name="ps5", bufs=1, side="right")
            # wo is streamed in [KA, 512] n-chunks (first preloaded now);
            # MLP weight stream starts prefetching here too (DMA idle).
            pool_b = tc.alloc_tile_pool(name="pb", bufs=1, side="right")
            attnT = pool_b.tile([128, KA, TS], BF)
            wo_t = {}

            def load_wo(n):
                wo_t[n] = pool_b.tile([128, KA, 512], BF, tag="wo",
                                      name=f"wo{n}", bufs=3)
                nc.sync.dma_start(wo_t[n][:], wo_p.ap()[:, :, n * 512:(n + 1) * 512])

            for _n in range(3):
                load_wo(_n)
            # post-attn / pre-ff norm weights: broadcast now so S4's SP queue
            # only carries the remaining wo chunks
            w1_pa_b = pool_b.tile([128, HID], BF)
            w1_pf_b = pool_b.tile([128, HID], BF)
            _bcast_row(nc, w1_pa_b, w1_pa, HID)
            _bcast_row(nc, w1_pf_b, w1_pf, HID)
            wgu_t = {}
            wd_t = {}

            def load_m(m):
                wgu_t[m] = pool_s.tile([128, 2, KH, 128], BF, tag="wgu",
                                       name=f"wgu{m}", bufs=PREF + 1)
                nc.sync.dma_start(wgu_t[m][:], wgu_f.ap()[:, m])
                wd_t[m] = pool_s.tile([128, HID], BF, tag="wd",
                                      name=f"wd{m}", bufs=GM + PREF)
                nc.sync.dma_start(wd_t[m][:], wd_f.ap()[:, m])

            for m in range(PREF):
                load_m(m)
            with tc.tile_pool(name="s3", bufs=2) as s3:
                for qt in range(OT):
                  for h in range(NH):
                    prs = PRS[h]
                    if True:
                        po = psP.tile([128, HD + 1], F32, tag="mm", bufs=6,
                                      name=f"po{h}_{qt}")
                        for kt in range(TL):
                            sl = slice(qt * 128, (qt + 1) * 128)
                            nc.tensor.matmul(po[:, 0:HD], prs[kt][:, sl],
                                             V[kt][:, (h // 2) * HD:(h // 2 + 1) * HD],
                                             start=(kt == 0), stop=(kt == TL - 1))
                        for kt in range(TL):
                            sl = slice(qt * 128, (qt + 1) * 128)
                            nc.tensor.matmul(po[:, HD:HD + 1], prs[kt][:, sl],
                                             ones_t[:], start=(kt == 0),
                                             stop=(kt == TL - 1))
                        rec = s3.tile([128, 1], F32, tag="rec",
                                      name=f"rec{h}_{qt}")
                        nc.vector.reciprocal(rec[:], po[:, HD:HD + 1])
                        an = s3.tile([128, HD], BF, tag="an",
                                     name=f"an{h}_{qt}")
                        nc.vector.tensor_scalar_mul(an[:], po[:, 0:HD], rec[:])
                        ptr = psP.tile([128, HD], BF, tag="tr", bufs=2,
                                       name=f"atr{h}_{qt}")
                        for mm in range(2):
                            nc.tensor.transpose(ptr[:, mm * 128:(mm + 1) * 128],
                                                an[:, mm * 128:(mm + 1) * 128],
                                                ident[:])
                        nc.scalar.copy(
                            out=attnT[:, 2 * h:2 * h + 2, qt * 128:(qt + 1) * 128],
                            in_=ptr[:].rearrange("p (a b) -> p a b", a=2))

            mark('S4')
            # ======== S4: wo + post_attn + residual + pre_ff norm ========
            sc_pool.release()   # probs
            pool_a2.release()   # frees KT/QT/V/masks
            # S4-S5 persistent: h2T (pre-ff normed own tokens, transposed)
            pool_c = tc.alloc_tile_pool(name="pc", bufs=1)
            h2T = pool_c.tile([128, KH, TS], BF)
            with tc.tile_pool(name="s4", bufs=2) as s4:
                xos = [s4.tile([128, HID], F32, tag="xo", name=f"xo{t}",
                               bufs=2) for t in range(OT)]
                for t in range(OT):
                    nc.scalar.dma_start(xos[t][:], x_own.ap()[:, t, :])
                ao32s = [s4.tile([128, HID], F32, tag="ao32",
                                 name=f"ao32_{t}", bufs=2) for t in range(OT)]
                stats_a = [nrm.tile([128, 5, 6], F32, tag="nst",
                                    name=f"pan{t}_st") for t in range(OT)]
                # n-outer so each wo chunk is loaded once and used for both t
                for n in range(5):
                    wo_n = wo_t.pop(n)
                    for t in range(OT):
                        pw = psP.tile([128, 512], F32, tag="mm", bufs=6,
                                      name=f"pw{t}_{n}")
                        for m in range(KA):
                            nc.tensor.matmul(
                                pw[:], attnT[:, m, t * 128:(t + 1) * 128],
                                wo_n[:, m, :],
                                start=(m == 0), stop=(m == KA - 1))
                        nc.scalar.copy(out=ao32s[t][:, n * 512:(n + 1) * 512],
                                       in_=pw[:])
                        nc.vector.bn_stats(
                            out=stats_a[t][:, n, :],
                            in_=ao32s[t][:, n * 512:(n + 1) * 512])
                    # prefetch AFTER this chunk's reads: with bufs=3, slot
                    # (n+3)%3 == n%3 is the one just consumed above
                    if n + 3 < 5:
                        load_wo(n + 3)
                for t in range(OT):
                    # t=0 chain runs on DVE, t=1 on Pool so they interleave
                    e0, e1 = (nc.vector, nc.gpsimd) if t == 0 else \
                             (nc.gpsimd, nc.vector)
                    ao32 = ao32s[t]
                    rinv_a = rinv_from_stats(stats_a[t], f"pan{t}")
                    xo = xos[t]
                    x2 = s4.tile([128, HID], F32, tag="x2", name=f"x2_{t}",
                                 bufs=1)
                    stats_f = nrm.tile([128, 5, 6], F32, tag="nst",
                                       name=f"pff{t}_st")
                    for n in range(5):
                        sl = slice(n * 512, (n + 1) * 512)
                        e0.scalar_tensor_tensor(
                            x2[:, sl], ao32[:, sl], rinv_a[:], w1_pa_b[:, sl],
                            op0=ALU.mult, op1=ALU.mult)
                        e1.tensor_add(x2[:, sl], x2[:, sl], xo[:, sl])
                        nc.vector.bn_stats(out=stats_f[:, n, :], in_=x2[:, sl])
                    nc.scalar.dma_start(x2_spill[t * 128:(t + 1) * 128, :], x2[:])
                    rinv_f = rinv_from_stats(stats_f, f"pff{t}")
                    h2 = s4.tile([128, HID], BF, tag="h2", name=f"h2_{t}",
                                 bufs=1)
                    for g in range(5):
                        sl = slice(g * 512, (g + 1) * 512)
                        e0.scalar_tensor_tensor(
                            h2[:, sl], x2[:, sl], rinv_f[:],
                            w1_pf_b[:, sl], op0=ALU.mult, op1=ALU.mult)
                        ptr = psP.tile([128, 512], BF, tag="tr", bufs=2,
                                       name=f"s4tr{t}_{g}")
                        for kk in range(4):
                            nc.tensor.transpose(
                                ptr[:, kk * 128:(kk + 1) * 128],
                                h2[:, (g * 4 + kk) * 128:(g * 4 + kk + 1) * 128],
                                ident[:])
                        nc.vector.tensor_copy(
                            h2T[:, g * 4:(g + 1) * 4, t * 128:(t + 1) * 128],
                            ptr[:].rearrange("p (a b) -> p a b", a=4))
            pool_b.release()   # attnT + wo

            mark('S5')
            # ===== S5: sequence-parallel MLP, weights streamed in m-chunks ==
            pool_acc = tc.alloc_tile_pool(name="pacc", bufs=1)
            accs = [pool_acc.tile([128, HID], F32, name=f"acc{t}")
                    for t in range(OT)]
            # S6 inputs: fetch during the MLP so the tail chain is short
            w1_po_b = pool_acc.tile([128, HID], F32)
            _bcast_row(nc, w1_po_b, w1_po, HID)
            x2ls = [pool_acc.tile([128, HID], F32, name=f"x2l{t}")
                    for t in range(OT)]
            o32s = [pool_acc.tile([128, HID], F32, name=f"o32_{t}")
                    for t in range(OT)]
            for t in range(OT):
                nc.scalar.dma_start(x2ls[t][:],
                                    x2_spill[t * 128:(t + 1) * 128, :])

            def s6_tile(t):
                # post_ff norm + residual + output for one token tile
                e0, e1 = (nc.vector, nc.gpsimd) if t == 0 else \
                         (nc.gpsimd, nc.vector)
                rinv_o = rmsnorm_rinv(accs[t][:], HID, f"pon{t}")
                o32 = o32s[t]
                for n in range(4):
                    sl = slice(n * 640, (n + 1) * 640)
                    e0.scalar_tensor_tensor(
                        o32[:, sl], accs[t][:, sl], rinv_o[:],
                        w1_po_b[:, sl], op0=ALU.mult, op1=ALU.mult)
                    e1.tensor_add(o32[:, sl], o32[:, sl], x2ls[t][:, sl])
                    nc.scalar.dma_start(
                        out_shard.ap()[t * 128:(t + 1) * 128, sl],
                        o32[:, sl])

            with tc.tile_pool(name="s5", bufs=2) as s5:
                actT_t = {}

                def down_group(g0, m, tt):
                    first = (g0 == 0)
                    for ng in ((0, 1), (2, 3), (4,)):
                        pds = {n: psP.tile([128, 512], F32, tag="mm",
                                           bufs=6, name=f"pd{m}_{tt}_{n}")
                               for n in ng}
                        for j in range(GM):
                            mj = g0 + j
                            for n in ng:
                                nc.tensor.matmul(
                                    pds[n][:],
                                    actT_t[mj][:, tt * 128:(tt + 1) * 128],
                                    wd_t[mj][:, n * 512:(n + 1) * 512],
                                    start=(j == 0), stop=(j == GM - 1))
                        for n in ng:
                            dst = accs[tt][:, n * 512:(n + 1) * 512]
                            if first:
                                if n % 2 == 0:
                                    nc.vector.tensor_copy(dst, pds[n][:])
                                else:
                                    nc.scalar.copy(out=dst, in_=pds[n][:])
                            else:
                                if n % 2 == 0:
                                    nc.vector.tensor_add(dst, dst, pds[n][:])
                                else:
                                    nc.gpsimd.tensor_add(dst, dst, pds[n][:])

                for m in range(MIF):
                    if m + PREF < MIF:
                        load_m(m + PREF)
                    wgu = wgu_t.pop(m)
                    pg = psP.tile([128, TS], F32, tag="mm", bufs=6,
                                  name=f"pg{m}")
                    pu = psP.tile([128, TS], F32, tag="mm", bufs=6,
                                  name=f"pu{m}")
                    gsc = s5.tile([128, TS], F32, tag="gsc",
                                  name=f"gsc{m}", bufs=3)
                    at = s5.tile([128, TS], BF, tag="act",
                                 name=f"act{m}", bufs=GM + 2)
                    if m < 8:
                        # per-token-tile so tile-0 work overlaps S4's t=1 tail
                        for tt in range(OT):
                            sl = slice(tt * 128, (tt + 1) * 128)
                            for k in range(KH):
                                nc.tensor.matmul(pg[:, sl], wgu[:, 0, k, :],
                                                 h2T[:, k, sl],
                                                 start=(k == 0),
                                                 stop=(k == KH - 1))
                            for k in range(KH):
                                nc.tensor.matmul(pu[:, sl], wgu[:, 1, k, :],
                                                 h2T[:, k, sl],
                                                 start=(k == 0),
                                                 stop=(k == KH - 1))
                            nc.scalar.activation(gsc[:, sl], pg[:, sl],
                                                 AF.Gelu_apprx_tanh)
                            nc.vector.tensor_mul(at[:, sl], gsc[:, sl],
                                                 pu[:, sl])
                    else:
                        for k in range(KH):
                            nc.tensor.matmul(pg[:], wgu[:, 0, k, :],
                                             h2T[:, k, :],
                                             start=(k == 0), stop=(k == KH - 1))
                        for k in range(KH):
                            nc.tensor.matmul(pu[:], wgu[:, 1, k, :],
                                             h2T[:, k, :],
                                             start=(k == 0), stop=(k == KH - 1))
                        nc.scalar.activation(gsc[:], pg[:],
                                             AF.Gelu_apprx_tanh)
                        nc.vector.tensor_mul(at[:], gsc[:], pu[:])
                    actT_t[m] = at
                    if m % GM != GM - 1:
                        continue
                    # ---- down for this group of GM m-chunks ----
                    g0 = m - GM + 1
                    if m == MIF - 1:
                        # last group: finish each token tile's output inline
                        down_group(g0, m, 0)
                        s6_tile(0)
                        down_group(g0, m, 1)
                        s6_tile(1)
                    else:
                        for tt in range(OT):
                            down_group(g0, m, tt)
                    for j in range(GM):
                        del actT_t[g0 + j], wd_t[g0 + j]
            pool_s.release()

            mark('S6')
            pool_acc.release()
            pool_c.release()

    nc.compile()
    return nc


_NC_CACHE = None


def _get_nc():
    global _NC_CACHE
    if _NC_CACHE is None:
        _NC_CACHE = build_nc()
    return _NC_CACHE


def make_in_maps(hidden_states, position_ids, wq, wk, wv, wo, q_ln_w, k_ln_w,
                 in_ln_w, post_attn_ln_w, pre_ff_ln_w, post_ff_ln_w,
                 w_gate, w_up, w_down):
    bf16 = ml_dtypes.bfloat16
    f32 = np.float32
    x = np.asarray(hidden_states, f32).reshape(S, HID)
    pos = np.asarray(position_ids).reshape(S).astype(np.float64)

    inv_freq = 1.0 / (BASE ** (np.arange(0, HD, 2, dtype=np.float64) / HD))
    w1q = 1.0 + np.asarray(q_ln_w, f32)
    w1k = 1.0 + np.asarray(k_ln_w, f32)

    def rope_tabs(p, w1):
        emb = np.concatenate([p[:, None] * inv_freq[None, :]] * 2, axis=1)
        cos = np.cos(emb).astype(f32)
        sin = np.sin(emb).astype(f32)
        w1sw = np.concatenate([w1[HALF:], w1[:HALF]])
        sgn = np.concatenate([-np.ones(HALF, f32), np.ones(HALF, f32)])
        n = len(p)
        c = (cos * w1[None, :]).astype(bf16).reshape(n // 128, 128, HD)
        s_ = (sin * (w1sw * sgn)[None, :]).astype(bf16).reshape(n // 128, 128, HD)
        return (np.ascontiguousarray(c.transpose(1, 0, 2)),
                np.ascontiguousarray(s_.transpose(1, 0, 2)))

    def pack(w, kt, n):
        return np.ascontiguousarray(
            np.asarray(w, f32).reshape(kt, 128, n).transpose(1, 0, 2)).astype(bf16)

    # gate+up interleaved [128, MIF, 2, KH, 128]
    def pack_gu(w):
        # [HID, INTER] -> [KH, 128, MIF, 128] -> [128, MIF, KH, 128]
        return np.asarray(w, f32).reshape(KH, 128, MIF, 128).transpose(1, 2, 0, 3)

    wgu = np.stack([pack_gu(w_gate), pack_gu(w_up)], axis=2)  # [128,MIF,2,KH,128]
    wd_pk = np.ascontiguousarray(
        np.asarray(w_down, f32).reshape(MIF, 128, HID).transpose(1, 0, 2))

    common = {
        "wq_p": pack(wq, KH, NH * HD),
        "wk_p": pack(wk, KH, NKV * HD),
        "wv_p": pack(wv, KH, NKV * HD),
        "wo_p": pack(wo, KA, HID),
        "wgu_f": np.ascontiguousarray(wgu).astype(bf16),
        "wd_f": wd_pk.astype(bf16),
        "w1_in": (1.0 + np.asarray(in_ln_w, f32)).astype(bf16),
        "w1_pa": (1.0 + np.asarray(post_attn_ln_w, f32)).astype(bf16),
        "w1_pf": (1.0 + np.asarray(pre_ff_ln_w, f32)).astype(bf16),
        "w1_po": 1.0 + np.asarray(post_ff_ln_w, f32),
    }
    in_maps = []
    for c in range(NC_):
        lo = c * TS - HALO
        xh = np.zeros((LT, HID), f32)
        src_lo = max(0, lo)
        xh[src_lo - lo:] = x[src_lo:lo + LT]
        x_pk = np.ascontiguousarray(
            xh.reshape(TL, 128, HID).transpose(1, 0, 2)).astype(bf16)
        x_ow = np.ascontiguousarray(
            x[c * TS:(c + 1) * TS].reshape(OT, 128, HID).transpose(1, 0, 2))
        padv = (np.arange(lo, lo + LT) >= 0).astype(f32)
        pad_pk = np.ascontiguousarray(padv.reshape(TL, 128).T)
        kpos = np.where(np.arange(lo, lo + LT) >= 0,
                        pos[np.clip(np.arange(lo, lo + LT), 0, S - 1)], 0.0)
        qpos = pos[c * TS:(c + 1) * TS]
        ckw, skw = rope_tabs(kpos, w1k)
        cqw, sqw = rope_tabs(qpos, w1q)
        in_maps.append({
            "x_p": x_pk,
            "x_own": x_ow,
            "pad_p": pad_pk,
            "cq": cqw, "sq": sqw, "ck": ckw, "sk": skw,
            **common,
        })
    return in_maps


def kernel(**inputs):
    in_maps = make_in_maps(**inputs)
    nc = _get_nc()
    res = run_bass_kernel_spmd(nc, in_maps, core_ids=list(range(NC_)))
    out = np.concatenate([res.results[c]["out_shard"] for c in range(NC_)], axis=0)
    return out.reshape(1, S, HID).astype(np.float32)


# revision 35
# speedup vs baseline: 1.1203x; 1.0115x over previous
"""Gemma3 decoder layer on 8 Trainium2 NeuronCores (Bass/Tile), v3.

Sharding (per core c): fully sequence-parallel, ZERO collectives.
  - attention: core c owns tokens [256c, 256c+256) and receives a 512-token
    halo (host-side sharding): x_halo = x[256c-512 : 256c+256] (zero-padded
    for c<2). All attn weights (wq/wk/wv/wo) replicated; K/V computed for all
    768 local tokens, Q only for the 256 own tokens. Sliding-window (512)
    attention is then fully local. Pad keys masked via per-core pad mask.
  - MLP: sequence-parallel too. Each core runs the FULL 10240-dim MLP for its
    own 256 tokens, streaming gate/up/down weights (157MB bf16) from DRAM in
    128-inter-dim chunks, double-buffered under the matmul stream. Down
    partials accumulate in PSUM per 8-chunk group, then into an f32 SBUF
    accumulator (DVE/Pool adds). No AllGather, no ReduceScatter, no DRAM
    spill of h2/x2.
  - norms/residual: token-local.
Matmuls in bf16 (fp32 PSUM accumulation); norms/softmax/residual fp32.
All weights host-prepacked into SBUF layout (contiguous 5-10KB/partition DMAs).
"""
import sys

if "/opt/trn_rl_repo" not in sys.path:
    sys.path.insert(0, "/opt/trn_rl_repo")

import numpy as np
import ml_dtypes

import concourse.bass as bass
import concourse.mybir as mybir
import concourse.tile as tile
from concourse import bacc
from concourse.bass_utils import run_bass_kernel_spmd
from concourse.masks import make_identity

dt = mybir.dt
AF = mybir.ActivationFunctionType
ALU = mybir.AluOpType
BF = dt.bfloat16
F32 = dt.float32

HID, NH, NKV, HD, INTER = 2560, 8, 4, 256, 10240
WIN, EPS, BASE = 512, 1e-6, 10000.0
S = 2048
NC_ = 8
TS = S // NC_              # 256 own tokens per core
HALO = 512
LT = TS + HALO             # 768 local tokens (halo + own)
TL = LT // 128             # 6 local token tiles
OT = TS // 128             # 2 own token tiles
KH = HID // 128            # 20 hidden-dim k-chunks
KA = (NH * HD) // 128      # 16 attn-dim chunks
MIF = INTER // 128         # 80 inter m-chunks (full, per core)
GM = 4                     # m-chunks per down-accumulation group
NGRP = MIF // GM           # 20 groups
PREF = 3                   # stream prefetch depth (m-chunks)
HALF = HD // 2


def _bcast_row(nc, sbuf_tile, dram_t, width):
    a = dram_t.ap()
    nc.sync.dma_start(sbuf_tile[:], bass.AP(
        tensor=a.tensor, offset=a.offset, ap=[[0, 128], [1, width]]))


def _swap_ap(src_ap, half):
    """View [128, 2*half] with halves swapped, as [128, 2, half]."""
    return bass.AP(tensor=src_ap.tensor, offset=src_ap.offset + half,
                   ap=[list(src_ap.ap[0]), [-half, 2], [1, half]])


def build_nc(sim=False):
    nc = bacc.Bacc("TRN2", target_bir_lowering=False, debug=False,
                   enable_asserts=True, num_devices=1 if sim else NC_)

    # ---- inputs (host-prepacked layouts) ----
    x_p = nc.dram_tensor("x_p", [128, TL, HID], BF, kind="ExternalInput")
    x_own = nc.dram_tensor("x_own", [128, OT, HID], F32, kind="ExternalInput")
    pad_p = nc.dram_tensor("pad_p", [128, TL], F32, kind="ExternalInput")
    cq = nc.dram_tensor("cq", [128, OT, HD], BF, kind="ExternalInput")
    sq = nc.dram_tensor("sq", [128, OT, HD], BF, kind="ExternalInput")
    ck = nc.dram_tensor("ck", [128, TL, HD], BF, kind="ExternalInput")
    sk = nc.dram_tensor("sk", [128, TL, HD], BF, kind="ExternalInput")
    wq_p = nc.dram_tensor("wq_p", [128, KH, NH * HD], BF, kind="ExternalInput")
    wk_p = nc.dram_tensor("wk_p", [128, KH, NKV * HD], BF, kind="ExternalInput")
    wv_p = nc.dram_tensor("wv_p", [128, KH, NKV * HD], BF, kind="ExternalInput")
    wo_p = nc.dram_tensor("wo_p", [128, KA, HID], BF, kind="ExternalInput")
    # gate+up interleaved: [kpart, m-chunk, {g,u}, k-chunk, m-col]
    wgu_f = nc.dram_tensor("wgu_f", [128, MIF, 2, KH, 128], BF,
                           kind="ExternalInput")
    # down: [inter-part-within-chunk, m-chunk, hid]
    wd_f = nc.dram_tensor("wd_f", [128, MIF, HID], BF, kind="ExternalInput")
    w1_in = nc.dram_tensor("w1_in", [HID], BF, kind="ExternalInput")
    w1_pa = nc.dram_tensor("w1_pa", [HID], BF, kind="ExternalInput")
    w1_pf = nc.dram_tensor("w1_pf", [HID], BF, kind="ExternalInput")
    w1_po = nc.dram_tensor("w1_po", [HID], F32, kind="ExternalInput")
    out_shard = nc.dram_tensor("out_shard", [TS, HID], F32, kind="ExternalOutput")

    stages = {}
    nc._stage_ids = stages

    def mark(name):
        stages[name] = nc.next_id()

    with tile.TileContext(nc) as tc:
        with (
            tc.tile_pool(name="dram", bufs=1, space="DRAM") as dram,
            tc.tile_pool(name="glob", bufs=1) as glob,
            tc.tile_pool(name="nrm", bufs=3) as nrm,
            tc.tile_pool(name="psP", bufs=1, space="PSUM") as psP,
        ):
            x2_spill = dram.tile([TS, HID], F32)
            ident = glob.tile([128, 128], BF)
            make_identity(nc, ident[:])
            eps_t = glob.tile([128, 1], F32)
            nc.vector.memset(eps_t[:], EPS)
            ones_t = glob.tile([128, 1], BF)
            nc.vector.memset(ones_t[:], 1.0)


            # attention-phase residents: a2 lives through S3, a1 dies at S2 end
            pool_a2 = tc.alloc_tile_pool(name="pa2", bufs=1)
            KT = pool_a2.tile([128, NKV * 2, LT], BF)  # K^T [d, tok]
            QT = pool_a2.tile([128, KA, TS], BF)       # Q^T [d, tok]
            V = [pool_a2.tile([128, NKV * HD], BF, name=f"V{t}")
                 for t in range(TL)]
            sc_pool = tc.alloc_tile_pool(name="sc", bufs=1)
            pool_a = tc.alloc_tile_pool(name="pa1", bufs=1)
            # first two token tiles + in_ln weight: issue these DMAs before
            # the wk/rope loads so S1's first norm chains start ASAP
            xt01 = [pool_a.tile([128, HID], BF, name=f"xt0{t}")
                    for t in range(2)]
            for t in range(2):
                nc.scalar.dma_start(xt01[t][:], x_p.ap()[:, t, :])
            w1_in_b = pool_a.tile([128, HID], BF)
            a_w1 = w1_in.ap()
            nc.gpsimd.dma_start(w1_in_b[:], bass.AP(
                tensor=a_w1.tensor, offset=a_w1.offset, ap=[[0, 128], [1, HID]]))
            # warm the activation-function tables while those DMAs run
            warm = glob.tile([128, 1], F32)
            for af in (AF.Sqrt, AF.Exp, AF.Gelu_apprx_tanh, AF.Copy):
                nc.scalar.activation(warm[:], eps_t[:], af)
            hT = pool_a.tile([128, KH, LT], BF)       # h^T
            wk_sb = pool_a.tile([128, KH, NKV * HD], BF)
            for _i in range(4):
                nc.sync.dma_start(wk_sb[:, :, 256 * _i:256 * (_i + 1)],
                                  wk_p.ap()[:, :, 256 * _i:256 * (_i + 1)])
            cq_sb = pool_a.tile([128, OT, HD], BF)
            sq_sb = pool_a.tile([128, OT, HD], BF)
            ck_sb = pool_a.tile([128, TL, HD], BF)
            sk_sb = pool_a.tile([128, TL, HD], BF)
            nc.sync.dma_start(cq_sb[:], cq.ap())
            nc.sync.dma_start(sq_sb[:], sq.ap())
            nc.sync.dma_start(ck_sb[:], ck.ap())
            nc.sync.dma_start(sk_sb[:], sk.ap())

            # window/causal masks [128 ko, 256 qo] for k-tiles 0,1,4,5
            # valid iff 0 <= (512+qo) - (128*kt+ko) < 512
            masks = pool_a2.tile([128, 4, TS], BF)
            for i, kt in enumerate((0, 1, 4, 5)):
                mk = masks[:, i, :]
                nc.gpsimd.memset(mk, 1.0)
                if kt in (0, 1):
                    # keep where ko + (128*kt - 1) - qo >= 0
                    nc.gpsimd.affine_select(
                        out=mk, in_=mk, compare_op=ALU.is_ge, fill=0.0,
                        base=128 * kt - 1, pattern=[[-1, TS]],
                        channel_multiplier=1)
                else:
                    # keep where qo - ko + (512 - 128*kt) >= 0
                    nc.gpsimd.affine_select(
                        out=mk, in_=mk, compare_op=ALU.is_ge, fill=0.0,
                        base=512 - 128 * kt, pattern=[[1, TS]],
                        channel_multiplier=-1)
            pad_sb = pool_a2.tile([128, TL], F32)
            nc.sync.dma_start(pad_sb[:], pad_p.ap())

            def rinv_from_stats(stats, name):
                mv = nrm.tile([128, 2], F32, tag="nmv", name=f"{name}_mv")
                nc.vector.bn_aggr(out=mv[:], in_=stats[:])
                ms = nrm.tile([128, 1], F32, tag="nms", name=f"{name}_ms")
                nc.vector.scalar_tensor_tensor(ms[:], mv[:, 0:1], mv[:, 0:1],
                                               mv[:, 1:2], op0=ALU.mult,
                                               op1=ALU.add)
                nc.vector.tensor_scalar_add(ms[:], ms[:], EPS)
                rec = nrm.tile([128, 1], F32, tag="nrc", name=f"{name}_rc")
                nc.vector.reciprocal(rec[:], ms[:])
                rinv = nrm.tile([128, 1], F32, tag="nrv", name=f"{name}_rv")
                nc.scalar.activation(rinv[:], rec[:], AF.Sqrt)
                return rinv

            def rmsnorm_rinv(src_ap, d, name, rows=128):
                """rinv[p,1] = 1/sqrt(mean(src^2)+EPS) via bn_stats."""
                nsub = max(1, d // 512)
                stats = nrm.tile([128, nsub, 6], F32, tag="nst", name=f"{name}_st")
                if nsub > 1:
                    view = src_ap.rearrange("p (s f) -> p s f", s=nsub)
                    for i in range(nsub):
                        nc.vector.bn_stats(out=stats[:rows, i, :],
                                           in_=view[:, i, :])
                else:
                    nc.vector.bn_stats(out=stats[:rows, 0, :], in_=src_ap)
                mv = nrm.tile([128, 2], F32, tag="nmv", name=f"{name}_mv")
                nc.vector.bn_aggr(out=mv[:rows], in_=stats[:rows])
                ms = nrm.tile([128, 1], F32, tag="nms", name=f"{name}_ms")
                nc.vector.scalar_tensor_tensor(ms[:rows], mv[:rows, 0:1],
                                               mv[:rows, 0:1], mv[:rows, 1:2],
                                               op0=ALU.mult, op1=ALU.add)
                nc.vector.tensor_scalar_add(ms[:rows], ms[:rows], EPS)
                rec = nrm.tile([128, 1], F32, tag="nrc", name=f"{name}_rc")
                nc.vector.reciprocal(rec[:rows], ms[:rows])
                rinv = nrm.tile([128, 1], F32, tag="nrv", name=f"{name}_rv")
                nc.scalar.activation(rinv[:rows], rec[:rows], AF.Sqrt)
                return rinv

            def rmsnorm_rinv2(src_ap, name):
                """Two per-head rinvs for a [128, 512] psum (2 heads of 256).
                Returns [128, 2] f32 tile."""
                stats = nrm.tile([128, 2, 6], F32, tag="hst", name=f"{name}_st")
                view = src_ap.rearrange("p (s f) -> p s f", s=2)
                for i in range(2):
                    nc.vector.bn_stats(out=stats[:, i, :], in_=view[:, i, :])
                mv = nrm.tile([128, 2, 2], F32, tag="hmv", name=f"{name}_mv")
                for i in range(2):
                    nc.vector.bn_aggr(out=mv[:, i, :], in_=stats[:, i, :])
                ms = nrm.tile([128, 2], F32, tag="hms", name=f"{name}_ms")
                nc.vector.tensor_mul(ms[:], mv[:, :, 0], mv[:, :, 0])
                nc.vector.tensor_add(ms[:], ms[:], mv[:, :, 1])
                nc.vector.tensor_scalar_add(ms[:], ms[:], EPS)
                rec = nrm.tile([128, 2], F32, tag="hrc", name=f"{name}_rc")
                nc.vector.reciprocal(rec[:], ms[:])
                rinv = nrm.tile([128, 2], F32, tag="hrv", name=f"{name}_rv")
                nc.scalar.activation(rinv[:], rec[:], AF.Sqrt)
                return rinv

            def k_group(pool, nch, t):
                ps = psP.tile([128, 512], F32, tag="mm", bufs=6,
                              name=f"psk{nch}_{t}")
                for k in range(KH):
                    nc.tensor.matmul(ps[:], hT[:, k, t * 128:(t + 1) * 128],
                                     wk_sb[:, k, nch * 512:(nch + 1) * 512],
                                     start=(k == 0), stop=(k == KH - 1))
                pcp = pool.tile([128, 512], BF, tag="pcp",
                                name=f"kcp{nch}_{t}", bufs=4)
                nc.scalar.copy(out=pcp[:], in_=ps[:])
                rinv2 = rmsnorm_rinv2(pcp[:], f"kn{nch}_{t}")
                kh = pool.tile([128, 512], BF, tag="kh",
                               name=f"kh{nch}_{t}", bufs=3)
                t2 = pool.tile([128, 512], BF, tag="t2",
                               name=f"t2k{nch}_{t}", bufs=3)
                for hh in range(2):
                    srcp = pcp[:, hh * HD:(hh + 1) * HD]
                    nc.vector.scalar_tensor_tensor(
                        kh[:, hh * HD:(hh + 1) * HD], srcp,
                        rinv2[:, hh:hh + 1], ck_sb[:, t, :],
                        op0=ALU.mult, op1=ALU.mult)
                    nc.vector.scalar_tensor_tensor(
                        t2[:, hh * HD:(hh + 1) * HD]
                        .rearrange("p (a b) -> p a b", a=2),
                        _swap_ap(srcp, HALF), rinv2[:, hh:hh + 1],
                        sk_sb[:, t, :].rearrange("p (a b) -> p a b", a=2),
                        op0=ALU.mult, op1=ALU.mult)
                nc.gpsimd.tensor_add(kh[:], kh[:], t2[:])
                ptr = psP.tile([128, 512], BF, tag="tr", bufs=2,
                               name=f"ktr{nch}_{t}")
                for mm in range(4):
                    nc.tensor.transpose(ptr[:, mm * 128:(mm + 1) * 128],
                                        kh[:, mm * 128:(mm + 1) * 128],
                                        ident[:])
                nc.scalar.copy(
                    out=KT[:, nch * 4:(nch + 1) * 4, t * 128:(t + 1) * 128],
                    in_=ptr[:].rearrange("p (a b) -> p a b", a=4))

            mark('S1')
            # ============ S1: in_ln over 768 local tokens + transpose ======
            with tc.tile_pool(name="s1", bufs=2) as s1:
                for t in range(TL):
                    if t < 2:
                        xt = xt01[t]
                    else:
                        xt = s1.tile([128, HID], BF, tag="xt", name=f"xt{t}",
                                     bufs=2)
                        nc.scalar.dma_start(xt[:], x_p.ap()[:, t, :])
                    rinv = rmsnorm_rinv(xt[:], HID, f"inln{t}")
                    ht = s1.tile([128, HID], BF, tag="ht", name=f"ht{t}", bufs=2)
                    for cch in range(5):
                        sl = slice(cch * 512, (cch + 1) * 512)
                        nc.vector.scalar_tensor_tensor(
                            ht[:, sl], xt[:, sl], rinv[:], w1_in_b[:, sl],
                            op0=ALU.mult, op1=ALU.mult)
                    for g in range(5):   # 4 k-chunks per transpose group
                        ptr = psP.tile([128, 512], BF, tag="tr", bufs=2,
                                       name=f"s1tr{t}_{g}")
                        for kk in range(4):
                            nc.tensor.transpose(
                                ptr[:, kk * 128:(kk + 1) * 128],
                                ht[:, (g * 4 + kk) * 128:(g * 4 + kk + 1) * 128],
                                ident[:])
                        dst = hT[:, g * 4:(g + 1) * 4, t * 128:(t + 1) * 128]
                        src_ = ptr[:].rearrange("p (a b) -> p a b", a=4)
                        nc.scalar.copy(out=dst, in_=src_)

            mark('S2')
            # ====== S2: Q projection, then scores+exp, then V (overlap) =====
            kt_mask = {0: 0, 1: 1, 4: 2, 5: 3}
            PRS = [[None] * TL for _ in range(NH)]
            with tc.tile_pool(name="s2", bufs=2) as s2:
                # --- K: per (nch, t): 2 heads norm+rope, transpose ---
                for nch in range(2):
                    for t in range(TL):
                        k_group(s2, nch, t)

                # --- Q: own tokens only (local tiles 4,5) ---
                for nch in range(4):
                    wqt = s2.tile([128, KH, 512], BF, tag="wst",
                                  name=f"wq{nch}", bufs=2)
                    nc.sync.dma_start(wqt[:], wq_p.ap()[:, :, nch * 512:(nch + 1) * 512])
                    for t in range(OT):
                        lt = 4 + t
                        ps = psP.tile([128, 512], F32, tag="mm", bufs=6,
                                      name=f"psq{nch}_{t}")
                        for k in range(KH):
                            nc.tensor.matmul(ps[:], hT[:, k, lt * 128:(lt + 1) * 128],
                                             wqt[:, k, :], start=(k == 0),
                                             stop=(k == KH - 1))
                        pcp = s2.tile([128, 512], BF, tag="pcp",
                                      name=f"qcp{nch}_{t}", bufs=4)
                        nc.scalar.copy(out=pcp[:], in_=ps[:])
                        rinv2 = rmsnorm_rinv2(pcp[:], f"qn{nch}_{t}")
                        qh = s2.tile([128, 512], BF, tag="qh",
                                     name=f"qh{nch}_{t}", bufs=3)
                        t2 = s2.tile([128, 512], BF, tag="t2",
                                     name=f"t2q{nch}_{t}", bufs=3)
                        for hh in range(2):
                            srcp = pcp[:, hh * HD:(hh + 1) * HD]
                            nc.vector.scalar_tensor_tensor(
                                qh[:, hh * HD:(hh + 1) * HD], srcp,
                                rinv2[:, hh:hh + 1], cq_sb[:, t, :],
                                op0=ALU.mult, op1=ALU.mult)
                            nc.vector.scalar_tensor_tensor(
                                t2[:, hh * HD:(hh + 1) * HD]
                                .rearrange("p (a b) -> p a b", a=2),
                                _swap_ap(srcp, HALF), rinv2[:, hh:hh + 1],
                                sq_sb[:, t, :].rearrange("p (a b) -> p a b", a=2),
                                op0=ALU.mult, op1=ALU.mult)
                        nc.gpsimd.tensor_add(qh[:], qh[:], t2[:])
                        ptr = psP.tile([128, 512], BF, tag="tr", bufs=2,
                                       name=f"qtr{nch}_{t}")
                        for mm in range(4):
                            nc.tensor.transpose(ptr[:, mm * 128:(mm + 1) * 128],
                                                qh[:, mm * 128:(mm + 1) * 128],
                                                ident[:])
                        nc.scalar.copy(
                            out=QT[:, nch * 4:(nch + 1) * 4, t * 128:(t + 1) * 128],
                            in_=ptr[:].rearrange("p (a b) -> p a b", a=4))

                # --- scores + exp + mask for all heads (PE overlaps V next) ---
                for h in range(NH):
                    md = h - (h % 2)      # KT m-tile base for kv head h//2
                    for kt in range(TL):
                        psc = psP.tile([128, TS], F32, tag="mm", bufs=6,
                                       name=f"psc{h}_{kt}")
                        for dh in range(2):
                            nc.tensor.matmul(
                                psc[:], KT[:, md + dh, kt * 128:(kt + 1) * 128],
                                QT[:, 2 * h + dh, :],
                                start=(dh == 0), stop=(dh == 1))
                        pr = sc_pool.tile([128, TS], BF, name=f"pr{h}_{kt}")
                        nc.scalar.activation(pr[:], psc[:], AF.Exp,
                                             scale=1.0 / 16.0)
                        if kt in (0, 1):
                            nc.vector.scalar_tensor_tensor(
                                pr[:], pr[:], pad_sb[:, kt:kt + 1],
                                masks[:, kt_mask[kt], :],
                                op0=ALU.mult, op1=ALU.mult)
                        elif kt in (2, 3):
                            nc.vector.tensor_scalar_mul(pr[:], pr[:],
                                                        pad_sb[:, kt:kt + 1])
                        else:
                            nc.vector.tensor_mul(pr[:], pr[:],
                                                 masks[:, kt_mask[kt], :])
                        PRS[h][kt] = pr

                # --- V projection (PE; exp/mask of scores overlaps) ---
                for nch in range(2):
                    wvt = s2.tile([128, KH, 512], BF, tag="wst",
                                  name=f"wv{nch}", bufs=2)
                    nc.sync.dma_start(wvt[:], wv_p.ap()[:, :, nch * 512:(nch + 1) * 512])
                    for t in range(TL):
                        ps = psP.tile([128, 512], F32, tag="mm", bufs=6,
                                      name=f"psv{nch}_{t}")
                        for k in range(KH):
                            nc.tensor.matmul(ps[:], hT[:, k, t * 128:(t + 1) * 128],
                                             wvt[:, k, :], start=(k == 0),
                                             stop=(k == KH - 1))
                        nc.scalar.copy(out=V[t][:, nch * 512:(nch + 1) * 512],
                                       in_=ps[:])

            mark('S3')
            # ===== S3: attention PV + output (scores already computed) =====
            pool_a.release()   # hT + wk + rope tables
            # MLP weight stream pool (persists through S5; allocated before
            # pool_b so the right-side pool stack pops in LIFO order)
            pool_s = tc.alloc_tile_pool(name="ps5", bufs=1, side="right")
            # wo is streamed in [KA, 512] n-chunks (first preloaded now);
            # MLP weight stream starts prefetching here too (DMA idle).
            pool_b = tc.alloc_tile_pool(name="pb", bufs=1, side="right")
            attnT = pool_b.tile([128, KA, TS], BF)
            wo_t = {}

            def load_wo(n):
                wo_t[n] = pool_b.tile([128, KA, 512], BF, tag="wo",
                                      name=f"wo{n}", bufs=3)
                nc.sync.dma_start(wo_t[n][:], wo_p.ap()[:, :, n * 512:(n + 1) * 512])

            for _n in range(3):
                load_wo(_n)
            # post-attn / pre-ff norm weights: broadcast now so S4's SP queue
            # only carries the remaining wo chunks
            w1_pa_b = pool_b.tile([128, HID], BF)
            w1_pf_b = pool_b.tile([128, HID], BF)
            _bcast_row(nc, w1_pa_b, w1_pa, HID)
            _bcast_row(nc, w1_pf_b, w1_pf, HID)
            wgu_t = {}
            wd_t = {}

            def load_m(m):
                wgu_t[m] = pool_s.tile([128, 2, KH, 128], BF, tag="wgu",
                                       name=f"wgu{m}", bufs=PREF + 1)
                nc.sync.dma_start(wgu_t[m][:], wgu_f.ap()[:, m])
                wd_t[m] = pool_s.tile([128, HID], BF, tag="wd",
                                      name=f"wd{m}", bufs=GM + PREF)
                nc.sync.dma_start(wd_t[m][:], wd_f.ap()[:, m])
            with tc.tile_pool(name="s3", bufs=2) as s3:
                for qt in range(OT):
                  for h in range(NH):
                    prs = PRS[h]
                    if True:
                        po = psP.tile([128, HD + 1], F32, tag="mm", bufs=6,
                                      name=f"po{h}_{qt}")
                        for kt in range(TL):
                            sl = slice(qt * 128, (qt + 1) * 128)
                            nc.tensor.matmul(po[:, 0:HD], prs[kt][:, sl],
                                             V[kt][:, (h // 2) * HD:(h // 2 + 1) * HD],
                                             start=(kt == 0), stop=(kt == TL - 1))
                        for kt in range(TL):
                            sl = slice(qt * 128, (qt + 1) * 128)
                            nc.tensor.matmul(po[:, HD:HD + 1], prs[kt][:, sl],
                                             ones_t[:], start=(kt == 0),
                                             stop=(kt == TL - 1))
                        rec = s3.tile([128, 1], F32, tag="rec",
                                      name=f"rec{h}_{qt}")
                        nc.vector.reciprocal(rec[:], po[:, HD:HD + 1])
                        an = s3.tile([128, HD], BF, tag="an",
                                     name=f"an{h}_{qt}")
                        nc.vector.tensor_scalar_mul(an[:], po[:, 0:HD], rec[:])
                        ptr = psP.tile([128, HD], BF, tag="tr", bufs=2,
                                       name=f"atr{h}_{qt}")
                        for mm in range(2):
                            nc.tensor.transpose(ptr[:, mm * 128:(mm + 1) * 128],
                                                an[:, mm * 128:(mm + 1) * 128],
                                                ident[:])
                        nc.scalar.copy(
                            out=attnT[:, 2 * h:2 * h + 2, qt * 128:(qt + 1) * 128],
                            in_=ptr[:].rearrange("p (a b) -> p a b", a=2))

            mark('S4')
            # ======== S4: wo + post_attn + residual + pre_ff norm ========
            sc_pool.release()   # probs
            pool_a2.release()   # frees KT/QT/V/masks
            # S4-S5 persistent: h2T (pre-ff normed own tokens, transposed)
            pool_c = tc.alloc_tile_pool(name="pc", bufs=1)
            h2T = pool_c.tile([128, KH, TS], BF)
            with tc.tile_pool(name="s4", bufs=2) as s4:
                xos = [s4.tile([128, HID], F32, tag="xo", name=f"xo{t}",
                               bufs=2) for t in range(OT)]
                ao32s = [s4.tile([128, HID], F32, tag="ao32",
                                 name=f"ao32_{t}", bufs=2) for t in range(OT)]
                stats_a = [nrm.tile([128, 5, 6], F32, tag="nst",
                                    name=f"pan{t}_st") for t in range(OT)]
                # n-outer so each wo chunk is loaded once and used for both t
                for n in range(5):
                    wo_n = wo_t.pop(n)
                    for t in range(OT):
                        pw = psP.tile([128, 512], F32, tag="mm", bufs=6,
                                      name=f"pw{t}_{n}")
                        for m in range(KA):
                            nc.tensor.matmul(
                                pw[:], attnT[:, m, t * 128:(t + 1) * 128],
                                wo_n[:, m, :],
                                start=(m == 0), stop=(m == KA - 1))
                        nc.scalar.copy(out=ao32s[t][:, n * 512:(n + 1) * 512],
                                       in_=pw[:])
                        nc.vector.bn_stats(
                            out=stats_a[t][:, n, :],
                            in_=ao32s[t][:, n * 512:(n + 1) * 512])
                    # prefetch AFTER this chunk's reads: with bufs=3, slot
                    # (n+3)%3 == n%3 is the one just consumed above
                    if n + 3 < 5:
                        load_wo(n + 3)
                # residual inputs + MLP stream prefetch: issued after the wo
                # chunks so wo keeps DMA priority during the matmul burst
                for t in range(OT):
                    nc.scalar.dma_start(xos[t][:], x_own.ap()[:, t, :])
                for m in range(PREF):
                    load_m(m)
                for t in range(OT):
                    # stt ops must run on DVE (Pool lacks TensorScalarPtr and
                    # cannot read PSUM); residual adds offload to Pool
                    e0, e1 = nc.vector, nc.gpsimd
                    ao32 = ao32s[t]
                    rinv_a = rinv_from_stats(stats_a[t], f"pan{t}")
                    xo = xos[t]
                    x2 = s4.tile([128, HID], F32, tag="x2", name=f"x2_{t}",
                                 bufs=1)
                    stats_f = nrm.tile([128, 5, 6], F32, tag="nst",
                                       name=f"pff{t}_st")
                    for n in range(5):
                        sl = slice(n * 512, (n + 1) * 512)
                        e0.scalar_tensor_tensor(
                            x2[:, sl], ao32[:, sl], rinv_a[:], w1_pa_b[:, sl],
                            op0=ALU.mult, op1=ALU.mult)
                        e1.tensor_add(x2[:, sl], x2[:, sl], xo[:, sl])
                        nc.vector.bn_stats(out=stats_f[:, n, :], in_=x2[:, sl])
                    nc.scalar.dma_start(x2_spill[t * 128:(t + 1) * 128, :], x2[:])
                    rinv_f = rinv_from_stats(stats_f, f"pff{t}")
                    h2 = s4.tile([128, HID], BF, tag="h2", name=f"h2_{t}",
                                 bufs=1)
                    for g in range(5):
                        sl = slice(g * 512, (g + 1) * 512)
                        e0.scalar_tensor_tensor(
                            h2[:, sl], x2[:, sl], rinv_f[:],
                            w1_pf_b[:, sl], op0=ALU.mult, op1=ALU.mult)
                        ptr = psP.tile([128, 512], BF, tag="tr", bufs=2,
                                       name=f"s4tr{t}_{g}")
                        for kk in range(4):
                            nc.tensor.transpose(
                                ptr[:, kk * 128:(kk + 1) * 128],
                                h2[:, (g * 4 + kk) * 128:(g * 4 + kk + 1) * 128],
                                ident[:])
                        nc.vector.tensor_copy(
                            h2T[:, g * 4:(g + 1) * 4, t * 128:(t + 1) * 128],
                            ptr[:].rearrange("p (a b) -> p a b", a=4))
            pool_b.release()   # attnT + wo

            mark('S5')
            # ===== S5: sequence-parallel MLP, weights streamed in m-chunks ==
            pool_acc = tc.alloc_tile_pool(name="pacc", bufs=1)
            accs = [pool_acc.tile([128, HID], F32, name=f"acc{t}")
                    for t in range(OT)]
            # S6 inputs: fetch during the MLP so the tail chain is short
            w1_po_b = pool_acc.tile([128, HID], F32)
            _bcast_row(nc, w1_po_b, w1_po, HID)
            x2ls = [pool_acc.tile([128, HID], F32, name=f"x2l{t}")
                    for t in range(OT)]
            o32s = [pool_acc.tile([128, HID], F32, name=f"o32_{t}")
                    for t in range(OT)]
            for t in range(OT):
                nc.scalar.dma_start(x2ls[t][:],
                                    x2_spill[t * 128:(t + 1) * 128, :])

            def s6_tile(t):
                # post_ff norm + residual + output for one token tile
                e0, e1 = nc.vector, nc.gpsimd
                rinv_o = rmsnorm_rinv(accs[t][:], HID, f"pon{t}")
                o32 = o32s[t]
                for n in range(4):
                    sl = slice(n * 640, (n + 1) * 640)
                    e0.scalar_tensor_tensor(
                        o32[:, sl], accs[t][:, sl], rinv_o[:],
                        w1_po_b[:, sl], op0=ALU.mult, op1=ALU.mult)
                    e1.tensor_add(o32[:, sl], o32[:, sl], x2ls[t][:, sl])
                    nc.scalar.dma_start(
                        out_shard.ap()[t * 128:(t + 1) * 128, sl],
                        o32[:, sl])

            with tc.tile_pool(name="s5", bufs=2) as s5:
                actT_t = {}

                def down_group(g0, m, tt):
                    first = (g0 == 0)
                    for ng in ((0, 1), (2, 3), (4,)):
                        pds = {n: psP.tile([128, 512], F32, tag="mm",
                                           bufs=6, name=f"pd{m}_{tt}_{n}")
                               for n in ng}
                        for j in range(GM):
                            mj = g0 + j
                            for n in ng:
                                nc.tensor.matmul(
                                    pds[n][:],
                                    actT_t[mj][:, tt * 128:(tt + 1) * 128],
                                    wd_t[mj][:, n * 512:(n + 1) * 512],
                                    start=(j == 0), stop=(j == GM - 1))
                        for n in ng:
                            # GPSIMD cannot read PSUM: copies/adds that touch
                            # pds must run on Act (copy) or DVE (add)
                            dst = accs[tt][:, n * 512:(n + 1) * 512]
                            if first:
                                if n % 2 == 0:
                                    nc.vector.tensor_copy(dst, pds[n][:])
                                else:
                                    nc.scalar.copy(out=dst, in_=pds[n][:])
                            else:
                                nc.vector.tensor_add(dst, dst, pds[n][:])

                for m in range(MIF):
                    if m + PREF < MIF:
                        load_m(m + PREF)
                    wgu = wgu_t.pop(m)
                    pg = psP.tile([128, TS], F32, tag="mm", bufs=6,
                                  name=f"pg{m}")
                    pu = psP.tile([128, TS], F32, tag="mm", bufs=6,
                                  name=f"pu{m}")
                    gsc = s5.tile([128, TS], F32, tag="gsc",
                                  name=f"gsc{m}", bufs=3)
                    at = s5.tile([128, TS], BF, tag="act",
                                 name=f"act{m}", bufs=GM + 2)
                    if m < 8:
                        # per-token-tile so tile-0 work overlaps S4's t=1 tail
                        for tt in range(OT):
                            sl = slice(tt * 128, (tt + 1) * 128)
                            for k in range(KH):
                                nc.tensor.matmul(pg[:, sl], wgu[:, 0, k, :],
                                                 h2T[:, k, sl],
                                                 start=(k == 0),
                                                 stop=(k == KH - 1))
                            for k in range(KH):
                                nc.tensor.matmul(pu[:, sl], wgu[:, 1, k, :],
                                                 h2T[:, k, sl],
                                                 start=(k == 0),
                                                 stop=(k == KH - 1))
                            nc.scalar.activation(gsc[:, sl], pg[:, sl],
                                                 AF.Gelu_apprx_tanh)
                            nc.vector.tensor_mul(at[:, sl], gsc[:, sl],
                                                 pu[:, sl])
                    else:
                        for k in range(KH):
                            nc.tensor.matmul(pg[:], wgu[:, 0, k, :],
                                             h2T[:, k, :],
                                             start=(k == 0), stop=(k == KH - 1))
                        for k in range(KH):
                            nc.tensor.matmul(pu[:], wgu[:, 1, k, :],
                                             h2T[:, k, :],
                                             start=(k == 0), stop=(k == KH - 1))
                        nc.scalar.activation(gsc[:], pg[:],
                                             AF.Gelu_apprx_tanh)
                        nc.vector.tensor_mul(at[:], gsc[:], pu[:])
                    actT_t[m] = at
                    if m % GM != GM - 1:
                        continue
                    # ---- down for this group of GM m-chunks ----
                    g0 = m - GM + 1
                    if m == MIF - 1:
                        # last group: finish each token tile's output inline
                        down_group(g0, m, 0)
                        s6_tile(0)
                        down_group(g0, m, 1)
                        s6_tile(1)
                    else:
                        for tt in range(OT):
                            down_group(g0, m, tt)
                    for j in range(GM):
                        del actT_t[g0 + j], wd_t[g0 + j]
            pool_s.release()

            mark('S6')
            pool_acc.release()
            pool_c.release()

    nc.compile()
    return nc


_NC_CACHE = None


def _get_nc():
    global _NC_CACHE
    if _NC_CACHE is None:
        _NC_CACHE = build_nc()
    return _NC_CACHE


def make_in_maps(hidden_states, position_ids, wq, wk, wv, wo, q_ln_w, k_ln_w,
                 in_ln_w, post_attn_ln_w, pre_ff_ln_w, post_ff_ln_w,
                 w_gate, w_up, w_down):
    bf16 = ml_dtypes.bfloat16
    f32 = np.float32
    x = np.asarray(hidden_states, f32).reshape(S, HID)
    pos = np.asarray(position_ids).reshape(S).astype(np.float64)

    inv_freq = 1.0 / (BASE ** (np.arange(0, HD, 2, dtype=np.float64) / HD))
    w1q = 1.0 + np.asarray(q_ln_w, f32)
    w1k = 1.0 + np.asarray(k_ln_w, f32)

    def rope_tabs(p, w1):
        emb = np.concatenate([p[:, None] * inv_freq[None, :]] * 2, axis=1)
        cos = np.cos(emb).astype(f32)
        sin = np.sin(emb).astype(f32)
        w1sw = np.concatenate([w1[HALF:], w1[:HALF]])
        sgn = np.concatenate([-np.ones(HALF, f32), np.ones(HALF, f32)])
        n = len(p)
        c = (cos * w1[None, :]).astype(bf16).reshape(n // 128, 128, HD)
        s_ = (sin * (w1sw * sgn)[None, :]).astype(bf16).reshape(n // 128, 128, HD)
        return (np.ascontiguousarray(c.transpose(1, 0, 2)),
                np.ascontiguousarray(s_.transpose(1, 0, 2)))

    def pack(w, kt, n):
        return np.ascontiguousarray(
            np.asarray(w, f32).reshape(kt, 128, n).transpose(1, 0, 2)).astype(bf16)

    # gate+up interleaved [128, MIF, 2, KH, 128]
    def pack_gu(w):
        # [HID, INTER] -> [KH, 128, MIF, 128] -> [128, MIF, KH, 128]
        return np.asarray(w, f32).reshape(KH, 128, MIF, 128).transpose(1, 2, 0, 3)

    wgu = np.stack([pack_gu(w_gate), pack_gu(w_up)], axis=2)  # [128,MIF,2,KH,128]
    wd_pk = np.ascontiguousarray(
        np.asarray(w_down, f32).reshape(MIF, 128, HID).transpose(1, 0, 2))

    common = {
        "wq_p": pack(wq, KH, NH * HD),
        "wk_p": pack(wk, KH, NKV * HD),
        "wv_p": pack(wv, KH, NKV * HD),
        "wo_p": pack(wo, KA, HID),
        "wgu_f": np.ascontiguousarray(wgu).astype(bf16),
        "wd_f": wd_pk.astype(bf16),
        "w1_in": (1.0 + np.asarray(in_ln_w, f32)).astype(bf16),
        "w1_pa": (1.0 + np.asarray(post_attn_ln_w, f32)).astype(bf16),
        "w1_pf": (1.0 + np.asarray(pre_ff_ln_w, f32)).astype(bf16),
        "w1_po": 1.0 + np.asarray(post_ff_ln_w, f32),
    }
    in_maps = []
    for c in range(NC_):
        lo = c * TS - HALO
        xh = np.zeros((LT, HID), f32)
        src_lo = max(0, lo)
        xh[src_lo - lo:] = x[src_lo:lo + LT]
        x_pk = np.ascontiguousarray(
            xh.reshape(TL, 128, HID).transpose(1, 0, 2)).astype(bf16)
        x_ow = np.ascontiguousarray(
            x[c * TS:(c + 1) * TS].reshape(OT, 128, HID).transpose(1, 0, 2))
        padv = (np.arange(lo, lo + LT) >= 0).astype(f32)
        pad_pk = np.ascontiguousarray(padv.reshape(TL, 128).T)
        kpos = np.where(np.arange(lo, lo + LT) >= 0,
                        pos[np.clip(np.arange(lo, lo + LT), 0, S - 1)], 0.0)
        qpos = pos[c * TS:(c + 1) * TS]
        ckw, skw = rope_tabs(kpos, w1k)
        cqw, sqw = rope_tabs(qpos, w1q)
        in_maps.append({
            "x_p": x_pk,
            "x_own": x_ow,
            "pad_p": pad_pk,
            "cq": cqw, "sq": sqw, "ck": ckw, "sk": skw,
            **common,
        })
    return in_maps


def kernel(**inputs):
    in_maps = make_in_maps(**inputs)
    nc = _get_nc()
    res = run_bass_kernel_spmd(nc, in_maps, core_ids=list(range(NC_)))
    out = np.concatenate([res.results[c]["out_shard"] for c in range(NC_)], axis=0)
    return out.reshape(1, S, HID).astype(np.float32)
